# revision 1
# baseline (speedup 1.0000x reference)
"""Trainium2 Bass kernel for nn_MultiHeadAttention_83056077570808.

GQA multi-head attention (32 q heads, 8 kv heads, d_head=128, T=2048,
D=4096) with RoPE, tanh soft-capping at 30, causal mask, fp32 reference.

Sharding: tensor-parallel over heads across 8 cores. Core c owns kv head c
and q heads 4c..4c+3: Wq/Wk/Wv column-sharded, Wo row-sharded; activations
replicated. Each core computes a partial output (its heads' contribution
through its Wo rows); the host sums the 8 partials.

Per-core pipeline (all layouts chosen so the matmul contraction dim is the
partition dim):
  - host supplies query/key/value pre-transposed as X^T [D, T] in bf16
  - K/V proj: kT/vT [dk, T] = Wk/Wv-tile.T @ X^T   (bf16 matmuls, fp32 acc)
  - RoPE on kT and qT via a 128x128 rotation matmul + cos/sin elementwise
    (fp32 math, f32r result so QK keeps ~tf32 logit precision)
  - V transposed on-PE to V [T, dk] bf16, augmented with a ones column so
    the PV matmul computes the softmax denominator for free
  - per t-chunk of 512: Q proj + RoPE, then per q head:
      S^T[Tt, t] = kT_rope-tile.T @ qT_rope      (f32r, N=512)
      tanh in-place in PSUM, exp to bf16 SBUF (both ACT scales fused);
      causal mask applied only on the diagonal tile group
      attn[t, 0:129] = P^T-slice.T @ [V | ones]  (bf16, accumulated over
      T tiles; col 128 = denominator)
      normalize by 1/denom (per-partition scalar), PE-transpose to
      attnT [dk, t] bf16 (kept in SBUF)
    then O proj for the chunk: out[t, :] += attnT-tile.T @ Wo-tile (bf16)

No max-subtraction is needed in the softmax: soft-capping bounds logits to
[-30, 30], so exp() stays in fp32 range (and bf16 dynamic range).

PSUM bank rule honored in the PV accumulation: start=True clears
has_written for the WHOLE bank, and two s-chains share each bank, so only
the bank's first chain issues start=True; the sibling chain's first write
lands on cleared bits and overwrites.
"""

import os
import sys

for _p in ("/opt/trn_rl_repo", os.path.expanduser("~/.axon_site/_ro/trn_rl_repo")):
    if os.path.isdir(_p) and _p not in sys.path:
        sys.path.insert(0, _p)

import numpy as np
import ml_dtypes

import concourse.bass as bass
import concourse.tile as tile
from concourse import bacc, mybir
from concourse.bass_utils import run_bass_kernel_spmd

F32 = mybir.dt.float32
F32R = mybir.dt.float32r
BF16 = mybir.dt.bfloat16

D_MODEL = 4096
KEY_SIZE = 128
NUM_Q_HEADS = 32
NUM_KV_HEADS = 8
N_CORES = 8
NH = NUM_Q_HEADS // NUM_KV_HEADS  # q heads per core = 4
ATTN_MULT = 0.08838834764831845
CAP = 30.0

Tanh = mybir.ActivationFunctionType.Tanh
Exp = mybir.ActivationFunctionType.Exp

# Projection / O-proj operand dtype: bf16 (fast, ~4e-3 rel err) or f32r
# (safe, ~2e-4). QK always runs f32r for logit precision.
PROJ_DT = os.environ.get("MHA_PROJ_DT", "bf16")


def build_nc(T: int, causal: bool, proj_dt: str = PROJ_DT):
    """Emit the Bass program for one core (SPMD: all cores run this).

    Tile builds a STATIC per-engine schedule in (priority = emission)
    order, so overlap must be engineered in the emission order itself:
    Q-proj chains for chunk tc+1 and O-proj chains for chunk tc-1 are
    emitted as "fillers" woven between attention groups of chunk tc,
    keeping PE busy while the scalar engine runs the tanh/exp chain.
    """
    skips = set(os.environ.get("MHA_SKIP", "").split(","))
    D = D_MODEL
    TC = 512                 # t-chunk width for attention
    NTC = T // TC            # t-chunks
    NTT = T // 128           # T tiles (key side)
    NDT = D // 128           # contraction tiles over d_model = 32
    JW = NH * KEY_SIZE       # per-core q/o width = 512
    GW = 2                   # T tiles per QK group (2 PSUM banks)
    PDT = BF16 if proj_dt == "bf16" else F32R

    nc = bacc.Bacc(None, target_bir_lowering=False)

    xq = nc.dram_tensor("xq", [D, T], PDT, kind="ExternalInput")
    xk = nc.dram_tensor("xk", [D, T], PDT, kind="ExternalInput")
    xv = nc.dram_tensor("xv", [D, T], PDT, kind="ExternalInput")
    wq = nc.dram_tensor("wq", [D, JW], PDT, kind="ExternalInput")
    wk = nc.dram_tensor("wk", [D, KEY_SIZE], PDT, kind="ExternalInput")
    wv = nc.dram_tensor("wv", [D, KEY_SIZE], PDT, kind="ExternalInput")
    wo = nc.dram_tensor("wo", [JW, D], PDT, kind="ExternalInput")
    cosd = nc.dram_tensor("cosT", [128, T], F32, kind="ExternalInput")
    sind = nc.dram_tensor("sinT", [128, T], F32, kind="ExternalInput")
    rotd = nc.dram_tensor("rot", [128, 128], F32R, kind="ExternalInput")
    identbd = nc.dram_tensor("identb", [128, 128], BF16, kind="ExternalInput")
    maskdd = nc.dram_tensor("maskd", [128, 4 * TC], BF16, kind="ExternalInput")
    vbgd = nc.dram_tensor("vbg", [128, NTT, 4], BF16, kind="ExternalInput")
    outd = nc.dram_tensor("out", [T, D], BF16, kind="ExternalOutput")

    with tile.TileContext(nc) as tc:
        with (
            tc.tile_pool(name="const", bufs=1) as constp,
            tc.tile_pool(name="persist", bufs=1) as persist,
            tc.tile_pool(name="qkps", bufs=2, space="PSUM") as qkps,
            tc.tile_pool(name="pvps", bufs=1, space="PSUM") as pvps,
            tc.tile_pool(name="mmps", bufs=2, space="PSUM") as mmps,
        ):
            rot_sb = constp.tile([128, 128], F32R)
            identb_sb = constp.tile([128, 128], BF16)
            cos_sb = constp.tile([128, T], F32)
            sin_sb = constp.tile([128, T], F32)
            mask_sb = constp.tile([128, 4, TC], BF16)
            nc.sync.dma_start(out=rot_sb, in_=rotd[:])
            nc.sync.dma_start(out=identb_sb, in_=identbd[:])
            nc.sync.dma_start(out=cos_sb, in_=cosd[:])
            nc.sync.dma_start(out=sin_sb, in_=sind[:])
            nc.sync.dma_start(
                out=mask_sb, in_=maskdd.rearrange("k (b t) -> k b t", b=4)
            )

            kT_rope = persist.tile([128, T], F32R)
            vaug = persist.tile([128, NTT, 132], BF16)
            nc.sync.dma_start(out=vaug[:, :, 128:132], in_=vbgd[:])

            wpool = tc.alloc_tile_pool(name="wpool", bufs=1)
            wq_sb = wpool.tile([128, NDT, JW], PDT)
            nc.sync.dma_start(out=wq_sb, in_=wq.rearrange("(n k) j -> k n j", k=128))
            wo_sb = wpool.tile([128, NH, D], PDT)

            qpool = tc.alloc_tile_pool(name="qpool", bufs=int(os.environ.get("MHA_QPBUFS","2")))
            paslab = tc.alloc_tile_pool(name="paslab", bufs=2)
            ropepool = tc.alloc_tile_pool(name="ropetmp", bufs=int(os.environ.get("MHA_RTBUFS","4")))

            def load_slabs(t0):
                slabs = []
                for dh in range(2):
                    slab = paslab.tile(
                        [128, 16, TC], PDT, tag="qslab", name="qslab"
                    )
                    nc.sync.dma_start(
                        out=slab,
                        in_=xq[
                            dh * 2048 : (dh + 1) * 2048, t0 : t0 + TC
                        ].rearrange("(n k) t -> k n t", k=128),
                    )
                    slabs.append(slab)
                return slabs

            def rope(dst, src, t0, tw):
                """dst[128, tw] (f32r) = RoPE(src[128, tw]) at positions t0.."""
                rp = mmps.tile([128, 512], F32, tag="mm", name="rope_ps")
                nc.tensor.matmul(rp[:, :tw], rot_sb, src, start=True, stop=True)
                t1 = ropepool.tile([128, 512], F32, tag="rt", name="rope_t1")
                nc.gpsimd.tensor_mul(
                    t1[:, :tw], src.bitcast(F32), cos_sb[:, t0 : t0 + tw]
                )
                t2 = ropepool.tile([128, 512], F32, tag="rt", name="rope_t2")
                nc.vector.tensor_mul(t2[:, :tw], rp[:, :tw], sin_sb[:, t0 : t0 + tw])
                nc.vector.tensor_add(dst, t1[:, :tw], t2[:, :tw])

            def qproj_chain(slabs, qraw, jh, psum_pool=None, psum_tag="mm"):
                if psum_pool is None:
                    ps = mmps.tile([128, 512], F32, tag="mm", name="q_ps")
                else:
                    ps = psum_pool.tile(
                        [128, 2, 512], F32, tag=psum_tag, name="q_ps"
                    )[:, 0, :]
                for dh in range(2):
                    for i in range(16):
                        nc.tensor.matmul(
                            ps,
                            wq_sb[:, dh * 16 + i, jh * 128 : (jh + 1) * 128],
                            slabs[dh][:, i, :],
                            start=(dh == 0 and i == 0),
                            stop=(dh == 1 and i == 15),
                        )
                nc.vector.tensor_copy(qraw[:, jh, :], ps)

            slabs0 = load_slabs(0)
            qraw0 = qpool.tile([128, NH, TC], F32R, tag="qraw", name="qraw")

            # ---------------- phase 1: K/V proj + RoPE-k + V transpose,
            # with Q-proj(tc0) chains interleaved into the K stream.
            with (
                tc.tile_pool(name="ph1", bufs=1) as ph1,
                tc.tile_pool(name="ph1slab", bufs=int(os.environ.get("MHA_KVSLABS","6"))) as ph1slab,
            ):
                wk_sb = ph1.tile([128, NDT, 128], PDT)
                wv_sb = ph1.tile([128, NDT, 128], PDT)
                nc.sync.dma_start(
                    out=wk_sb, in_=wk.rearrange("(n k) j -> k n j", k=128)
                )
                nc.sync.dma_start(
                    out=wv_sb, in_=wv.rearrange("(n k) j -> k n j", k=128)
                )
                kproj = ph1.tile([128, T], F32R)
                vproj = ph1.tile([128, T], BF16)

                def kv_chunk(w_sb, xsrc, dest, tch):
                    ps = mmps.tile([128, 512], F32, tag="mm", name="kv_ps")
                    for dh in range(4):
                        slab = ph1slab.tile(
                            [128, 8, 256], PDT, tag="slab", name="kvslab"
                        )
                        nc.sync.dma_start(
                            out=slab,
                            in_=xsrc[
                                dh * 1024 : (dh + 1) * 1024,
                                tch * 256 : (tch + 1) * 256,
                            ].rearrange("(n k) t -> k n t", k=128),
                        )
                        for i in range(8):
                            nc.tensor.matmul(
                                ps[:, :256],
                                w_sb[:, dh * 8 + i, :],
                                slab[:, i, :],
                                start=(dh == 0 and i == 0),
                                stop=(dh == 3 and i == 7),
                            )
                    nc.scalar.copy(
                        out=dest[:, tch * 256 : (tch + 1) * 256], in_=ps[:, :256]
                    )

                with nc.named_scope("kproj"):
                    qdone = 0
                    for tch in range(T // 256):
                        kv_chunk(wk_sb, xk, kproj, tch)
                        if tch % 2 == 1:
                            ch = tch // 2
                            rope(
                                kT_rope[:, ch * TC : (ch + 1) * TC],
                                kproj[:, ch * TC : (ch + 1) * TC],
                                ch * TC,
                                TC,
                            )
                            if qdone < NH:
                                qproj_chain(
                                    slabs0, qraw0, qdone,
                                    psum_pool=qkps, psum_tag="qk",
                                )
                                qdone += 1
                    while qdone < NH:
                        qproj_chain(
                            slabs0, qraw0, qdone, psum_pool=qkps, psum_tag="qk"
                        )
                        qdone += 1

                with nc.named_scope("vproj"):
                    for tch in range(T // 256):
                        kv_chunk(wv_sb, xv, vproj, tch)
                        for b in (2 * tch, 2 * tch + 1):
                            tp = mmps.tile(
                                [128, 512], BF16, tag="mm", name="vtr_ps"
                            )
                            nc.tensor.transpose(
                                tp[:, :128],
                                vproj[:, b * 128 : (b + 1) * 128],
                                identb_sb,
                            )
                            nc.vector.tensor_copy(vaug[:, b, 0:128], tp[:, :128])

            # ---------------- main loop over t-chunks with filler weaving
            # (wo loads here, off phase 1's DMA critical path; first needed
            # by O-proj of tc0 during attention of tc1)
            nc.sync.dma_start(out=wo_sb, in_=wo.rearrange("(n k) d -> k n d", k=128))
            with tc.tile_pool(name="pa", bufs=1) as pa:

                def make_oproj_fillers(attnT, t0, evac_alt):
                    fillers = []
                    for s4 in range(4):
                        for nch in range(D // 512):
                            def f(s4=s4, nch=nch):
                                with nc.named_scope("oproj"):
                                    ps = mmps.tile(
                                        [128, 512], F32, tag="mm", name="o_ps"
                                    )
                                    for jh in range(NH):
                                        nc.tensor.matmul(
                                            ps,
                                            attnT[
                                                :, jh, s4 * 128 : (s4 + 1) * 128
                                            ],
                                            wo_sb[
                                                :, jh, nch * 512 : (nch + 1) * 512
                                            ],
                                            start=(jh == 0),
                                            stop=(jh == NH - 1),
                                        )
                                    osb = pa.tile(
                                        [128, 512], BF16, tag="osb", bufs=int(os.environ.get("MHA_OSBUFS","5")),
                                        name="osb",
                                    )
                                    if evac_alt and nch % 2 == 1:
                                        nc.scalar.copy(out=osb, in_=ps)
                                    else:
                                        nc.vector.tensor_copy(osb, ps)
                                    nc.sync.dma_start(
                                        out=outd[
                                            t0 + s4 * 128 : t0 + (s4 + 1) * 128,
                                            nch * 512 : (nch + 1) * 512,
                                        ],
                                        in_=osb,
                                    )
                            fillers.append(f)
                    return fillers

                qraw_cur = qraw0
                prev_attnT = None
                prev_t0 = 0
                slabs_next = load_slabs(TC) if NTC > 1 else None
                for tcx in range(NTC):
                    t0 = tcx * TC
                    with nc.named_scope("ropeq"):
                        qrope = pa.tile([128, NH, TC], F32R, tag="qrope", bufs=2)
                        for jh in range(NH):
                            rope(qrope[:, jh, :], qraw_cur[:, jh, :], t0, TC)

                    fillers = []
                    if tcx + 1 < NTC:
                        qraw_next = qpool.tile(
                            [128, NH, TC], F32R, tag="qraw", name="qraw"
                        )
                        slabs_cap = slabs_next
                        for jh in range(NH):
                            fillers.append(
                                lambda jh=jh: qproj_chain(
                                    slabs_cap, qraw_next, jh
                                )
                            )
                        if tcx + 2 < NTC:
                            def prefetch(t0n=(tcx + 2) * TC):
                                nonlocal slabs_next
                                slabs_next = load_slabs(t0n)
                            fillers.append(prefetch)
                    if prev_attnT is not None:
                        fillers.extend(
                            make_oproj_fillers(prev_attnT, prev_t0, False)
                        )

                    nt_valid = 4 * (tcx + 1) if causal else NTT
                    ngroups = nt_valid // GW
                    attnT = pa.tile([128, NH, TC], BF16, tag="attnT", bufs=int(os.environ.get("MHA_ATBUFS","2")))
                    if "attn" not in skips:
                        total_groups = NH * ngroups
                        gidx = 0
                        n_fill0 = len(fillers)
                        n_popped = 0
                        for h in range(NH):
                            with nc.named_scope("attn"):
                                pv = pvps.tile(
                                    [128, 4, 256], F32, tag="pv", name="pv_ps"
                                )
                                for gg in range(ngroups):
                                    qk = qkps.tile(
                                        [128, GW, 512], F32, tag="qk", name="qk_ps"
                                    )
                                    for b in range(GW):
                                        Tt = GW * gg + b
                                        nc.tensor.matmul(
                                            qk[:, b, :],
                                            kT_rope[:, Tt * 128 : (Tt + 1) * 128],
                                            qrope[:, h, :],
                                            start=True,
                                            stop=True,
                                        )
                                    # tanh in place in PSUM, then exp to bf16
                                    # SBUF; soft-capping scales fused into ACT.
                                    nc.scalar.activation(
                                        out=qk, in_=qk, func=Tanh,
                                        scale=ATTN_MULT / CAP,
                                    )
                                    pt = pa.tile(
                                        [128, GW, TC], BF16, tag="pt", bufs=int(os.environ.get("MHA_PTBUFS","3")),
                                        name="ptile",
                                    )
                                    nc.scalar.activation(
                                        out=pt, in_=qk, func=Exp, scale=CAP
                                    )
                                    rel = GW * gg - 4 * tcx
                                    if causal and 0 <= rel < 4:
                                        nc.gpsimd.tensor_mul(
                                            pt, pt, mask_sb[:, rel : rel + GW, :]
                                        )
                                    for s4 in range(4):
                                        for b in range(GW):
                                            Tt = GW * gg + b
                                            nc.tensor.matmul(
                                                pv[:, s4, 0:129],
                                                pt[:, b, s4 * 128 : (s4 + 1) * 128],
                                                vaug[:, Tt, 0:129],
                                                start=(
                                                    gg == 0 and b == 0
                                                    and s4 % 2 == 0
                                                ),
                                                stop=(
                                                    gg == ngroups - 1
                                                    and b == GW - 1
                                                ),
                                                skip_group_check=True,
                                            )
                                    # weave fillers so PE stays busy under ACT
                                    gidx += 1
                                    target = min(
                                        n_fill0,
                                        n_fill0 * gidx * int(os.environ.get("MHA_FPACE", "4")) // (total_groups * 4),
                                    )
                                    while fillers and n_popped < target:
                                        fillers.pop(0)()
                                        n_popped += 1
                            with nc.named_scope("attn_fin"):
                                for s4 in range(4):
                                    rc = pa.tile(
                                        [128, 1], F32, tag="rc", bufs=4, name="rc"
                                    )
                                    nc.vector.reciprocal(rc, pv[:, s4, 128:129])
                                    an = pa.tile(
                                        [128, 128], BF16, tag="an", bufs=int(os.environ.get("MHA_ANBUFS","2")),
                                        name="an",
                                    )
                                    nc.vector.tensor_scalar_mul(
                                        an, pv[:, s4, 0:128], rc
                                    )
                                    tp = mmps.tile(
                                        [128, 512], BF16, tag="mm", name="atr"
                                    )
                                    nc.tensor.transpose(
                                        tp[:, :128], an, identb_sb
                                    )
                                    nc.vector.tensor_copy(
                                        attnT[:, h, s4 * 128 : (s4 + 1) * 128],
                                        tp[:, :128],
                                    )
                    for f in fillers:
                        f()
                    if tcx + 1 < NTC:
                        qraw_cur = qraw_next
                    prev_attnT, prev_t0 = attnT, t0

                # tail: O proj of the last chunk
                for f in make_oproj_fillers(prev_attnT, prev_t0, True):
                    f()

            ropepool.release()
            paslab.release()
            qpool.release()
            wpool.release()

    nc.compile()
    return nc


def _host_constants(T: int):
    d = KEY_SIZE
    inv_freq = 1.0 / (10000.0 ** (np.arange(0, d, 2, dtype=np.float64) / d))  # [64]
    pos = np.arange(T, dtype=np.float64)
    phase_half = pos[None, :] * inv_freq[:, None]  # [64, T]
    phase = np.concatenate([phase_half, phase_half], axis=0)  # [128, T] (tiled)
    cosT = np.cos(phase).astype(np.float32)
    sinT = np.sin(phase).astype(np.float32)

    R = np.zeros((128, 128), dtype=np.float32)
    R[:64, 64:] = -np.eye(64, dtype=np.float32)
    R[64:, :64] = np.eye(64, dtype=np.float32)
    rot = np.ascontiguousarray(R.T)

    ident = np.eye(128, dtype=np.float32)

    TC = 512
    tl = np.arange(TC)
    Tl = np.arange(128)
    maskd = np.zeros((128, 4, TC), dtype=np.float32)
    for b in range(4):
        maskd[:, b, :] = (128 * b + Tl[:, None]) <= tl[None, :]
    maskd = maskd.reshape(128, 4 * TC).astype(ml_dtypes.bfloat16)

    NTT = T // 128
    vbg = np.zeros((128, NTT, 4), dtype=ml_dtypes.bfloat16)
    vbg[:, :, 0] = 1.0
    return cosT, sinT, rot, ident, maskd, vbg


_NC_CACHE: dict = {}
LAST_RESULT = None
_LAST_IN_MAPS = None


def kernel(query, key, value, mask, Wq, Wk, Wv, Wo):
    global LAST_RESULT, _LAST_IN_MAPS
    query = np.asarray(query)
    key = np.asarray(key)
    value = np.asarray(value)
    mask = np.asarray(mask)
    Wq = np.asarray(Wq, dtype=np.float32)
    Wk = np.asarray(Wk, dtype=np.float32)
    Wv = np.asarray(Wv, dtype=np.float32)
    Wo = np.asarray(Wo, dtype=np.float32)

    b, T, D = query.shape
    assert b == 1 and D == D_MODEL, (b, D)

    m2 = np.asarray(mask).reshape(T, T).astype(bool)
    if np.array_equal(m2, np.tril(np.ones((T, T), dtype=bool))):
        causal = True
    elif m2.all():
        causal = False
    else:
        raise ValueError("unsupported mask pattern (expected causal or full)")

    kkey = (T, causal, PROJ_DT)
    if kkey not in _NC_CACHE:
        _NC_CACHE[kkey] = build_nc(T, causal)
    nc = _NC_CACHE[kkey]

    pnp = ml_dtypes.bfloat16 if PROJ_DT == "bf16" else np.float32
    xq = np.ascontiguousarray(query[0].T).astype(pnp)  # [D, T]
    xk = np.ascontiguousarray(key[0].T).astype(pnp)
    xv = np.ascontiguousarray(value[0].T).astype(pnp)
    cosT, sinT, rot, ident, maskd, vbg = _host_constants(T)

    JW = NH * KEY_SIZE
    in_maps = []
    for c in range(N_CORES):
        in_maps.append(
            {
                "xq": xq,
                "xk": xk,
                "xv": xv,
                "wq": np.ascontiguousarray(Wq[:, c * JW : (c + 1) * JW]).astype(pnp),
                "wk": np.ascontiguousarray(
                    Wk[:, c * KEY_SIZE : (c + 1) * KEY_SIZE]
                ).astype(pnp),
                "wv": np.ascontiguousarray(
                    Wv[:, c * KEY_SIZE : (c + 1) * KEY_SIZE]
                ).astype(pnp),
                "wo": np.ascontiguousarray(Wo[c * JW : (c + 1) * JW, :]).astype(pnp),
                "cosT": cosT,
                "sinT": sinT,
                "rot": rot,
                "identb": ident.astype(ml_dtypes.bfloat16),
                "maskd": maskd,
                "vbg": vbg,
            }
        )

    _LAST_IN_MAPS = in_maps
    trace = os.environ.get("MHA_TRACE") == "1"
    res = run_bass_kernel_spmd(nc, in_maps, list(range(N_CORES)), trace=trace)
    LAST_RESULT = res

    out = np.zeros((T, D), dtype=np.float64)
    for c in range(N_CORES):
        out += res.results[c]["out"].astype(np.float64)
    return out.astype(np.float32).reshape(1, T, D)



# revision 20
# speedup vs baseline: 1.1139x; 1.1139x over previous
"""Trainium2 Bass kernel for nn_MultiHeadAttention_83056077570808.

GQA multi-head attention (32 q heads, 8 kv heads, d_head=128, T=2048,
D=4096) with RoPE, tanh soft-capping at 30, causal mask, fp32 reference.

Sharding: tensor-parallel over heads across 8 cores. Core c owns kv head c
and q heads 4c..4c+3: Wq/Wk/Wv column-sharded, Wo row-sharded; activations
replicated. Each core computes a partial output (its heads' contribution
through its Wo rows); the host sums the 8 partials.

Fully streamed schedule: causality means attention chunk tcx only needs
K/V tiles 0..4*tcx+3, so K/V/Q projections for chunk tcx+1 run *during*
attention of chunk tcx as filler work woven between QK groups (covering
the ACT-engine tanh/exp latency); O-proj of chunk tcx-1 likewise. DMA is
spread across the whole timeline instead of front-loaded. Fillers are
paced by an explicit cost model (popping too fast blocks the in-order PE
stream on un-arrived slab DMAs; too slow starves PE under ACT).

Causal diagonal trim: for key tile Tt in the diagonal block of chunk tcx
(rel = Tt-4*tcx in 0..3), query columns < 128*rel are entirely masked, so
QK / tanh / exp are column-trimmed, only the [128,128] diagonal block is
tri-masked (Pool), and PV skips s4-blocks with s4 < rel.

All matmuls are bf16; PSUM accumulation fp32; rope arithmetic fp32.

PSUM bank rule in the PV accumulation: start=True clears has_written for
the WHOLE bank and two s-chains share each bank, so only the bank's first
chain issues start=True; the sibling chain's first write lands on cleared
bits and overwrites.
"""

import os
import sys

for _p in ("/opt/trn_rl_repo", os.path.expanduser("~/.axon_site/_ro/trn_rl_repo")):
    if os.path.isdir(_p) and _p not in sys.path:
        sys.path.insert(0, _p)

import numpy as np
import ml_dtypes

import concourse.bass as bass
import concourse.tile as tile
from concourse import bacc, mybir
from concourse.bass_utils import run_bass_kernel_spmd

F32 = mybir.dt.float32
BF16 = mybir.dt.bfloat16

D_MODEL = 4096
KEY_SIZE = 128
NUM_Q_HEADS = 32
NUM_KV_HEADS = 8
N_CORES = 8
NH = NUM_Q_HEADS // NUM_KV_HEADS  # q heads per core = 4
ATTN_MULT = 0.08838834764831845
CAP = 30.0

Tanh = mybir.ActivationFunctionType.Tanh
Exp = mybir.ActivationFunctionType.Exp


def build_nc(T: int, causal: bool):
    """Emit the Bass program for one core (SPMD: all cores run this).

    Tile builds a STATIC per-engine schedule in (priority = emission)
    order, so overlap must be engineered in the emission order itself.
    """
    D = D_MODEL
    TC = 512                 # t-chunk width
    NTC = T // TC            # t-chunks
    NTT = T // 128           # 128-tiles along T (key side)
    NDT = D // 128           # contraction tiles over d_model = 32
    JW = NH * KEY_SIZE       # per-core q/o width = 512
    GW = 2                   # key tiles per QK group (1 PSUM bank each)

    SPLIT_O = os.environ.get("MHA_SPLIT_O", "0") == "1"
    EVAC_ALT = os.environ.get("MHA_EVAC_ALT", "0") == "1"
    PROJ_ORDER2 = os.environ.get("MHA_PORDER2", "0") == "1"
    DRAIN2 = os.environ.get("MHA_DRAIN2", "0") == "1"

    nc = bacc.Bacc(None, target_bir_lowering=False)

    xq = nc.dram_tensor("xq", [D, T], BF16, kind="ExternalInput")
    xk = nc.dram_tensor("xk", [D, T], BF16, kind="ExternalInput")
    xv = nc.dram_tensor("xv", [D, T], BF16, kind="ExternalInput")
    wq = nc.dram_tensor("wq", [128, NH * NDT * 128], BF16, kind="ExternalInput")
    wk = nc.dram_tensor("wk", [128, NDT * 128], BF16, kind="ExternalInput")
    wv = nc.dram_tensor("wv", [128, NDT * 128], BF16, kind="ExternalInput")
    wo = nc.dram_tensor("wo", [JW, D], BF16, kind="ExternalInput")
    cosd = nc.dram_tensor("cosT", [128, T], F32, kind="ExternalInput")
    sind = nc.dram_tensor("sinT", [128, T], F32, kind="ExternalInput")
    rotd = nc.dram_tensor("rot", [128, 128], BF16, kind="ExternalInput")
    identbd = nc.dram_tensor("identb", [128, 128], BF16, kind="ExternalInput")
    trid = nc.dram_tensor("tri", [128, 128], BF16, kind="ExternalInput")
    outd = nc.dram_tensor("out", [T, D], BF16, kind="ExternalOutput")
    outa = nc.dram_tensor("outa", [TC, D], BF16, kind="ExternalOutput")
    outb = nc.dram_tensor("outb", [TC, D], BF16, kind="ExternalOutput")

    with tile.TileContext(nc) as tc:
        with (
            tc.tile_pool(name="const", bufs=1) as constp,
            tc.tile_pool(name="persist", bufs=1) as persist,
            tc.tile_pool(name="slabs", bufs=2) as slabp,
            tc.tile_pool(name="tmps", bufs=2) as tmpp,
            tc.tile_pool(name="pa", bufs=2) as pa,
            tc.tile_pool(name="qkps", bufs=2, space="PSUM") as qkps,
            tc.tile_pool(name="pvps", bufs=1, space="PSUM") as pvps,
            tc.tile_pool(name="mmps", bufs=2, space="PSUM") as mmps,
        ):
            # ---- persistent SBUF ----
            rot_sb = constp.tile([128, 128], BF16)
            identb_sb = constp.tile([128, 128], BF16)
            tri_sb = constp.tile([128, 128], BF16)
            cos_sb = constp.tile([128, T], F32)
            sin_sb = constp.tile([128, T], F32)
            kT_rope = persist.tile([128, T], BF16)
            vaug = persist.tile([128, NTT, 132], BF16)
            wq_sb = persist.tile([128, NH, NDT, 128], BF16)
            wk_sb = persist.tile([128, NDT, 128], BF16)
            wv_sb = persist.tile([128, NDT, 128], BF16)
            wo_sb = persist.tile([128, NH, D], BF16)

            # ---- tiny consts first (clears the DMA queue fast) ----
            nc.sync.dma_start(out=rot_sb, in_=rotd[:])
            nc.sync.dma_start(out=identb_sb, in_=identbd[:])
            nc.sync.dma_start(out=tri_sb, in_=trid[:])
            nc.any.memset(vaug[:, :, 128:132], 1.0)

            # ---------------- emit-helper closures ----------------
            # Filler items are (cost_ns, fn) pairs.

            def kv_fillers(xsrc, w_sb, tch, dst_cb):
                """K or V projection of t-columns [tch*512,(tch+1)*512)."""
                st = {}

                def dma_i(i):
                    def f():
                        slab = slabp.tile(
                            [128, 8, TC], BF16, tag="kvslab", bufs=3,
                            name="kvslab",
                        )
                        nc.sync.dma_start(
                            out=slab,
                            in_=xsrc[
                                i * 1024 : (i + 1) * 1024,
                                tch * TC : (tch + 1) * TC,
                            ].rearrange("(n k) t -> k n t", k=128),
                        )
                        st[i] = slab
                    return {"cost": 100, "fn": f, "dma": 2912, "kind": "dma"}

                def comp_i(i):
                    def f():
                        if i == 0:
                            st["ps"] = mmps.tile(
                                [128, TC], F32, tag="mm", name="kv_ps"
                            )
                        ps = st["ps"]
                        for j in range(8):
                            nc.tensor.matmul(
                                ps,
                                w_sb[:, i * 8 + j, :],
                                st[i][:, j, :],
                                start=(i == 0 and j == 0),
                                stop=(i == 3 and j == 7),
                            )
                        if i == 3:
                            dst_cb(ps)
                    return {"cost": 1710, "fn": f, "dma": 0, "kind": "comp"}

                return [dma_i(0), dma_i(1), comp_i(0), dma_i(2), comp_i(1),
                        dma_i(3), comp_i(2), comp_i(3)]

            def rope(dst, src, t0, tw):
                """dst[128, tw] = RoPE(src[128, tw]) at positions t0.. (fp32
                math; src/dst bf16)."""
                rp = mmps.tile([128, TC], F32, tag="mm", name="rope_ps")
                nc.tensor.matmul(rp[:, :tw], rot_sb, src, start=True, stop=True)
                t1 = pa.tile([128, TC], F32, tag="rt1", bufs=1, name="rope_t1")
                nc.gpsimd.tensor_mul(t1[:, :tw], src, cos_sb[:, t0 : t0 + tw])
                t2 = pa.tile([128, TC], F32, tag="rt2", bufs=1, name="rope_t2")
                nc.vector.tensor_mul(t2[:, :tw], rp[:, :tw], sin_sb[:, t0 : t0 + tw])
                nc.vector.tensor_add(dst, t1[:, :tw], t2[:, :tw])

            def k_chunk_fillers(tch):
                ktmp = tmpp.tile([128, TC], BF16, tag="ktmp", name="ktmp")

                def evac(ps):
                    nc.vector.tensor_copy(ktmp, ps)

                items = kv_fillers(xk, wk_sb, tch, evac)

                def rope_k():
                    rope(kT_rope[:, tch * TC : (tch + 1) * TC], ktmp,
                         tch * TC, TC)

                return items + [{"cost": 350, "fn": rope_k, "dma": 0, "kind": "comp"}]

            def v_chunk_fillers(tch):
                vtmp = tmpp.tile([128, TC], BF16, tag="vtmp", name="vtmp")

                def evac(ps):
                    nc.vector.tensor_copy(vtmp, ps)

                items = kv_fillers(xv, wv_sb, tch, evac)

                def vtr(half):
                    def f():
                        for b2 in range(2):
                            b = 4 * tch + 2 * half + b2
                            tp = mmps.tile(
                                [128, TC], BF16, tag="mm", name="vtr_ps"
                            )
                            nc.tensor.transpose(
                                tp[:, :128],
                                vtmp[:, (2 * half + b2) * 128 :
                                     (2 * half + b2 + 1) * 128],
                                identb_sb,
                            )
                            nc.vector.tensor_copy(vaug[:, b, 0:128], tp[:, :128])
                    return {"cost": 220, "fn": f, "dma": 0, "kind": "comp"}

                return items + [vtr(0), vtr(1)]

            def qslab_dma_fillers(tcx):
                slabs = []

                def dma_h(dh):
                    def f():
                        slab = slabp.tile(
                            [128, 16, TC], BF16, tag="qslab", name="qslab"
                        )
                        nc.sync.dma_start(
                            out=slab,
                            in_=xq[
                                dh * 2048 : (dh + 1) * 2048,
                                tcx * TC : (tcx + 1) * TC,
                            ].rearrange("(n k) t -> k n t", k=128),
                        )
                        slabs.append(slab)
                    return {"cost": 100, "fn": f, "dma": 5825, "kind": "dma"}

                return slabs, [dma_h(0), dma_h(1)]

            def qproj_chain(slabs, qraw, jh):
                ps = mmps.tile([128, TC], F32, tag="mm", name="q_ps")
                for dh in range(2):
                    for i in range(16):
                        nc.tensor.matmul(
                            ps,
                            wq_sb[:, jh, dh * 16 + i, :],
                            slabs[dh][:, i, :],
                            start=(dh == 0 and i == 0),
                            stop=(dh == 1 and i == 15),
                        )
                nc.vector.tensor_copy(qraw[:, jh, :], ps)

            def q_chunk_fillers(tcx, slabs):
                """Q proj + rope for chunk tcx; returns (qrope, items)."""
                qraw = tmpp.tile([128, NH, TC], BF16, tag="qraw", name="qraw")
                qrope = tmpp.tile([128, NH, TC], BF16, tag="qrope", name="qrope")
                items = []
                for jh in range(NH):
                    items.append({
                        "cost": 6830, "dma": 0, "kind": "comp",
                        "fn": lambda jh=jh: qproj_chain(slabs, qraw, jh),
                    })
                for jh in range(NH):
                    items.append({
                        "cost": 350, "dma": 0, "kind": "comp",
                        "fn": lambda jh=jh: rope(
                            qrope[:, jh, :], qraw[:, jh, :], tcx * TC, TC
                        ),
                    })
                return qrope, items

            def make_oproj_fillers(attnT, t0, jhs=range(NH), dest=None,
                                   dest_t0=None, evac_alt=False):
                dest = outd if dest is None else dest
                dest_t0 = t0 if dest_t0 is None else dest_t0
                jhs = list(jhs)
                fillers = []
                for nch in range(D // TC):
                    for s4 in range(4):
                        def f(s4=s4, nch=nch):
                            with nc.named_scope("oproj"):
                                ps = mmps.tile(
                                    [128, TC], F32, tag="mm", name="o_ps"
                                )
                                for x, jh in enumerate(jhs):
                                    nc.tensor.matmul(
                                        ps,
                                        attnT[:, jh, s4 * 128 : (s4 + 1) * 128],
                                        wo_sb[:, jh, nch * TC : (nch + 1) * TC],
                                        start=(x == 0),
                                        stop=(x == len(jhs) - 1),
                                    )
                                osb = pa.tile(
                                    [128, TC], BF16, tag="osb", bufs=3,
                                    name="osb",
                                )
                                if evac_alt and (s4 + nch) % 2 == 0:
                                    nc.scalar.copy(out=osb, in_=ps)
                                else:
                                    nc.vector.tensor_copy(osb, ps)
                                nc.sync.dma_start(
                                    out=dest[
                                        dest_t0 + s4 * 128 :
                                        dest_t0 + (s4 + 1) * 128,
                                        nch * TC : (nch + 1) * TC,
                                    ],
                                    in_=osb,
                                )
                        fillers.append(
                            {"cost": 218 * len(jhs), "fn": f,
                             "dma": 364, "kind": "oproj"}
                        )
                return fillers

            def interleave(a, b):
                out = []
                ia = ib = 0
                na, nb = len(a), len(b)
                while ia < na or ib < nb:
                    if ia * max(nb, 1) <= ib * max(na, 1) and ia < na:
                        out.append(a[ia]); ia += 1
                    elif ib < nb:
                        out.append(b[ib]); ib += 1
                    else:
                        out.append(a[ia]); ia += 1
                return out

            def proj_items_for(tcx):
                """All projection work for chunk tcx as a filler list, DMA
                items placed so transfers land just ahead of their use."""
                slabs_n, qdma = qslab_dma_fillers(tcx)
                kn = k_chunk_fillers(tcx)
                vn = v_chunk_fillers(tcx)
                qrope_n, qn = q_chunk_fillers(tcx, slabs_n)
                if PROJ_ORDER2:
                    items = (
                        [kn[0], kn[1], kn[2], qdma[0], kn[3], kn[4], qdma[1],
                         kn[5], kn[6], kn[7], kn[8],
                         qn[0], vn[0], qn[1], vn[1], qn[2], vn[3], qn[3],
                         vn[5], vn[2], vn[4], vn[6], vn[7], vn[8], vn[9]]
                        + qn[NH:]
                    )
                else:
                    items = (
                        [kn[0], kn[1], kn[2], qdma[0], kn[3], kn[4], qdma[1],
                         kn[5], kn[6], kn[7], kn[8]]
                        + vn[:8] + [vn[8], vn[9]]
                        + qn
                    )
                return qrope_n, items

            # ---------------- chunk 0 prologue (inline, DMA-ordered) ----
            # Critical path to the first q chain: wq head 0 + both qslabs;
            # everything else (k/v slabs, cos/sin) streams behind and PE
            # picks it up between/after the q chains.
            def wq_head_dma(jh):
                nc.sync.dma_start(
                    out=wq_sb[:, jh, :, :],
                    in_=wq[:, jh * NDT * 128 : (jh + 1) * NDT * 128].rearrange(
                        "k (n j) -> k n j", j=128
                    ),
                )

            k0 = k_chunk_fillers(0)
            v0 = v_chunk_fillers(0)
            qslabs0, qdma0 = qslab_dma_fillers(0)
            wq_head_dma(0)
            qdma0[0]['fn']()                      # dma qslab dh0
            qdma0[1]['fn']()                      # dma qslab dh1
            wq_head_dma(1)
            nc.sync.dma_start(out=wk_sb, in_=wk.rearrange("k (n j) -> k n j", j=128))
            wq_head_dma(2)
            k0[0]['fn']()                         # dma kslab0
            wq_head_dma(3)
            k0[1]['fn']()                         # dma kslab1
            qrope0, q0 = q_chunk_fillers(0, qslabs0)
            q0[0]['fn'](); q0[1]['fn']()       # qproj chains 0,1
            k0[3]['fn']()                         # dma kslab2
            q0[2]['fn']()                      # qproj chain 2
            k0[2]['fn']()                         # comp k piece 0
            nc.sync.dma_start(out=cos_sb, in_=cosd[:])
            q0[3]['fn']()                      # qproj chain 3
            nc.sync.dma_start(out=sin_sb, in_=sind[:])
            k0[5]['fn']()                         # dma kslab3
            k0[4]['fn'](); k0[6]['fn'](); k0[7]['fn']() # comp k pieces 1-3 + evac
            nc.sync.dma_start(out=wv_sb, in_=wv.rearrange("k (n j) -> k n j", j=128))
            k0[8]['fn']()                         # rope-k(0)
            for it in q0[NH:]:                 # 4 rope-q(0)
                it['fn']()
            v0[0]['fn'](); v0[1]['fn']()             # dma vslab0/1
            v0[2]['fn']()                         # comp v piece 0
            v0[3]['fn'](); v0[5]['fn']()             # dma vslab2/3
            v0[4]['fn']()                         # comp v piece 1
            v0[6]['fn'](); v0[7]['fn']()             # comp v pieces 2,3 + evac
            v0[8]['fn'](); v0[9]['fn']()             # vtr halves

            def wo_slice_dma(nch):
                def f():
                    nc.sync.dma_start(
                        out=wo_sb[:, :, nch * TC : (nch + 1) * TC],
                        in_=wo[:, nch * TC : (nch + 1) * TC].rearrange(
                            "(n k) d -> k n d", k=128
                        ),
                    )
                return {"cost": 100, "fn": f, "dma": 1456, "kind": "dma"}

            # ---------------- main loop over t-chunks ----------------
            qrope_cur = qrope0
            prev_attnT = None
            prev_t0 = 0
            carry = []          # deferred oproj fillers from chunk tcx-1
            for tcx in range(NTC):
                t0 = tcx * TC

                if tcx + 1 < NTC:
                    qrope_next, proj_items = proj_items_for(tcx + 1)
                else:
                    qrope_next, proj_items = None, []
                if tcx == 0:
                    wos = [wo_slice_dma(n) for n in range(D // TC)]
                    proj_items = interleave(proj_items, wos)
                oproj_items = carry + (
                    make_oproj_fillers(prev_attnT, prev_t0)
                    if prev_attnT is not None
                    else []
                )
                fillers = interleave(proj_items, oproj_items)
                # annotate each compute item with the cumulative input-DMA
                # time that precedes it in this window's queue — popping it
                # earlier than that would head-of-line block the in-order
                # PE stream on an un-arrived transfer.
                cum_dma = 0.0
                for it in fillers:
                    if it["kind"] == "dma":
                        cum_dma += it["dma"]
                    it["ready"] = cum_dma if it["kind"] == "comp" else 0.0

                nt_valid = 4 * (tcx + 1) if causal else NTT
                ngroups = nt_valid // GW
                attnT = pa.tile(
                    [128, NH, TC], BF16, tag="attnT", bufs=3, name="attnT"
                )
                budget = 0.0
                popped = 0.0
                qkpv_clock = 0.0
                act_clock = 0.0
                popped_dma = 0.0
                SLACK = float(os.environ.get("MHA_SLACK", "2000"))
                LOOKAHEAD = float(os.environ.get("MHA_LOOKAHEAD", "9000"))
                BMULT = float(os.environ.get("MHA_BMULT", "1.0"))

                def pop_fillers():
                    nonlocal popped, popped_dma
                    while popped < budget and fillers:
                        elapsed = max(act_clock, qkpv_clock + popped)
                        # pull any leading dma items (keep the queue fed,
                        # but no more than LOOKAHEAD ahead of real time)
                        i = 0
                        progress = False
                        while i < len(fillers):
                            it = fillers[i]
                            if (it["kind"] == "dma"
                                    and popped_dma < elapsed + LOOKAHEAD):
                                fillers.pop(i)
                                it["fn"]()
                                popped_dma += it["dma"]
                                progress = True
                                continue
                            if it["kind"] != "dma":
                                break
                            i += 1
                        if not fillers or popped >= budget:
                            break
                        head = fillers[0]
                        if (head["kind"] != "dma"
                                and head["ready"] <= elapsed + SLACK):
                            fillers.pop(0)
                            head["fn"]()
                            popped += head["cost"]
                            popped_dma += head["dma"]
                            progress = True
                        if not progress:
                            break
                for h in range(NH):
                    with nc.named_scope("attn"):
                        pv = pvps.tile(
                            [128, 4, 256], F32, tag="pv", name="pv_ps"
                        )
                        for gg in range(ngroups):
                            qk = qkps.tile(
                                [128, GW, TC], F32, tag="qk", name="qk_ps"
                            )
                            rels = []
                            for b in range(GW):
                                Tt = GW * gg + b
                                rel = Tt - 4 * tcx if causal else -1
                                rels.append(rel)
                                c0 = 128 * rel if rel > 0 else 0
                                nc.tensor.matmul(
                                    qk[:, b, c0:TC],
                                    kT_rope[:, Tt * 128 : (Tt + 1) * 128],
                                    qrope_cur[:, h, c0:TC],
                                    start=True,
                                    stop=True,
                                )
                            # tanh in place in PSUM, then exp to bf16 SBUF;
                            # soft-capping scales fused into ACT. Columns
                            # below the causal diagonal are skipped.
                            pt = pa.tile(
                                [128, GW, TC], BF16, tag="pt", bufs=3,
                                name="ptile",
                            )
                            act_cols = 0
                            if max(rels) <= 0:
                                nc.scalar.activation(
                                    out=qk, in_=qk, func=Tanh,
                                    scale=ATTN_MULT / CAP,
                                )
                                nc.scalar.activation(
                                    out=pt, in_=qk, func=Exp, scale=CAP
                                )
                                act_cols = GW * TC
                            else:
                                for b in range(GW):
                                    c0 = 128 * max(rels[b], 0)
                                    nc.scalar.activation(
                                        out=qk[:, b, c0:TC],
                                        in_=qk[:, b, c0:TC],
                                        func=Tanh, scale=ATTN_MULT / CAP,
                                    )
                                    nc.scalar.activation(
                                        out=pt[:, b, c0:TC],
                                        in_=qk[:, b, c0:TC],
                                        func=Exp, scale=CAP,
                                    )
                                    act_cols += TC - c0
                            for b in range(GW):
                                rel = rels[b]
                                if 0 <= rel < 4:
                                    # triangular mask on the diagonal block
                                    nc.gpsimd.tensor_mul(
                                        pt[:, b, rel * 128 : (rel + 1) * 128],
                                        pt[:, b, rel * 128 : (rel + 1) * 128],
                                        tri_sb,
                                    )
                            n_pv = 0
                            for s4 in range(4):
                                for b in range(GW):
                                    Tt = GW * gg + b
                                    rel = rels[b]
                                    if causal and rel > s4:
                                        continue
                                    n_pv += 1
                                    nc.tensor.matmul(
                                        pv[:, s4, 0:129],
                                        pt[:, b, s4 * 128 : (s4 + 1) * 128],
                                        vaug[:, Tt, 0:129],
                                        start=(
                                            gg == 0 and b == 0 and s4 % 2 == 0
                                        ),
                                        stop=(
                                            (Tt == 4 * tcx + s4)
                                            if causal
                                            else (gg == ngroups - 1
                                                  and b == GW - 1)
                                        ),
                                        skip_group_check=True,
                                    )
                            # weave fillers so PE stays busy under ACT
                            act_ns = act_cols * 2 * 0.833 + (
                                330 if max(rels) <= 0 else 660
                            )
                            qkpv_ns = (act_cols + 129 * n_pv) * 0.4167
                            act_clock += act_ns
                            qkpv_clock += qkpv_ns
                            budget += BMULT * max(act_ns - qkpv_ns, 0.0)
                            pop_fillers()
                    with nc.named_scope("attn_fin"):
                        ans = []
                        for s4 in range(4):
                            rc = pa.tile(
                                [128, 1], F32, tag="rc", bufs=4, name="rc"
                            )
                            nc.vector.reciprocal(rc, pv[:, s4, 128:129])
                            an = pa.tile(
                                [128, 128], BF16, tag="an", bufs=4, name="an"
                            )
                            nc.vector.tensor_scalar_mul(an, pv[:, s4, 0:128], rc)
                            ans.append(an)
                        # cover the DVE normalize latency with a filler
                        budget += 700
                        act_clock += 700
                        pop_fillers()
                        for s4 in range(4):
                            tp = mmps.tile([128, TC], BF16, tag="mm", name="atr")
                            nc.tensor.transpose(tp[:, :128], ans[s4], identb_sb)
                            nc.vector.tensor_copy(
                                attnT[:, h, s4 * 128 : (s4 + 1) * 128],
                                tp[:, :128],
                            )
                    if SPLIT_O and tcx == NTC - 1 and h == 1:
                        for it in make_oproj_fillers(
                            attnT, t0, jhs=[0, 1], dest=outa, dest_t0=0
                        ):
                            it["ready"] = 0.0
                            fillers.append(it)
                # drain: proj items must finish before attn(tcx+1); carry
                # up to 14 oproj items into the next chunk (attnT bufs=3
                # keeps chunk tcx-1's attnT alive through attn(tcx+1)).
                carry = []
                rest = fillers
                if tcx + 1 < NTC:
                    keep = []
                    for it in reversed(rest):
                        if it["kind"] == "oproj" and len(carry) < int(os.environ.get("MHA_CARRY", "14")):
                            carry.append(it)
                        else:
                            keep.append(it)
                    carry.reverse()
                    keep.reverse()
                    rest = keep
                # drain with the same dma-forwarding discipline: keep
                # transfers ~LOOKAHEAD ahead of the estimated PE clock so
                # in-order compute items rarely wait on arrival.
                if not DRAIN2:
                    for it in rest:
                        it["fn"]()
                    rest = []
                el = max(act_clock, qkpv_clock + popped)
                dma_el = popped_dma
                while rest:
                    i = 0
                    while i < len(rest):
                        if (rest[i]["kind"] == "dma"
                                and dma_el < el + LOOKAHEAD):
                            it = rest.pop(i)
                            it["fn"]()
                            dma_el += it["dma"]
                            continue
                        if rest[i]["kind"] != "dma":
                            break
                        i += 1
                    if not rest:
                        break
                    it = rest.pop(0)
                    it["fn"]()
                    el = max(el, it.get("ready", 0.0)) + it["cost"]
                    dma_el += it["dma"]
                qrope_cur = qrope_next
                prev_attnT, prev_t0 = attnT, t0

            # tail: O proj pass B of the last chunk (host adds outa+outb)
            tail_items = (
                make_oproj_fillers(prev_attnT, prev_t0, jhs=[2, 3],
                                   dest=outb, dest_t0=0, evac_alt=EVAC_ALT)
                if SPLIT_O
                else make_oproj_fillers(prev_attnT, prev_t0,
                                        evac_alt=EVAC_ALT)
            )
            for it in carry + tail_items:
                it["fn"]()

    nc.compile()
    return nc


def vbgd_dst(vaug):
    return vaug[:, :, 128:132]


def _host_constants(T: int):
    d = KEY_SIZE
    inv_freq = 1.0 / (10000.0 ** (np.arange(0, d, 2, dtype=np.float64) / d))  # [64]
    pos = np.arange(T, dtype=np.float64)
    phase_half = pos[None, :] * inv_freq[:, None]  # [64, T]
    phase = np.concatenate([phase_half, phase_half], axis=0)  # [128, T] (tiled)
    cosT = np.cos(phase).astype(np.float32)
    sinT = np.sin(phase).astype(np.float32)

    R = np.zeros((128, 128), dtype=np.float32)
    R[:64, 64:] = -np.eye(64, dtype=np.float32)
    R[64:, :64] = np.eye(64, dtype=np.float32)
    rot = np.ascontiguousarray(R.T)

    ident = np.eye(128, dtype=np.float32)

    # tri[k, c] = 1 if k <= c (valid: query col >= key row inside the
    # diagonal 128x128 block)
    tri = (np.arange(128)[:, None] <= np.arange(128)[None, :]).astype(
        ml_dtypes.bfloat16
    )

    NTT = T // 128
    vbg = np.zeros((128, NTT, 4), dtype=ml_dtypes.bfloat16)
    vbg[:, :, 0] = 1.0
    return cosT, sinT, rot, ident, tri, vbg


_NC_CACHE: dict = {}
LAST_RESULT = None
_LAST_IN_MAPS = None


def kernel(query, key, value, mask, Wq, Wk, Wv, Wo):
    global LAST_RESULT, _LAST_IN_MAPS
    query = np.asarray(query)
    key = np.asarray(key)
    value = np.asarray(value)
    mask = np.asarray(mask)
    Wq = np.asarray(Wq, dtype=np.float32)
    Wk = np.asarray(Wk, dtype=np.float32)
    Wv = np.asarray(Wv, dtype=np.float32)
    Wo = np.asarray(Wo, dtype=np.float32)

    b, T, D = query.shape
    assert b == 1 and D == D_MODEL, (b, D)

    m2 = np.asarray(mask).reshape(T, T).astype(bool)
    if np.array_equal(m2, np.tril(np.ones((T, T), dtype=bool))):
        causal = True
    elif m2.all():
        causal = False
    else:
        raise ValueError("unsupported mask pattern (expected causal or full)")

    kkey = (T, causal)
    if kkey not in _NC_CACHE:
        _NC_CACHE[kkey] = build_nc(T, causal)
    nc = _NC_CACHE[kkey]

    pnp = ml_dtypes.bfloat16
    xq = np.ascontiguousarray(query[0].T).astype(pnp)  # [D, T]
    xk = np.ascontiguousarray(key[0].T).astype(pnp)
    xv = np.ascontiguousarray(value[0].T).astype(pnp)
    cosT, sinT, rot, ident, tri, vbg = _host_constants(T)

    JW = NH * KEY_SIZE
    NDT = D // 128

    def pack_w(w, nh):
        # [D, nh*128] -> [k, jh, n, j] flattened per-partition-contiguous
        a = np.ascontiguousarray(w).astype(pnp)
        a = a.reshape(NDT, 128, nh, 128).transpose(1, 2, 0, 3)
        return np.ascontiguousarray(a.reshape(128, nh * NDT * 128))

    in_maps = []
    for c in range(N_CORES):
        in_maps.append(
            {
                "xq": xq,
                "xk": xk,
                "xv": xv,
                "wq": pack_w(Wq[:, c * JW : (c + 1) * JW], NH),
                "wk": pack_w(Wk[:, c * KEY_SIZE : (c + 1) * KEY_SIZE], 1),
                "wv": pack_w(Wv[:, c * KEY_SIZE : (c + 1) * KEY_SIZE], 1),
                "wo": np.ascontiguousarray(Wo[c * JW : (c + 1) * JW, :]).astype(pnp),
                "cosT": cosT,
                "sinT": sinT,
                "rot": rot.astype(pnp),
                "identb": ident.astype(pnp),
                "tri": tri,
            }
        )

    _LAST_IN_MAPS = in_maps
    trace = os.environ.get("MHA_TRACE") == "1"
    res = run_bass_kernel_spmd(nc, in_maps, list(range(N_CORES)), trace=trace)
    LAST_RESULT = res

    out = np.zeros((T, D), dtype=np.float64)
    for c in range(N_CORES):
        out += res.results[c]["out"].astype(np.float64)
    return out.astype(np.float32).reshape(1, T, D)


# revision 33
# speedup vs baseline: 1.1489x; 1.0315x over previous
"""Trainium2 Bass kernel for nn_MultiHeadAttention_83056077570808.

GQA multi-head attention (32 q heads, 8 kv heads, d_head=128, T=2048,
D=4096) with RoPE, tanh soft-capping at 30, causal mask, fp32 reference.

Sharding: tensor-parallel over heads across 8 cores. Core c owns kv head c
and q heads 4c..4c+3: Wq/Wk/Wv column-sharded, Wo row-sharded; activations
replicated. Each core computes a partial output (its heads' contribution
through its Wo rows); the host sums the 8 partials.

Fully streamed schedule: causality means attention chunk tcx only needs
K/V tiles 0..4*tcx+3, so K/V/Q projections for chunk tcx+1 run *during*
attention of chunk tcx as filler work woven between QK groups (covering
the ACT-engine tanh/exp latency); O-proj of chunk tcx-1 likewise. DMA is
spread across the whole timeline instead of front-loaded. Fillers are
paced by an explicit cost model (popping too fast blocks the in-order PE
stream on un-arrived slab DMAs; too slow starves PE under ACT).

Causal diagonal trim: for key tile Tt in the diagonal block of chunk tcx
(rel = Tt-4*tcx in 0..3), query columns < 128*rel are entirely masked, so
QK / tanh / exp are column-trimmed, only the [128,128] diagonal block is
tri-masked (Pool), and PV skips s4-blocks with s4 < rel.

All matmuls are bf16; PSUM accumulation fp32; rope arithmetic fp32.

PSUM bank rule in the PV accumulation: start=True clears has_written for
the WHOLE bank and two s-chains share each bank, so only the bank's first
chain issues start=True; the sibling chain's first write lands on cleared
bits and overwrites.
"""

import os
import sys

for _p in ("/opt/trn_rl_repo", os.path.expanduser("~/.axon_site/_ro/trn_rl_repo")):
    if os.path.isdir(_p) and _p not in sys.path:
        sys.path.insert(0, _p)

import numpy as np
import ml_dtypes

import concourse.bass as bass
import concourse.tile as tile
from concourse import bacc, mybir
from concourse.bass_utils import run_bass_kernel_spmd

F32 = mybir.dt.float32
BF16 = mybir.dt.bfloat16

D_MODEL = 4096
KEY_SIZE = 128
NUM_Q_HEADS = 32
NUM_KV_HEADS = 8
N_CORES = 8
NH = NUM_Q_HEADS // NUM_KV_HEADS  # q heads per core = 4
ATTN_MULT = 0.08838834764831845
CAP = 30.0

Tanh = mybir.ActivationFunctionType.Tanh
Exp = mybir.ActivationFunctionType.Exp


def build_nc(T: int, causal: bool):
    """Emit the Bass program for one core (SPMD: all cores run this).

    Tile builds a STATIC per-engine schedule in (priority = emission)
    order, so overlap must be engineered in the emission order itself.
    """
    D = D_MODEL
    TC = 512                 # t-chunk width
    NTC = T // TC            # t-chunks
    NTT = T // 128           # 128-tiles along T (key side)
    NDT = D // 128           # contraction tiles over d_model = 32
    JW = NH * KEY_SIZE       # per-core q/o width = 512
    GW = 2                   # key tiles per QK group (1 PSUM bank each)

    SPLIT_O = os.environ.get("MHA_SPLIT_O", "0") == "1"
    EVAC_ALT = os.environ.get("MHA_EVAC_ALT", "0") == "1"
    PROJ_ORDER2 = os.environ.get("MHA_PORDER2", "0") == "1"
    DRAIN2 = os.environ.get("MHA_DRAIN2", "0") == "1"
    DEFER_O = os.environ.get("MHA_DEFER_O", "0") == "1"
    KVSPILL = os.environ.get("MHA_KVSPILL", "0") == "1"

    nc = bacc.Bacc(None, target_bir_lowering=False)

    xq = nc.dram_tensor("xq", [D, T], BF16, kind="ExternalInput")
    xk = nc.dram_tensor("xk", [D, T], BF16, kind="ExternalInput")
    xv = nc.dram_tensor("xv", [D, T], BF16, kind="ExternalInput")
    wq = nc.dram_tensor("wq", [128, NH * NDT * 128], BF16, kind="ExternalInput")
    wk = nc.dram_tensor("wk", [128, NDT * 128], BF16, kind="ExternalInput")
    wv = nc.dram_tensor("wv", [128, NDT * 128], BF16, kind="ExternalInput")
    wo = nc.dram_tensor("wo", [JW, D], BF16, kind="ExternalInput")
    cosd = nc.dram_tensor("cosT", [128, T], BF16, kind="ExternalInput")
    sind = nc.dram_tensor("sinT", [128, T], BF16, kind="ExternalInput")
    rotd = nc.dram_tensor("rot", [128, 128], BF16, kind="ExternalInput")
    identbd = nc.dram_tensor("identb", [128, 128], BF16, kind="ExternalInput")
    trid = nc.dram_tensor("tri", [128, 128], BF16, kind="ExternalInput")
    outd = nc.dram_tensor("out", [T, D], BF16, kind="ExternalOutput")
    outa = nc.dram_tensor("outa", [TC, D], BF16, kind="ExternalOutput")
    outb = nc.dram_tensor("outb", [TC, D], BF16, kind="ExternalOutput")

    with tile.TileContext(nc) as tc:
        with (
            tc.tile_pool(name="const", bufs=1) as constp,
            tc.tile_pool(name="persist", bufs=1) as persist,
            tc.tile_pool(name="slabs", bufs=2) as slabp,
            tc.tile_pool(name="tmps", bufs=2) as tmpp,
            tc.tile_pool(name="pa", bufs=2) as pa,
            tc.tile_pool(name="qkps", bufs=2, space="PSUM") as qkps,
            tc.tile_pool(name="pvps", bufs=1, space="PSUM") as pvps,
            tc.tile_pool(name="mmps", bufs=2, space="PSUM") as mmps,
        ):
            # ---- persistent SBUF ----
            rot_sb = constp.tile([128, 128], BF16)
            identb_sb = constp.tile([128, 128], BF16)
            tri_sb = constp.tile([128, 128], BF16)
            cos_sb = constp.tile([128, T], BF16)
            sin_sb = constp.tile([128, T], BF16)
            kT_rope = persist.tile([128, T], BF16)
            vaug = persist.tile([128, NTT, 132], BF16)
            wq_sb = persist.tile([128, NH, NDT, 128], BF16)
            wk_sb = persist.tile([128, NDT, 128], BF16)
            wv_sb = persist.tile([128, NDT, 128], BF16)
            wo_sb = persist.tile([128, NH, D], BF16)

            # ---- tiny consts first (clears the DMA queue fast) ----
            nc.sync.dma_start(out=rot_sb, in_=rotd[:])
            nc.sync.dma_start(out=identb_sb, in_=identbd[:])
            nc.sync.dma_start(out=tri_sb, in_=trid[:])
            nc.any.memset(vaug[:, :, 128:132], 1.0)

            # ---------------- emit-helper closures ----------------
            # Filler items are (cost_ns, fn) pairs.

            def kv_fillers(xsrc, w_sb, tch, dst_cb):
                """K or V projection of t-columns [tch*512,(tch+1)*512)."""
                st = {}

                def dma_i(i, half):
                    def f():
                        if half == 0:
                            st[i] = slabp.tile(
                                [128, 8, TC], BF16, tag="kvslab", bufs=3,
                                name="kvslab",
                            )
                        nc.sync.dma_start(
                            out=st[i][:, 4 * half : 4 * half + 4, :],
                            in_=xsrc[
                                i * 1024 + half * 512 :
                                i * 1024 + (half + 1) * 512,
                                tch * TC : (tch + 1) * TC,
                            ].rearrange("(n k) t -> k n t", k=128),
                        )
                    return {"cost": 100, "fn": f, "dma": 1456, "kind": "dma",
                            "grp": "kv", "bar": tch}

                def comp_i(i):
                    def f():
                        if i == 0:
                            st["ps"] = mmps.tile(
                                [128, TC], F32, tag="mm", name="kv_ps"
                            )
                        ps = st["ps"]
                        for j in range(8):
                            nc.tensor.matmul(
                                ps,
                                w_sb[:, i * 8 + j, :],
                                st[i][:, j, :],
                                start=(i == 0 and j == 0),
                                stop=(i == 3 and j == 7),
                            )
                        if i == 3:
                            dst_cb(ps)
                    return {"cost": 1710, "fn": f, "dma": 0, "kind": "comp",
                            "grp": "kv", "bar": tch}

                return [dma_i(0, 0), dma_i(0, 1), dma_i(1, 0), comp_i(0),
                        dma_i(1, 1), dma_i(2, 0), comp_i(1), dma_i(2, 1),
                        dma_i(3, 0), comp_i(2), dma_i(3, 1), comp_i(3)]

            def rope(dst, src, t0, tw):
                """dst[128, tw] = RoPE(src[128, tw]) at positions t0.. (fp32
                math; src/dst bf16)."""
                rp = mmps.tile([128, TC], F32, tag="mm", name="rope_ps")
                nc.tensor.matmul(rp[:, :tw], rot_sb, src, start=True, stop=True)
                t1 = pa.tile([128, TC], F32, tag="rt1", bufs=1, name="rope_t1")
                nc.gpsimd.tensor_mul(t1[:, :tw], src, cos_sb[:, t0 : t0 + tw])
                t2 = pa.tile([128, TC], F32, tag="rt2", bufs=1, name="rope_t2")
                nc.vector.tensor_mul(t2[:, :tw], rp[:, :tw], sin_sb[:, t0 : t0 + tw])
                nc.vector.tensor_add(dst, t1[:, :tw], t2[:, :tw])

            def k_chunk_fillers(tch):
                ktmp = tmpp.tile([128, TC], BF16, tag="ktmp", name="ktmp")

                def evac(ps):
                    nc.vector.tensor_copy(ktmp, ps)

                items = kv_fillers(xk, wk_sb, tch, evac)

                def rope_k():
                    rope(kT_rope[:, tch * TC : (tch + 1) * TC], ktmp,
                         tch * TC, TC)

                return items + [{"cost": 350, "fn": rope_k, "dma": 0,
                                 "kind": "comp", "grp": "kv", "bar": tch}]

            def v_chunk_fillers(tch):
                vtmp = tmpp.tile([128, TC], BF16, tag="vtmp", name="vtmp")

                def evac(ps):
                    nc.vector.tensor_copy(vtmp, ps)

                items = kv_fillers(xv, wv_sb, tch, evac)

                def vtr(half):
                    def f():
                        for b2 in range(2):
                            b = 4 * tch + 2 * half + b2
                            tp = mmps.tile(
                                [128, TC], BF16, tag="mm", name="vtr_ps"
                            )
                            nc.tensor.transpose(
                                tp[:, :128],
                                vtmp[:, (2 * half + b2) * 128 :
                                     (2 * half + b2 + 1) * 128],
                                identb_sb,
                            )
                            nc.vector.tensor_copy(vaug[:, b, 0:128], tp[:, :128])
                    return {"cost": 220, "fn": f, "dma": 0, "kind": "comp",
                            "grp": "kv", "bar": tch}

                return items + [vtr(0), vtr(1)]

            def qslab_dma_fillers(tcx):
                slabs = []

                def dma_h(dh, q):
                    def f():
                        if q == 0:
                            slab = slabp.tile(
                                [128, 16, TC], BF16, tag="qslab", name="qslab"
                            )
                            slabs.append(slab)
                        slab = slabs[dh]
                        nc.sync.dma_start(
                            out=slab[:, 4 * q : 4 * q + 4, :],
                            in_=xq[
                                dh * 2048 + q * 512 : dh * 2048 + (q + 1) * 512,
                                tcx * TC : (tcx + 1) * TC,
                            ].rearrange("(n k) t -> k n t", k=128),
                        )
                    return {"cost": 100, "fn": f, "dma": 1456, "kind": "dma",
                            "grp": "pre"}

                return slabs, [dma_h(0, q) for q in range(4)] + [
                    dma_h(1, q) for q in range(4)
                ]

            def qproj_chain(slabs, qraw, jh):
                ps = mmps.tile([128, TC], F32, tag="mm", name="q_ps")
                for dh in range(2):
                    for i in range(16):
                        nc.tensor.matmul(
                            ps,
                            wq_sb[:, jh, dh * 16 + i, :],
                            slabs[dh][:, i, :],
                            start=(dh == 0 and i == 0),
                            stop=(dh == 1 and i == 15),
                        )
                nc.vector.tensor_copy(qraw[:, jh, :], ps)

            def q_chunk_fillers(tcx, slabs):
                """Q proj + rope for chunk tcx; returns (qrope, items)."""
                qraw = tmpp.tile([128, NH, TC], BF16, tag="qraw", name="qraw")
                qrope = tmpp.tile([128, NH, TC], BF16, tag="qrope", name="qrope")
                items = []
                for jh in range(NH):
                    items.append({
                        "cost": 6830, "dma": 0, "kind": "comp", "grp": "q",
                        "bar": tcx, "qbar": jh,
                        "fn": lambda jh=jh: qproj_chain(slabs, qraw, jh),
                    })
                for jh in range(NH):
                    items.append({
                        "cost": 350, "dma": 0, "kind": "comp", "grp": "q",
                        "bar": tcx, "qbar": jh,
                        "fn": lambda jh=jh: rope(
                            qrope[:, jh, :], qraw[:, jh, :], tcx * TC, TC
                        ),
                    })
                return qrope, items

            def make_oproj_fillers(attnT, t0, jhs=range(NH), dest=None,
                                   dest_t0=None, evac_alt=False):
                dest = outd if dest is None else dest
                dest_t0 = t0 if dest_t0 is None else dest_t0
                jhs = list(jhs)
                fillers = []
                for nch in range(D // TC):
                    for s4 in range(4):
                        def f(s4=s4, nch=nch):
                            with nc.named_scope("oproj"):
                                ps = mmps.tile(
                                    [128, TC], F32, tag="mm", name="o_ps"
                                )
                                for x, jh in enumerate(jhs):
                                    nc.tensor.matmul(
                                        ps,
                                        attnT[:, jh, s4 * 128 : (s4 + 1) * 128],
                                        wo_sb[:, jh, nch * TC : (nch + 1) * TC],
                                        start=(x == 0),
                                        stop=(x == len(jhs) - 1),
                                    )
                                osb = pa.tile(
                                    [128, TC], BF16, tag="osb", bufs=4,
                                    name="osb",
                                )
                                if evac_alt and (s4 + nch) % 2 == 0:
                                    nc.scalar.copy(out=osb, in_=ps)
                                else:
                                    nc.vector.tensor_copy(osb, ps)
                                nc.sync.dma_start(
                                    out=dest[
                                        dest_t0 + s4 * 128 :
                                        dest_t0 + (s4 + 1) * 128,
                                        nch * TC : (nch + 1) * TC,
                                    ],
                                    in_=osb,
                                )
                        fillers.append(
                            {"cost": 218 * len(jhs), "fn": f,
                             "dma": 364, "kind": "oproj", "grp": "o"}
                        )
                return fillers

            def interleave(a, b):
                out = []
                ia = ib = 0
                na, nb = len(a), len(b)
                while ia < na or ib < nb:
                    if ia * max(nb, 1) <= ib * max(na, 1) and ia < na:
                        out.append(a[ia]); ia += 1
                    elif ib < nb:
                        out.append(b[ib]); ib += 1
                    else:
                        out.append(a[ia]); ia += 1
                return out

            def proj_items_for(tcx):
                """All projection work for chunk tcx as a filler list, DMA
                items placed so transfers land just ahead of their use."""
                slabs_n, qdma = qslab_dma_fillers(tcx)
                kn = k_chunk_fillers(tcx)
                vn = v_chunk_fillers(tcx)
                qrope_n, qn = q_chunk_fillers(tcx, slabs_n)
                qpairs = [qn[0], qn[NH], qn[1], qn[NH + 1], qn[2],
                          qn[NH + 2], qn[3], qn[NH + 3]]
                items = (
                    [kn[0], kn[1], qdma[0], qdma[1], kn[2], kn[3], qdma[2],
                     qdma[3], kn[4], kn[5], qdma[4], qdma[5], kn[6], kn[7],
                     qdma[6], qdma[7], kn[8], kn[9], kn[10], kn[11], kn[12]]
                    + vn[:12] + [vn[12], vn[13]]
                    + qpairs
                )
                return qrope_n, items

            # ---------------- chunk 0 prologue (inline, DMA-ordered) ----
            # Critical path to the first q chain: wq head 0 + both qslabs;
            # everything else (k/v slabs, cos/sin) streams behind and PE
            # picks it up between/after the q chains.
            def wq_head_dma(jh):
                nc.sync.dma_start(
                    out=wq_sb[:, jh, :, :],
                    in_=wq[:, jh * NDT * 128 : (jh + 1) * NDT * 128].rearrange(
                        "k (n j) -> k n j", j=128
                    ),
                )

            k0 = k_chunk_fillers(0)
            v0 = v_chunk_fillers(0)
            qslabs0, qdma0 = qslab_dma_fillers(0)
            wq_head_dma(0)
            for it in qdma0:                   # 8 quarter-slab dmas
                it["fn"]()
            wq_head_dma(1)
            nc.sync.dma_start(out=wk_sb, in_=wk.rearrange("k (n j) -> k n j", j=128))
            wq_head_dma(2)
            for it in k0[0:3]:                 # kslab dmas
                it["fn"]()
            wq_head_dma(3)
            qrope0, q0 = q_chunk_fillers(0, qslabs0)
            q0[0]["fn"](); q0[1]["fn"]()       # qproj chains 0,1
            k0[4]["fn"](); k0[5]["fn"]()       # kslab dmas
            q0[2]["fn"]()                      # qproj chain 2
            k0[3]["fn"]()                      # comp k piece 0
            nc.sync.dma_start(out=cos_sb, in_=cosd[:])
            q0[3]["fn"]()                      # qproj chain 3
            nc.sync.dma_start(out=sin_sb, in_=sind[:])
            k0[7]["fn"](); k0[8]["fn"](); k0[10]["fn"]()   # kslab dmas
            k0[6]["fn"](); k0[9]["fn"](); k0[11]["fn"]()   # comp k 1-3 + evac
            nc.sync.dma_start(out=wv_sb, in_=wv.rearrange("k (n j) -> k n j", j=128))
            k0[12]["fn"]()                     # rope-k(0)
            for it in q0[NH:]:                 # 4 rope-q(0)
                it["fn"]()
            v0[0]["fn"](); v0[1]["fn"](); v0[2]["fn"]()    # vslab dmas
            v0[3]["fn"]()                      # comp v piece 0
            v0[4]["fn"](); v0[5]["fn"]()       # vslab dmas
            v0[6]["fn"]()                      # comp v piece 1
            v0[7]["fn"](); v0[8]["fn"]()       # vslab dmas
            v0[9]["fn"]()                      # comp v piece 2
            v0[10]["fn"]()                     # vslab dma
            v0[11]["fn"]()                     # comp v piece 3 + evac
            v0[12]["fn"](); v0[13]["fn"]()     # vtr halves

            def wo_slice_dma(nch):
                def f():
                    nc.sync.dma_start(
                        out=wo_sb[:, :, nch * TC : (nch + 1) * TC],
                        in_=wo[:, nch * TC : (nch + 1) * TC].rearrange(
                            "(n k) d -> k n d", k=128
                        ),
                    )
                return {"cost": 100, "fn": f, "dma": 1456, "kind": "dma",
                        "grp": "pre"}

            # ---------------- main loop over t-chunks ----------------
            qrope_cur = qrope0
            prev_attnT = None
            prev_t0 = 0
            carry = []          # deferred oproj fillers from chunk tcx-1
            kv_carry = []       # K/V-proj fillers spilled into their own
                                # attention window (barrier at group 2*tcx)
            for tcx in range(NTC):
                t0 = tcx * TC

                if tcx + 1 < NTC:
                    qrope_next, proj_items = proj_items_for(tcx + 1)
                else:
                    qrope_next, proj_items = None, []
                # wo: first 2 slices during attn(0) (needed by the first
                # oproj pops early in attn(1)), the rest during attn(1)
                # where the DMA queue has slack.
                if tcx == 0:
                    wos = [wo_slice_dma(n) for n in range(D // TC)]
                    proj_items = interleave(proj_items, wos[:2])
                elif tcx == 1:
                    proj_items = interleave(proj_items, wos[2:])
                oproj_items = carry + (
                    make_oproj_fillers(prev_attnT, prev_t0)
                    if prev_attnT is not None
                    else []
                )
                if DEFER_O:
                    if tcx == 1:
                        deferred_o = oproj_items
                        oproj_items = []
                    elif tcx == 2:
                        oproj_items = deferred_o + oproj_items
                fillers = kv_carry + interleave(proj_items, oproj_items)
                kv_carry = []
                # annotate each compute item with the cumulative input-DMA
                # time that precedes it in this window's queue — popping it
                # earlier than that would head-of-line block the in-order
                # PE stream on an un-arrived transfer.
                cum_dma = 0.0
                for it in fillers:
                    if it["kind"] == "dma":
                        cum_dma += it["dma"]
                    it["ready"] = cum_dma if it["kind"] == "comp" else 0.0

                nt_valid = 4 * (tcx + 1) if causal else NTT
                ngroups = nt_valid // GW
                attnT = pa.tile(
                    [128, NH, TC], BF16, tag="attnT", bufs=3, name="attnT"
                )
                budget = 0.0
                popped = 0.0
                qkpv_clock = 0.0
                act_clock = 0.0
                popped_dma = 0.0
                SLACK = float(os.environ.get("MHA_SLACK", "2000"))
                LOOKAHEAD = float(os.environ.get("MHA_LOOKAHEAD", "9000"))
                BMULT = float(os.environ.get("MHA_BMULT", "1.0"))

                def pop_fillers():
                    nonlocal popped, popped_dma
                    while popped < budget and fillers:
                        elapsed = max(act_clock, qkpv_clock + popped)
                        # pull any leading dma items (keep the queue fed,
                        # but no more than LOOKAHEAD ahead of real time)
                        i = 0
                        progress = False
                        while i < len(fillers):
                            it = fillers[i]
                            if (it["kind"] == "dma"
                                    and popped_dma < elapsed + LOOKAHEAD):
                                fillers.pop(i)
                                it["fn"]()
                                popped_dma += it["dma"]
                                progress = True
                                continue
                            if it["kind"] != "dma":
                                break
                            i += 1
                        if not fillers or popped >= budget:
                            break
                        head = fillers[0]
                        if (head["kind"] != "dma"
                                and head["ready"] <= elapsed + SLACK):
                            fillers.pop(0)
                            head["fn"]()
                            popped += head["cost"]
                            popped_dma += head["dma"]
                            progress = True
                        if not progress:
                            break
                for h in range(NH):
                    if tcx >= 1:
                        i = 0
                        while i < len(fillers):
                            it = fillers[i]
                            if (it.get("grp") == "q" and it.get("bar") == tcx
                                    and it.get("qbar", 9) <= h):
                                fillers.pop(i)
                                it["fn"]()
                                popped += it["cost"]
                                popped_dma += it["dma"]
                            else:
                                i += 1
                    with nc.named_scope("attn"):
                        pv = pvps.tile(
                            [128, 4, 256], F32, tag="pv", name="pv_ps"
                        )
                        for gg in range(ngroups):
                            if h == 0 and tcx >= 1 and gg == (
                                2 * tcx if causal else 0
                            ):
                                # force-drain this chunk's spilled K/V work:
                                # the next QK group reads the new tiles
                                i = 0
                                while i < len(fillers):
                                    if fillers[i].get("bar") == tcx:
                                        it = fillers.pop(i)
                                        it["fn"]()
                                        popped += it["cost"]
                                        popped_dma += it["dma"]
                                    else:
                                        i += 1
                            qk = qkps.tile(
                                [128, GW, TC], F32, tag="qk", name="qk_ps"
                            )
                            rels = []
                            for b in range(GW):
                                Tt = GW * gg + b
                                rel = Tt - 4 * tcx if causal else -1
                                rels.append(rel)
                                c0 = 128 * rel if rel > 0 else 0
                                nc.tensor.matmul(
                                    qk[:, b, c0:TC],
                                    kT_rope[:, Tt * 128 : (Tt + 1) * 128],
                                    qrope_cur[:, h, c0:TC],
                                    start=True,
                                    stop=True,
                                )
                            # tanh in place in PSUM, then exp to bf16 SBUF;
                            # soft-capping scales fused into ACT. Columns
                            # below the causal diagonal are skipped.
                            pt = pa.tile(
                                [128, GW, TC], BF16, tag="pt", bufs=3,
                                name="ptile",
                            )
                            act_cols = 0
                            if max(rels) <= 0:
                                nc.scalar.activation(
                                    out=qk, in_=qk, func=Tanh,
                                    scale=ATTN_MULT / CAP,
                                )
                                nc.scalar.activation(
                                    out=pt, in_=qk, func=Exp, scale=CAP
                                )
                                act_cols = GW * TC
                            else:
                                for b in range(GW):
                                    c0 = 128 * max(rels[b], 0)
                                    nc.scalar.activation(
                                        out=qk[:, b, c0:TC],
                                        in_=qk[:, b, c0:TC],
                                        func=Tanh, scale=ATTN_MULT / CAP,
                                    )
                                    nc.scalar.activation(
                                        out=pt[:, b, c0:TC],
                                        in_=qk[:, b, c0:TC],
                                        func=Exp, scale=CAP,
                                    )
                                    act_cols += TC - c0
                            for b in range(GW):
                                rel = rels[b]
                                if 0 <= rel < 4:
                                    # triangular mask on the diagonal block
                                    nc.gpsimd.tensor_mul(
                                        pt[:, b, rel * 128 : (rel + 1) * 128],
                                        pt[:, b, rel * 128 : (rel + 1) * 128],
                                        tri_sb,
                                    )
                            n_pv = 0
                            for s4 in range(4):
                                for b in range(GW):
                                    Tt = GW * gg + b
                                    rel = rels[b]
                                    if causal and rel > s4:
                                        continue
                                    n_pv += 1
                                    nc.tensor.matmul(
                                        pv[:, s4, 0:129],
                                        pt[:, b, s4 * 128 : (s4 + 1) * 128],
                                        vaug[:, Tt, 0:129],
                                        start=(
                                            gg == 0 and b == 0 and s4 % 2 == 0
                                        ),
                                        stop=(
                                            (Tt == 4 * tcx + s4)
                                            if causal
                                            else (gg == ngroups - 1
                                                  and b == GW - 1)
                                        ),
                                        skip_group_check=True,
                                    )
                            # weave fillers so PE stays busy under ACT
                            act_ns = act_cols * 2 * 0.833 + (
                                330 if max(rels) <= 0 else 660
                            )
                            qkpv_ns = (act_cols + 129 * n_pv) * 0.4167
                            act_clock += act_ns
                            qkpv_clock += qkpv_ns
                            budget += BMULT * max(act_ns - qkpv_ns, 0.0)
                            pop_fillers()
                    with nc.named_scope("attn_fin"):
                        ans = []
                        for s4 in range(4):
                            rc = pa.tile(
                                [128, 1], F32, tag="rc", bufs=4, name="rc"
                            )
                            nc.vector.reciprocal(rc, pv[:, s4, 128:129])
                            an = pa.tile(
                                [128, 128], BF16, tag="an", bufs=4, name="an"
                            )
                            nc.vector.tensor_scalar_mul(an, pv[:, s4, 0:128], rc)
                            ans.append(an)
                        # cover the DVE normalize latency with a filler
                        budget += 700
                        act_clock += 700
                        pop_fillers()
                        for s4 in range(4):
                            tp = mmps.tile([128, TC], BF16, tag="mm", name="atr")
                            nc.tensor.transpose(tp[:, :128], ans[s4], identb_sb)
                            nc.vector.tensor_copy(
                                attnT[:, h, s4 * 128 : (s4 + 1) * 128],
                                tp[:, :128],
                            )
                    if SPLIT_O and tcx == NTC - 1 and h == 1:
                        for it in make_oproj_fillers(
                            attnT, t0, jhs=[0, 1], dest=outa, dest_t0=0
                        ):
                            it["ready"] = 0.0
                            fillers.append(it)
                # drain: 'pre' items (q proj/rope of tc+1) must finish
                # before attn(tcx+1) emits its first QK; K/V items of tc+1
                # spill into attn(tcx+1) (barrier at group 2*(tcx+1)), and
                # up to MHA_CARRY oproj items carry over (attnT bufs=3).
                carry = []
                rest = fillers
                if tcx + 1 < NTC:
                    cap = int(os.environ.get("MHA_CARRY", "16"))
                    o_total = sum(1 for it in rest if it["grp"] == "o")
                    drain_o = max(0, o_total - cap)
                    drain = []
                    for it in rest:
                        if it["grp"] == "pre":
                            drain.append(it)
                        elif it["grp"] == "q":
                            if it["qbar"] == 0:
                                drain.append(it)
                            else:
                                kv_carry.append(it)
                        elif it["grp"] == "o" and drain_o > 0:
                            drain.append(it)
                            drain_o -= 1
                        elif it["grp"] == "kv" and KVSPILL:
                            kv_carry.append(it)
                        elif it["grp"] == "kv":
                            drain.append(it)
                        else:
                            carry.append(it)
                    rest = drain
                # drain with the same dma-forwarding discipline: keep
                # transfers ~LOOKAHEAD ahead of the estimated PE clock so
                # in-order compute items rarely wait on arrival.
                if not DRAIN2:
                    for it in rest:
                        it["fn"]()
                    rest = []
                el = max(act_clock, qkpv_clock + popped)
                dma_el = popped_dma
                while rest:
                    i = 0
                    while i < len(rest):
                        if (rest[i]["kind"] == "dma"
                                and dma_el < el + LOOKAHEAD):
                            it = rest.pop(i)
                            it["fn"]()
                            dma_el += it["dma"]
                            continue
                        if rest[i]["kind"] != "dma":
                            break
                        i += 1
                    if not rest:
                        break
                    it = rest.pop(0)
                    it["fn"]()
                    el = max(el, it.get("ready", 0.0)) + it["cost"]
                    dma_el += it["dma"]
                qrope_cur = qrope_next
                prev_attnT, prev_t0 = attnT, t0

            # tail: O proj pass B of the last chunk (host adds outa+outb)
            tail_items = (
                make_oproj_fillers(prev_attnT, prev_t0, jhs=[2, 3],
                                   dest=outb, dest_t0=0, evac_alt=True)
                if SPLIT_O
                else make_oproj_fillers(prev_attnT, prev_t0, evac_alt=True)
            )
            for it in carry + tail_items:
                it["fn"]()

    nc.compile()
    return nc


def vbgd_dst(vaug):
    return vaug[:, :, 128:132]


def _host_constants(T: int):
    d = KEY_SIZE
    inv_freq = 1.0 / (10000.0 ** (np.arange(0, d, 2, dtype=np.float64) / d))  # [64]
    pos = np.arange(T, dtype=np.float64)
    phase_half = pos[None, :] * inv_freq[:, None]  # [64, T]
    phase = np.concatenate([phase_half, phase_half], axis=0)  # [128, T] (tiled)
    cosT = np.cos(phase).astype(np.float32)
    sinT = np.sin(phase).astype(np.float32)

    R = np.zeros((128, 128), dtype=np.float32)
    R[:64, 64:] = -np.eye(64, dtype=np.float32)
    R[64:, :64] = np.eye(64, dtype=np.float32)
    rot = np.ascontiguousarray(R.T)

    ident = np.eye(128, dtype=np.float32)

    # tri[k, c] = 1 if k <= c (valid: query col >= key row inside the
    # diagonal 128x128 block)
    tri = (np.arange(128)[:, None] <= np.arange(128)[None, :]).astype(
        ml_dtypes.bfloat16
    )

    NTT = T // 128
    vbg = np.zeros((128, NTT, 4), dtype=ml_dtypes.bfloat16)
    vbg[:, :, 0] = 1.0
    return cosT, sinT, rot, ident, tri, vbg


_NC_CACHE: dict = {}
LAST_RESULT = None
_LAST_IN_MAPS = None


def kernel(query, key, value, mask, Wq, Wk, Wv, Wo):
    global LAST_RESULT, _LAST_IN_MAPS
    query = np.asarray(query)
    key = np.asarray(key)
    value = np.asarray(value)
    mask = np.asarray(mask)
    Wq = np.asarray(Wq, dtype=np.float32)
    Wk = np.asarray(Wk, dtype=np.float32)
    Wv = np.asarray(Wv, dtype=np.float32)
    Wo = np.asarray(Wo, dtype=np.float32)

    b, T, D = query.shape
    assert b == 1 and D == D_MODEL, (b, D)

    m2 = np.asarray(mask).reshape(T, T).astype(bool)
    if np.array_equal(m2, np.tril(np.ones((T, T), dtype=bool))):
        causal = True
    elif m2.all():
        causal = False
    else:
        raise ValueError("unsupported mask pattern (expected causal or full)")

    kkey = (T, causal)
    if kkey not in _NC_CACHE:
        _NC_CACHE[kkey] = build_nc(T, causal)
    nc = _NC_CACHE[kkey]

    pnp = ml_dtypes.bfloat16
    xq = np.ascontiguousarray(query[0].T).astype(pnp)  # [D, T]
    xk = np.ascontiguousarray(key[0].T).astype(pnp)
    xv = np.ascontiguousarray(value[0].T).astype(pnp)
    cosT, sinT, rot, ident, tri, vbg = _host_constants(T)

    JW = NH * KEY_SIZE
    NDT = D // 128

    def pack_w(w, nh):
        # [D, nh*128] -> [k, jh, n, j] flattened per-partition-contiguous
        a = np.ascontiguousarray(w).astype(pnp)
        a = a.reshape(NDT, 128, nh, 128).transpose(1, 2, 0, 3)
        return np.ascontiguousarray(a.reshape(128, nh * NDT * 128))

    in_maps = []
    for c in range(N_CORES):
        in_maps.append(
            {
                "xq": xq,
                "xk": xk,
                "xv": xv,
                "wq": pack_w(Wq[:, c * JW : (c + 1) * JW], NH),
                "wk": pack_w(Wk[:, c * KEY_SIZE : (c + 1) * KEY_SIZE], 1),
                "wv": pack_w(Wv[:, c * KEY_SIZE : (c + 1) * KEY_SIZE], 1),
                "wo": np.ascontiguousarray(Wo[c * JW : (c + 1) * JW, :]).astype(pnp),
                "cosT": cosT.astype(pnp),
                "sinT": sinT.astype(pnp),
                "rot": rot.astype(pnp),
                "identb": ident.astype(pnp),
                "tri": tri,
            }
        )

    _LAST_IN_MAPS = in_maps
    trace = os.environ.get("MHA_TRACE") == "1"
    res = run_bass_kernel_spmd(nc, in_maps, list(range(N_CORES)), trace=trace)
    LAST_RESULT = res

    out = np.zeros((T, D), dtype=np.float64)
    for c in range(N_CORES):
        out += res.results[c]["out"].astype(np.float64)
    return out.astype(np.float32).reshape(1, T, D)


# revision 37
# speedup vs baseline: 1.2080x; 1.0514x over previous
"""Trainium2 Bass kernel for nn_MultiHeadAttention_83056077570808.

GQA multi-head attention (32 q heads, 8 kv heads, d_head=128, T=2048,
D=4096) with RoPE, tanh soft-capping at 30, causal mask, fp32 reference.

Sharding: tensor-parallel over heads across 8 cores. Core c owns kv head c
and q heads 4c..4c+3: Wq/Wk/Wv column-sharded, Wo row-sharded; activations
replicated. Each core computes a partial output (its heads' contribution
through its Wo rows); the host sums the 8 partials.

Fully streamed schedule: causality means attention chunk tcx only needs
K/V tiles 0..4*tcx+3, so K/V/Q projections for chunk tcx+1 run *during*
attention of chunk tcx as filler work woven between QK groups (covering
the ACT-engine tanh/exp latency); O-proj of chunk tcx-1 likewise. DMA is
spread across the whole timeline instead of front-loaded. Fillers are
paced by an explicit cost model (popping too fast blocks the in-order PE
stream on un-arrived slab DMAs; too slow starves PE under ACT).

Causal diagonal trim: for key tile Tt in the diagonal block of chunk tcx
(rel = Tt-4*tcx in 0..3), query columns < 128*rel are entirely masked, so
QK / tanh / exp are column-trimmed, only the [128,128] diagonal block is
tri-masked (Pool), and PV skips s4-blocks with s4 < rel.

All matmuls are bf16; PSUM accumulation fp32; rope arithmetic fp32.

PSUM bank rule in the PV accumulation: start=True clears has_written for
the WHOLE bank and two s-chains share each bank, so only the bank's first
chain issues start=True; the sibling chain's first write lands on cleared
bits and overwrites.
"""

import os
import sys

for _p in ("/opt/trn_rl_repo", os.path.expanduser("~/.axon_site/_ro/trn_rl_repo")):
    if os.path.isdir(_p) and _p not in sys.path:
        sys.path.insert(0, _p)

import numpy as np
import ml_dtypes

import concourse.bass as bass
import concourse.tile as tile
from concourse import bacc, mybir
from concourse.bass_utils import run_bass_kernel_spmd

F32 = mybir.dt.float32
BF16 = mybir.dt.bfloat16

D_MODEL = 4096
KEY_SIZE = 128
NUM_Q_HEADS = 32
NUM_KV_HEADS = 8
N_CORES = 8
NH = NUM_Q_HEADS // NUM_KV_HEADS  # q heads per core = 4
ATTN_MULT = 0.08838834764831845
CAP = 30.0

Tanh = mybir.ActivationFunctionType.Tanh
Exp = mybir.ActivationFunctionType.Exp


def build_nc(T: int, causal: bool):
    """Emit the Bass program for one core (SPMD: all cores run this).

    Tile builds a STATIC per-engine schedule in (priority = emission)
    order, so overlap must be engineered in the emission order itself.
    """
    D = D_MODEL
    TC = 512                 # t-chunk width
    NTC = T // TC            # t-chunks
    NTT = T // 128           # 128-tiles along T (key side)
    NDT = D // 128           # contraction tiles over d_model = 32
    JW = NH * KEY_SIZE       # per-core q/o width = 512
    GW = 2                   # key tiles per QK group (1 PSUM bank each)

    SPLIT_O = os.environ.get("MHA_SPLIT_O", "0") == "1"
    EVAC_ALT = os.environ.get("MHA_EVAC_ALT", "0") == "1"
    PROJ_ORDER2 = os.environ.get("MHA_PORDER2", "0") == "1"
    DRAIN2 = os.environ.get("MHA_DRAIN2", "0") == "1"
    DEFER_O = os.environ.get("MHA_DEFER_O", "0") == "1"
    KVSPILL = os.environ.get("MHA_KVSPILL", "1") == "1"
    QSPILL = os.environ.get("MHA_QSPILL", "0") == "1"

    nc = bacc.Bacc(None, target_bir_lowering=False)

    xq = nc.dram_tensor("xq", [D, T], BF16, kind="ExternalInput")
    xk = nc.dram_tensor("xk", [D, T], BF16, kind="ExternalInput")
    xv = nc.dram_tensor("xv", [D, T], BF16, kind="ExternalInput")
    wq = nc.dram_tensor("wq", [128, NH * NDT * 128], BF16, kind="ExternalInput")
    wk = nc.dram_tensor("wk", [128, NDT * 128], BF16, kind="ExternalInput")
    wv = nc.dram_tensor("wv", [128, NDT * 128], BF16, kind="ExternalInput")
    wo = nc.dram_tensor("wo", [JW, D], BF16, kind="ExternalInput")
    cosd = nc.dram_tensor("cosT", [128, T], BF16, kind="ExternalInput")
    sind = nc.dram_tensor("sinT", [128, T], BF16, kind="ExternalInput")
    rotd = nc.dram_tensor("rot", [128, 128], BF16, kind="ExternalInput")
    identbd = nc.dram_tensor("identb", [128, 128], BF16, kind="ExternalInput")
    trid = nc.dram_tensor("tri", [128, 128], BF16, kind="ExternalInput")
    outd = nc.dram_tensor("out", [T, D], BF16, kind="ExternalOutput")
    outa = nc.dram_tensor("outa", [TC, D], BF16, kind="ExternalOutput")
    outb = nc.dram_tensor("outb", [TC, D], BF16, kind="ExternalOutput")

    with tile.TileContext(nc) as tc:
        with (
            tc.tile_pool(name="const", bufs=1) as constp,
            tc.tile_pool(name="persist", bufs=1) as persist,
            tc.tile_pool(name="slabs", bufs=2) as slabp,
            tc.tile_pool(name="tmps", bufs=2) as tmpp,
            tc.tile_pool(name="pa", bufs=2) as pa,
            tc.tile_pool(name="qkps", bufs=2, space="PSUM") as qkps,
            tc.tile_pool(name="pvps", bufs=1, space="PSUM") as pvps,
            tc.tile_pool(name="mmps", bufs=2, space="PSUM") as mmps,
        ):
            # ---- persistent SBUF ----
            rot_sb = constp.tile([128, 128], BF16)
            identb_sb = constp.tile([128, 128], BF16)
            tri_sb = constp.tile([128, 128], BF16)
            cos_sb = constp.tile([128, T], BF16)
            sin_sb = constp.tile([128, T], BF16)
            kT_rope = persist.tile([128, T], BF16)
            vaug = persist.tile([128, NTT, 132], BF16)
            wq_sb = persist.tile([128, NH, NDT, 128], BF16)
            wk_sb = persist.tile([128, NDT, 128], BF16)
            wv_sb = persist.tile([128, NDT, 128], BF16)
            wo_sb = persist.tile([128, NH, D], BF16)

            # ---- tiny consts first (clears the DMA queue fast) ----
            nc.sync.dma_start(out=rot_sb, in_=rotd[:])
            nc.sync.dma_start(out=identb_sb, in_=identbd[:])
            nc.sync.dma_start(out=tri_sb, in_=trid[:])
            nc.any.memset(vaug[:, :, 128:132], 1.0)

            # ---------------- emit-helper closures ----------------
            # Filler items are (cost_ns, fn) pairs.

            def kv_fillers(xsrc, w_sb, tch, dst_cb):
                """K or V projection of t-columns [tch*512,(tch+1)*512)."""
                st = {}

                def dma_i(i, half):
                    def f():
                        if half == 0:
                            st[i] = slabp.tile(
                                [128, 8, TC], BF16, tag="kvslab", bufs=3,
                                name="kvslab",
                            )
                        nc.sync.dma_start(
                            out=st[i][:, 4 * half : 4 * half + 4, :],
                            in_=xsrc[
                                i * 1024 + half * 512 :
                                i * 1024 + (half + 1) * 512,
                                tch * TC : (tch + 1) * TC,
                            ].rearrange("(n k) t -> k n t", k=128),
                        )
                    return {"cost": 100, "fn": f, "dma": 1456, "kind": "dma",
                            "grp": "kv", "bar": tch}

                def comp_i(i):
                    def f():
                        if i == 0:
                            st["ps"] = mmps.tile(
                                [128, TC], F32, tag="mm", name="kv_ps"
                            )
                        ps = st["ps"]
                        for j in range(8):
                            nc.tensor.matmul(
                                ps,
                                w_sb[:, i * 8 + j, :],
                                st[i][:, j, :],
                                start=(i == 0 and j == 0),
                                stop=(i == 3 and j == 7),
                            )
                        if i == 3:
                            dst_cb(ps)
                    return {"cost": 1710, "fn": f, "dma": 0, "kind": "comp",
                            "grp": "kv", "bar": tch}

                return [dma_i(0, 0), dma_i(0, 1), dma_i(1, 0), comp_i(0),
                        dma_i(1, 1), dma_i(2, 0), comp_i(1), dma_i(2, 1),
                        dma_i(3, 0), comp_i(2), dma_i(3, 1), comp_i(3)]

            def rope(dst, src, t0, tw):
                """dst[128, tw] = RoPE(src[128, tw]) at positions t0.. (fp32
                math; src/dst bf16)."""
                rp = mmps.tile([128, TC], F32, tag="mm", name="rope_ps")
                nc.tensor.matmul(rp[:, :tw], rot_sb, src, start=True, stop=True)
                t1 = pa.tile([128, TC], F32, tag="rt1", bufs=1, name="rope_t1")
                nc.gpsimd.tensor_mul(t1[:, :tw], src, cos_sb[:, t0 : t0 + tw])
                t2 = pa.tile([128, TC], F32, tag="rt2", bufs=1, name="rope_t2")
                nc.vector.tensor_mul(t2[:, :tw], rp[:, :tw], sin_sb[:, t0 : t0 + tw])
                nc.vector.tensor_add(dst, t1[:, :tw], t2[:, :tw])

            def k_chunk_fillers(tch):
                ktmp = tmpp.tile([128, TC], BF16, tag="ktmp", name="ktmp")

                def evac(ps):
                    nc.vector.tensor_copy(ktmp, ps)

                items = kv_fillers(xk, wk_sb, tch, evac)

                def rope_k():
                    rope(kT_rope[:, tch * TC : (tch + 1) * TC], ktmp,
                         tch * TC, TC)

                return items + [{"cost": 350, "fn": rope_k, "dma": 0,
                                 "kind": "comp", "grp": "kv", "bar": tch}]

            def v_chunk_fillers(tch):
                vtmp = tmpp.tile([128, TC], BF16, tag="vtmp", name="vtmp")

                def evac(ps):
                    nc.vector.tensor_copy(vtmp, ps)

                items = kv_fillers(xv, wv_sb, tch, evac)

                def vtr(half):
                    def f():
                        for b2 in range(2):
                            b = 4 * tch + 2 * half + b2
                            tp = mmps.tile(
                                [128, TC], BF16, tag="mm", name="vtr_ps"
                            )
                            nc.tensor.transpose(
                                tp[:, :128],
                                vtmp[:, (2 * half + b2) * 128 :
                                     (2 * half + b2 + 1) * 128],
                                identb_sb,
                            )
                            nc.vector.tensor_copy(vaug[:, b, 0:128], tp[:, :128])
                    return {"cost": 220, "fn": f, "dma": 0, "kind": "comp",
                            "grp": "kv", "bar": tch}

                return items + [vtr(0), vtr(1)]

            def qslab_dma_fillers(tcx):
                slabs = []

                def dma_h(dh, q):
                    def f():
                        if q == 0:
                            slab = slabp.tile(
                                [128, 16, TC], BF16, tag="qslab", name="qslab"
                            )
                            slabs.append(slab)
                        slab = slabs[dh]
                        nc.sync.dma_start(
                            out=slab[:, 4 * q : 4 * q + 4, :],
                            in_=xq[
                                dh * 2048 + q * 512 : dh * 2048 + (q + 1) * 512,
                                tcx * TC : (tcx + 1) * TC,
                            ].rearrange("(n k) t -> k n t", k=128),
                        )
                    return {"cost": 100, "fn": f, "dma": 1456, "kind": "dma",
                            "grp": "pre"}

                return slabs, [dma_h(0, q) for q in range(4)] + [
                    dma_h(1, q) for q in range(4)
                ]

            def qproj_chain(slabs, qraw, jh):
                ps = mmps.tile([128, TC], F32, tag="mm", name="q_ps")
                for dh in range(2):
                    for i in range(16):
                        nc.tensor.matmul(
                            ps,
                            wq_sb[:, jh, dh * 16 + i, :],
                            slabs[dh][:, i, :],
                            start=(dh == 0 and i == 0),
                            stop=(dh == 1 and i == 15),
                        )
                nc.vector.tensor_copy(qraw[:, jh, :], ps)

            def q_chunk_fillers(tcx, slabs):
                """Q proj + rope for chunk tcx; returns (qrope, items)."""
                qraw = tmpp.tile([128, NH, TC], BF16, tag="qraw", name="qraw")
                qrope = tmpp.tile([128, NH, TC], BF16, tag="qrope", bufs=2, name="qrope")
                items = []
                for jh in range(NH):
                    items.append({
                        "cost": 6830, "dma": 0, "kind": "comp", "grp": "q",
                        "bar": tcx, "qbar": jh,
                        "fn": lambda jh=jh: qproj_chain(slabs, qraw, jh),
                    })
                for jh in range(NH):
                    items.append({
                        "cost": 350, "dma": 0, "kind": "comp", "grp": "q",
                        "bar": tcx, "qbar": jh,
                        "fn": lambda jh=jh: rope(
                            qrope[:, jh, :], qraw[:, jh, :], tcx * TC, TC
                        ),
                    })
                return qrope, items

            def make_oproj_fillers(attnT, t0, jhs=range(NH), dest=None,
                                   dest_t0=None, evac_alt=False):
                dest = outd if dest is None else dest
                dest_t0 = t0 if dest_t0 is None else dest_t0
                jhs = list(jhs)
                fillers = []
                for nch in range(D // TC):
                    for s4 in range(4):
                        def f(s4=s4, nch=nch):
                            with nc.named_scope("oproj"):
                                ps = mmps.tile(
                                    [128, TC], F32, tag="mm", name="o_ps"
                                )
                                for x, jh in enumerate(jhs):
                                    nc.tensor.matmul(
                                        ps,
                                        attnT[:, jh, s4 * 128 : (s4 + 1) * 128],
                                        wo_sb[:, jh, nch * TC : (nch + 1) * TC],
                                        start=(x == 0),
                                        stop=(x == len(jhs) - 1),
                                    )
                                osb = pa.tile(
                                    [128, TC], BF16, tag="osb", bufs=4,
                                    name="osb",
                                )
                                if evac_alt and (s4 + nch) % 2 == 0:
                                    nc.scalar.copy(out=osb, in_=ps)
                                else:
                                    nc.vector.tensor_copy(osb, ps)
                                nc.sync.dma_start(
                                    out=dest[
                                        dest_t0 + s4 * 128 :
                                        dest_t0 + (s4 + 1) * 128,
                                        nch * TC : (nch + 1) * TC,
                                    ],
                                    in_=osb,
                                )
                        fillers.append(
                            {"cost": 218 * len(jhs), "fn": f,
                             "dma": 364, "kind": "oproj", "grp": "o"}
                        )
                return fillers

            def interleave(a, b):
                out = []
                ia = ib = 0
                na, nb = len(a), len(b)
                while ia < na or ib < nb:
                    if ia * max(nb, 1) <= ib * max(na, 1) and ia < na:
                        out.append(a[ia]); ia += 1
                    elif ib < nb:
                        out.append(b[ib]); ib += 1
                    else:
                        out.append(a[ia]); ia += 1
                return out

            def proj_items_for(tcx):
                """All projection work for chunk tcx as a filler list, DMA
                items placed so transfers land just ahead of their use."""
                slabs_n, qdma = qslab_dma_fillers(tcx)
                kn = k_chunk_fillers(tcx)
                vn = v_chunk_fillers(tcx)
                qrope_n, qn = q_chunk_fillers(tcx, slabs_n)
                qpairs = [qn[0], qn[NH], qn[1], qn[NH + 1], qn[2],
                          qn[NH + 2], qn[3], qn[NH + 3]]
                items = (
                    [kn[0], kn[1], qdma[0], qdma[1], kn[2], kn[3], qdma[2],
                     qdma[3], kn[4], kn[5], qdma[4], qdma[5], kn[6], kn[7],
                     qdma[6], qdma[7], kn[8], kn[9], kn[10], kn[11], kn[12]]
                    + vn[:12] + [vn[12], vn[13]]
                    + qpairs
                )
                return qrope_n, items

            # ---------------- chunk 0 prologue (inline, DMA-ordered) ----
            # Critical path to the first q chain: wq head 0 + both qslabs;
            # everything else (k/v slabs, cos/sin) streams behind and PE
            # picks it up between/after the q chains.
            def wq_head_dma(jh):
                nc.sync.dma_start(
                    out=wq_sb[:, jh, :, :],
                    in_=wq[:, jh * NDT * 128 : (jh + 1) * NDT * 128].rearrange(
                        "k (n j) -> k n j", j=128
                    ),
                )

            k0 = k_chunk_fillers(0)
            v0 = v_chunk_fillers(0)
            qslabs0, qdma0 = qslab_dma_fillers(0)
            wq_head_dma(0)
            for it in qdma0:                   # 8 quarter-slab dmas
                it["fn"]()
            wq_head_dma(1)
            nc.sync.dma_start(out=wk_sb, in_=wk.rearrange("k (n j) -> k n j", j=128))
            wq_head_dma(2)
            for it in k0[0:3]:                 # kslab dmas
                it["fn"]()
            wq_head_dma(3)
            qrope0, q0 = q_chunk_fillers(0, qslabs0)
            q0[0]["fn"](); q0[1]["fn"]()       # qproj chains 0,1
            k0[4]["fn"](); k0[5]["fn"]()       # kslab dmas
            q0[2]["fn"]()                      # qproj chain 2
            k0[3]["fn"]()                      # comp k piece 0
            nc.sync.dma_start(out=cos_sb, in_=cosd[:])
            q0[3]["fn"]()                      # qproj chain 3
            nc.sync.dma_start(out=sin_sb, in_=sind[:])
            k0[7]["fn"](); k0[8]["fn"](); k0[10]["fn"]()   # kslab dmas
            k0[6]["fn"](); k0[9]["fn"](); k0[11]["fn"]()   # comp k 1-3 + evac
            nc.sync.dma_start(out=wv_sb, in_=wv.rearrange("k (n j) -> k n j", j=128))
            k0[12]["fn"]()                     # rope-k(0)
            for it in q0[NH:]:                 # 4 rope-q(0)
                it["fn"]()
            v0[0]["fn"](); v0[1]["fn"](); v0[2]["fn"]()    # vslab dmas
            v0[3]["fn"]()                      # comp v piece 0
            v0[4]["fn"](); v0[5]["fn"]()       # vslab dmas
            v0[6]["fn"]()                      # comp v piece 1
            v0[7]["fn"](); v0[8]["fn"]()       # vslab dmas
            v0[9]["fn"]()                      # comp v piece 2
            v0[10]["fn"]()                     # vslab dma
            v0[11]["fn"]()                     # comp v piece 3 + evac
            v0[12]["fn"](); v0[13]["fn"]()     # vtr halves

            def wo_slice_dma(nch):
                def f():
                    nc.sync.dma_start(
                        out=wo_sb[:, :, nch * TC : (nch + 1) * TC],
                        in_=wo[:, nch * TC : (nch + 1) * TC].rearrange(
                            "(n k) d -> k n d", k=128
                        ),
                    )
                return {"cost": 100, "fn": f, "dma": 1456, "kind": "dma",
                        "grp": "pre"}

            # ---------------- main loop over t-chunks ----------------
            qrope_cur = qrope0
            prev_attnT = None
            prev_t0 = 0
            carry = []          # deferred oproj fillers from chunk tcx-1
            kv_carry = []       # K/V-proj fillers spilled into their own
                                # attention window (barrier at group 2*tcx)
            for tcx in range(NTC):
                t0 = tcx * TC

                if tcx + 1 < NTC:
                    qrope_next, proj_items = proj_items_for(tcx + 1)
                else:
                    qrope_next, proj_items = None, []
                # wo: first 2 slices during attn(0) (needed by the first
                # oproj pops early in attn(1)), the rest during attn(1)
                # where the DMA queue has slack.
                if tcx == 0:
                    wos = [wo_slice_dma(n) for n in range(D // TC)]
                    proj_items = interleave(proj_items, wos[:2])
                elif tcx == 1:
                    proj_items = interleave(proj_items, wos[2:])
                oproj_items = carry + (
                    make_oproj_fillers(prev_attnT, prev_t0)
                    if prev_attnT is not None
                    else []
                )
                if DEFER_O:
                    if tcx == 1:
                        deferred_o = oproj_items
                        oproj_items = []
                    elif tcx == 2:
                        oproj_items = deferred_o + oproj_items
                fillers = kv_carry + interleave(proj_items, oproj_items)
                kv_carry = []
                # annotate each compute item with the cumulative input-DMA
                # time that precedes it in this window's queue — popping it
                # earlier than that would head-of-line block the in-order
                # PE stream on an un-arrived transfer.
                cum_dma = 0.0
                for it in fillers:
                    if it["kind"] == "dma":
                        cum_dma += it["dma"]
                    it["ready"] = cum_dma if it["kind"] == "comp" else 0.0

                nt_valid = 4 * (tcx + 1) if causal else NTT
                ngroups = nt_valid // GW
                attnT = pa.tile(
                    [128, NH, TC], BF16, tag="attnT", bufs=3, name="attnT"
                )
                budget = 0.0
                popped = 0.0
                qkpv_clock = 0.0
                act_clock = 0.0
                popped_dma = 0.0
                SLACK = float(os.environ.get("MHA_SLACK", "2000"))
                LOOKAHEAD = float(os.environ.get("MHA_LOOKAHEAD", "9000"))
                BMULT = float(os.environ.get("MHA_BMULT", "1.0"))

                def pop_fillers():
                    nonlocal popped, popped_dma
                    while popped < budget and fillers:
                        elapsed = max(act_clock, qkpv_clock + popped)
                        # pull any leading dma items (keep the queue fed,
                        # but no more than LOOKAHEAD ahead of real time)
                        i = 0
                        progress = False
                        while i < len(fillers):
                            it = fillers[i]
                            if (it["kind"] == "dma"
                                    and popped_dma < elapsed + LOOKAHEAD):
                                fillers.pop(i)
                                it["fn"]()
                                popped_dma += it["dma"]
                                progress = True
                                continue
                            if it["kind"] != "dma":
                                break
                            i += 1
                        if not fillers or popped >= budget:
                            break
                        head = fillers[0]
                        if (head["kind"] != "dma"
                                and head["ready"] <= elapsed + SLACK):
                            fillers.pop(0)
                            head["fn"]()
                            popped += head["cost"]
                            popped_dma += head["dma"]
                            progress = True
                        if not progress:
                            break
                for h in range(NH):
                    if tcx >= 1:
                        i = 0
                        while i < len(fillers):
                            it = fillers[i]
                            if (it.get("grp") == "q" and it.get("bar") == tcx
                                    and it.get("qbar", 9) <= h):
                                fillers.pop(i)
                                it["fn"]()
                                popped += it["cost"]
                                popped_dma += it["dma"]
                            else:
                                i += 1
                    with nc.named_scope("attn"):
                        pv = pvps.tile(
                            [128, 4, 256], F32, tag="pv", name="pv_ps"
                        )
                        for gg in range(ngroups):
                            if h == 0 and tcx >= 1 and gg == (
                                2 * tcx if causal else 0
                            ):
                                # force-drain this chunk's spilled K/V work:
                                # the next QK group reads the new tiles
                                i = 0
                                while i < len(fillers):
                                    if fillers[i].get("bar") == tcx:
                                        it = fillers.pop(i)
                                        it["fn"]()
                                        popped += it["cost"]
                                        popped_dma += it["dma"]
                                    else:
                                        i += 1
                            qk = qkps.tile(
                                [128, GW, TC], F32, tag="qk", name="qk_ps"
                            )
                            rels = []
                            for b in range(GW):
                                Tt = GW * gg + b
                                rel = Tt - 4 * tcx if causal else -1
                                rels.append(rel)
                                c0 = 128 * rel if rel > 0 else 0
                                nc.tensor.matmul(
                                    qk[:, b, c0:TC],
                                    kT_rope[:, Tt * 128 : (Tt + 1) * 128],
                                    qrope_cur[:, h, c0:TC],
                                    start=True,
                                    stop=True,
                                )
                            # tanh in place in PSUM, then exp to bf16 SBUF;
                            # soft-capping scales fused into ACT. Columns
                            # below the causal diagonal are skipped.
                            pt = pa.tile(
                                [128, GW, TC], BF16, tag="pt", bufs=3,
                                name="ptile",
                            )
                            act_cols = 0
                            if max(rels) <= 0:
                                nc.scalar.activation(
                                    out=qk, in_=qk, func=Tanh,
                                    scale=ATTN_MULT / CAP,
                                )
                                nc.scalar.activation(
                                    out=pt, in_=qk, func=Exp, scale=CAP
                                )
                                act_cols = GW * TC
                            else:
                                for b in range(GW):
                                    c0 = 128 * max(rels[b], 0)
                                    nc.scalar.activation(
                                        out=qk[:, b, c0:TC],
                                        in_=qk[:, b, c0:TC],
                                        func=Tanh, scale=ATTN_MULT / CAP,
                                    )
                                    nc.scalar.activation(
                                        out=pt[:, b, c0:TC],
                                        in_=qk[:, b, c0:TC],
                                        func=Exp, scale=CAP,
                                    )
                                    act_cols += TC - c0
                            for b in range(GW):
                                rel = rels[b]
                                if 0 <= rel < 4:
                                    # triangular mask on the diagonal block
                                    nc.gpsimd.tensor_mul(
                                        pt[:, b, rel * 128 : (rel + 1) * 128],
                                        pt[:, b, rel * 128 : (rel + 1) * 128],
                                        tri_sb,
                                    )
                            n_pv = 0
                            for s4 in range(4):
                                for b in range(GW):
                                    Tt = GW * gg + b
                                    rel = rels[b]
                                    if causal and rel > s4:
                                        continue
                                    n_pv += 1
                                    nc.tensor.matmul(
                                        pv[:, s4, 0:129],
                                        pt[:, b, s4 * 128 : (s4 + 1) * 128],
                                        vaug[:, Tt, 0:129],
                                        start=(
                                            gg == 0 and b == 0 and s4 % 2 == 0
                                        ),
                                        stop=(
                                            (Tt == 4 * tcx + s4)
                                            if causal
                                            else (gg == ngroups - 1
                                                  and b == GW - 1)
                                        ),
                                        skip_group_check=True,
                                    )
                            # weave fillers so PE stays busy under ACT
                            act_ns = act_cols * 2 * 0.833 + (
                                330 if max(rels) <= 0 else 660
                            )
                            qkpv_ns = (act_cols + 129 * n_pv) * 0.4167
                            act_clock += act_ns
                            qkpv_clock += qkpv_ns
                            budget += BMULT * max(act_ns - qkpv_ns, 0.0)
                            pop_fillers()
                    with nc.named_scope("attn_fin"):
                        ans = []
                        for s4 in range(4):
                            rc = pa.tile(
                                [128, 1], F32, tag="rc", bufs=4, name="rc"
                            )
                            nc.vector.reciprocal(rc, pv[:, s4, 128:129])
                            an = pa.tile(
                                [128, 128], BF16, tag="an", bufs=4, name="an"
                            )
                            nc.vector.tensor_scalar_mul(an, pv[:, s4, 0:128], rc)
                            ans.append(an)
                        # cover the DVE normalize latency with a filler
                        budget += 700
                        act_clock += 700
                        pop_fillers()
                        for s4 in range(4):
                            tp = mmps.tile([128, TC], BF16, tag="mm", name="atr")
                            nc.tensor.transpose(tp[:, :128], ans[s4], identb_sb)
                            nc.vector.tensor_copy(
                                attnT[:, h, s4 * 128 : (s4 + 1) * 128],
                                tp[:, :128],
                            )
                    if SPLIT_O and tcx == NTC - 1 and h == 1:
                        for it in make_oproj_fillers(
                            attnT, t0, jhs=[0, 1], dest=outa, dest_t0=0
                        ):
                            it["ready"] = 0.0
                            fillers.append(it)
                # drain: 'pre' items (q proj/rope of tc+1) must finish
                # before attn(tcx+1) emits its first QK; K/V items of tc+1
                # spill into attn(tcx+1) (barrier at group 2*(tcx+1)), and
                # up to MHA_CARRY oproj items carry over (attnT bufs=3).
                carry = []
                rest = fillers
                if tcx + 1 < NTC:
                    cap = int(os.environ.get("MHA_CARRY", "16"))
                    o_total = sum(1 for it in rest if it["grp"] == "o")
                    drain_o = max(0, o_total - cap)
                    drain = []
                    for it in rest:
                        if it["grp"] == "pre":
                            drain.append(it)
                        elif it["grp"] == "q":
                            if it["qbar"] == 0 or not QSPILL:
                                drain.append(it)
                            else:
                                kv_carry.append(it)
                        elif it["grp"] == "o" and drain_o > 0:
                            drain.append(it)
                            drain_o -= 1
                        elif it["grp"] == "kv" and KVSPILL:
                            kv_carry.append(it)
                        elif it["grp"] == "kv":
                            drain.append(it)
                        else:
                            carry.append(it)
                    rest = drain
                # drain with the same dma-forwarding discipline: keep
                # transfers ~LOOKAHEAD ahead of the estimated PE clock so
                # in-order compute items rarely wait on arrival.
                if not DRAIN2:
                    for it in rest:
                        it["fn"]()
                    rest = []
                el = max(act_clock, qkpv_clock + popped)
                dma_el = popped_dma
                while rest:
                    i = 0
                    while i < len(rest):
                        if (rest[i]["kind"] == "dma"
                                and dma_el < el + LOOKAHEAD):
                            it = rest.pop(i)
                            it["fn"]()
                            dma_el += it["dma"]
                            continue
                        if rest[i]["kind"] != "dma":
                            break
                        i += 1
                    if not rest:
                        break
                    it = rest.pop(0)
                    it["fn"]()
                    el = max(el, it.get("ready", 0.0)) + it["cost"]
                    dma_el += it["dma"]
                qrope_cur = qrope_next
                prev_attnT, prev_t0 = attnT, t0

            # tail: O proj pass B of the last chunk (host adds outa+outb)
            tail_items = (
                make_oproj_fillers(prev_attnT, prev_t0, jhs=[2, 3],
                                   dest=outb, dest_t0=0, evac_alt=True)
                if SPLIT_O
                else make_oproj_fillers(prev_attnT, prev_t0, evac_alt=True)
            )
            for it in carry + tail_items:
                it["fn"]()

    nc.compile()
    return nc


def vbgd_dst(vaug):
    return vaug[:, :, 128:132]


def _host_constants(T: int):
    d = KEY_SIZE
    inv_freq = 1.0 / (10000.0 ** (np.arange(0, d, 2, dtype=np.float64) / d))  # [64]
    pos = np.arange(T, dtype=np.float64)
    phase_half = pos[None, :] * inv_freq[:, None]  # [64, T]
    phase = np.concatenate([phase_half, phase_half], axis=0)  # [128, T] (tiled)
    cosT = np.cos(phase).astype(np.float32)
    sinT = np.sin(phase).astype(np.float32)

    R = np.zeros((128, 128), dtype=np.float32)
    R[:64, 64:] = -np.eye(64, dtype=np.float32)
    R[64:, :64] = np.eye(64, dtype=np.float32)
    rot = np.ascontiguousarray(R.T)

    ident = np.eye(128, dtype=np.float32)

    # tri[k, c] = 1 if k <= c (valid: query col >= key row inside the
    # diagonal 128x128 block)
    tri = (np.arange(128)[:, None] <= np.arange(128)[None, :]).astype(
        ml_dtypes.bfloat16
    )

    NTT = T // 128
    vbg = np.zeros((128, NTT, 4), dtype=ml_dtypes.bfloat16)
    vbg[:, :, 0] = 1.0
    return cosT, sinT, rot, ident, tri, vbg


_NC_CACHE: dict = {}
LAST_RESULT = None
_LAST_IN_MAPS = None


def kernel(query, key, value, mask, Wq, Wk, Wv, Wo):
    global LAST_RESULT, _LAST_IN_MAPS
    query = np.asarray(query)
    key = np.asarray(key)
    value = np.asarray(value)
    mask = np.asarray(mask)
    Wq = np.asarray(Wq, dtype=np.float32)
    Wk = np.asarray(Wk, dtype=np.float32)
    Wv = np.asarray(Wv, dtype=np.float32)
    Wo = np.asarray(Wo, dtype=np.float32)

    b, T, D = query.shape
    assert b == 1 and D == D_MODEL, (b, D)

    m2 = np.asarray(mask).reshape(T, T).astype(bool)
    if np.array_equal(m2, np.tril(np.ones((T, T), dtype=bool))):
        causal = True
    elif m2.all():
        causal = False
    else:
        raise ValueError("unsupported mask pattern (expected causal or full)")

    kkey = (T, causal)
    if kkey not in _NC_CACHE:
        _NC_CACHE[kkey] = build_nc(T, causal)
    nc = _NC_CACHE[kkey]

    pnp = ml_dtypes.bfloat16
    xq = np.ascontiguousarray(query[0].T).astype(pnp)  # [D, T]
    xk = np.ascontiguousarray(key[0].T).astype(pnp)
    xv = np.ascontiguousarray(value[0].T).astype(pnp)
    cosT, sinT, rot, ident, tri, vbg = _host_constants(T)

    JW = NH * KEY_SIZE
    NDT = D // 128

    def pack_w(w, nh):
        # [D, nh*128] -> [k, jh, n, j] flattened per-partition-contiguous
        a = np.ascontiguousarray(w).astype(pnp)
        a = a.reshape(NDT, 128, nh, 128).transpose(1, 2, 0, 3)
        return np.ascontiguousarray(a.reshape(128, nh * NDT * 128))

    in_maps = []
    for c in range(N_CORES):
        in_maps.append(
            {
                "xq": xq,
                "xk": xk,
                "xv": xv,
                "wq": pack_w(Wq[:, c * JW : (c + 1) * JW], NH),
                "wk": pack_w(Wk[:, c * KEY_SIZE : (c + 1) * KEY_SIZE], 1),
                "wv": pack_w(Wv[:, c * KEY_SIZE : (c + 1) * KEY_SIZE], 1),
                "wo": np.ascontiguousarray(Wo[c * JW : (c + 1) * JW, :]).astype(pnp),
                "cosT": cosT.astype(pnp),
                "sinT": sinT.astype(pnp),
                "rot": rot.astype(pnp),
                "identb": ident.astype(pnp),
                "tri": tri,
            }
        )

    _LAST_IN_MAPS = in_maps
    trace = os.environ.get("MHA_TRACE") == "1"
    res = run_bass_kernel_spmd(nc, in_maps, list(range(N_CORES)), trace=trace)
    LAST_RESULT = res

    out = np.zeros((T, D), dtype=np.float64)
    for c in range(N_CORES):
        out += res.results[c]["out"].astype(np.float64)
    return out.astype(np.float32).reshape(1, T, D)


# revision 43
# speedup vs baseline: 1.2355x; 1.0228x over previous
"""Trainium2 Bass kernel for nn_MultiHeadAttention_83056077570808.

GQA multi-head attention (32 q heads, 8 kv heads, d_head=128, T=2048,
D=4096) with RoPE, tanh soft-capping at 30, causal mask, fp32 reference.

Sharding: tensor-parallel over heads across 8 cores. Core c owns kv head c
and q heads 4c..4c+3: Wq/Wk/Wv column-sharded, Wo row-sharded; activations
replicated. Each core computes a partial output (its heads' contribution
through its Wo rows); the host sums the 8 partials.

Fully streamed schedule: causality means attention chunk tcx only needs
K/V tiles 0..4*tcx+3, so K/V/Q projections for chunk tcx+1 run *during*
attention of chunk tcx as filler work woven between QK groups (covering
the ACT-engine tanh/exp latency); O-proj of chunk tcx-1 likewise. DMA is
spread across the whole timeline instead of front-loaded. Fillers are
paced by an explicit cost model (popping too fast blocks the in-order PE
stream on un-arrived slab DMAs; too slow starves PE under ACT).

Causal diagonal trim: for key tile Tt in the diagonal block of chunk tcx
(rel = Tt-4*tcx in 0..3), query columns < 128*rel are entirely masked, so
QK / tanh / exp are column-trimmed, only the [128,128] diagonal block is
tri-masked (Pool), and PV skips s4-blocks with s4 < rel.

All matmuls are bf16; PSUM accumulation fp32; rope arithmetic fp32.

PSUM bank rule in the PV accumulation: start=True clears has_written for
the WHOLE bank and two s-chains share each bank, so only the bank's first
chain issues start=True; the sibling chain's first write lands on cleared
bits and overwrites.
"""

import os
import sys

for _p in ("/opt/trn_rl_repo", os.path.expanduser("~/.axon_site/_ro/trn_rl_repo")):
    if os.path.isdir(_p) and _p not in sys.path:
        sys.path.insert(0, _p)

import numpy as np
import ml_dtypes

import concourse.bass as bass
import concourse.tile as tile
from concourse import bacc, mybir
from concourse.bass_utils import run_bass_kernel_spmd

F32 = mybir.dt.float32
BF16 = mybir.dt.bfloat16

D_MODEL = 4096
KEY_SIZE = 128
NUM_Q_HEADS = 32
NUM_KV_HEADS = 8
N_CORES = 8
NH = NUM_Q_HEADS // NUM_KV_HEADS  # q heads per core = 4
ATTN_MULT = 0.08838834764831845
CAP = 30.0

Tanh = mybir.ActivationFunctionType.Tanh
Exp = mybir.ActivationFunctionType.Exp


def build_nc(T: int, causal: bool):
    """Emit the Bass program for one core (SPMD: all cores run this).

    Tile builds a STATIC per-engine schedule in (priority = emission)
    order, so overlap must be engineered in the emission order itself.
    """
    D = D_MODEL
    TC = 512                 # t-chunk width
    NTC = T // TC            # t-chunks
    NTT = T // 128           # 128-tiles along T (key side)
    NDT = D // 128           # contraction tiles over d_model = 32
    JW = NH * KEY_SIZE       # per-core q/o width = 512
    GW = 2                   # key tiles per QK group (1 PSUM bank each)

    SPLIT_O = os.environ.get("MHA_SPLIT_O", "0") == "1"
    EVAC_ALT = os.environ.get("MHA_EVAC_ALT", "0") == "1"
    PROJ_ORDER2 = os.environ.get("MHA_PORDER2", "0") == "1"
    DRAIN2 = os.environ.get("MHA_DRAIN2", "0") == "1"
    DEFER_O = os.environ.get("MHA_DEFER_O", "0") == "1"
    KVSPILL = os.environ.get("MHA_KVSPILL", "1") == "1"
    QSPILL = os.environ.get("MHA_QSPILL", "0") == "1"
    PIPED = int(os.environ.get("MHA_PIPED", "2"))
    FINFILL = os.environ.get("MHA_FINFILL", "0") == "1"

    nc = bacc.Bacc(None, target_bir_lowering=False)

    xq = nc.dram_tensor("xq", [D, T], BF16, kind="ExternalInput")
    xk = nc.dram_tensor("xk", [D, T], BF16, kind="ExternalInput")
    xv = nc.dram_tensor("xv", [D, T], BF16, kind="ExternalInput")
    wq = nc.dram_tensor("wq", [128, NH * NDT * 128], BF16, kind="ExternalInput")
    wk = nc.dram_tensor("wk", [128, NDT * 128], BF16, kind="ExternalInput")
    wv = nc.dram_tensor("wv", [128, NDT * 128], BF16, kind="ExternalInput")
    wo = nc.dram_tensor("wo", [JW, D], BF16, kind="ExternalInput")
    cosd = nc.dram_tensor("cosT", [128, T], BF16, kind="ExternalInput")
    sind = nc.dram_tensor("sinT", [128, T], BF16, kind="ExternalInput")
    rotd = nc.dram_tensor("rot", [128, 128], BF16, kind="ExternalInput")
    identbd = nc.dram_tensor("identb", [128, 128], BF16, kind="ExternalInput")
    trid = nc.dram_tensor("tri", [128, 128], BF16, kind="ExternalInput")
    outd = nc.dram_tensor("out", [T, D], BF16, kind="ExternalOutput")
    outa = nc.dram_tensor("outa", [TC, D], BF16, kind="ExternalOutput")
    outb = nc.dram_tensor("outb", [TC, D], BF16, kind="ExternalOutput")

    with tile.TileContext(nc) as tc:
        with (
            tc.tile_pool(name="const", bufs=1) as constp,
            tc.tile_pool(name="persist", bufs=1) as persist,
            tc.tile_pool(name="slabs", bufs=2) as slabp,
            tc.tile_pool(name="tmps", bufs=2) as tmpp,
            tc.tile_pool(name="pa", bufs=2) as pa,
            tc.tile_pool(name="qkps", bufs=2, space="PSUM") as qkps,
            tc.tile_pool(name="pvps", bufs=1, space="PSUM") as pvps,
            tc.tile_pool(name="mmps", bufs=2, space="PSUM") as mmps,
        ):
            # ---- persistent SBUF ----
            rot_sb = constp.tile([128, 128], BF16)
            identb_sb = constp.tile([128, 128], BF16)
            tri_sb = constp.tile([128, 128], BF16)
            cos_sb = constp.tile([128, T], BF16)
            sin_sb = constp.tile([128, T], BF16)
            kT_rope = persist.tile([128, T], BF16)
            vaug = persist.tile([128, NTT, 132], BF16)
            wq_sb = persist.tile([128, NH, NDT, 128], BF16)
            wk_sb = persist.tile([128, NDT, 128], BF16)
            wv_sb = persist.tile([128, NDT, 128], BF16)
            wo_sb = persist.tile([128, NH, D], BF16)

            # ---- tiny consts first (clears the DMA queue fast) ----
            nc.sync.dma_start(out=rot_sb, in_=rotd[:])
            nc.sync.dma_start(out=identb_sb, in_=identbd[:])
            nc.sync.dma_start(out=tri_sb, in_=trid[:])
            nc.any.memset(vaug[:, :, 128:132], 1.0)

            # ---------------- emit-helper closures ----------------
            # Filler items are (cost_ns, fn) pairs.

            def kv_fillers(xsrc, w_sb, tch, dst_cb):
                """K or V projection of t-columns [tch*512,(tch+1)*512)."""
                st = {}

                def dma_i(i, half):
                    def f():
                        if half == 0:
                            st[i] = slabp.tile(
                                [128, 8, TC], BF16, tag="kvslab", bufs=3,
                                name="kvslab",
                            )
                        nc.sync.dma_start(
                            out=st[i][:, 4 * half : 4 * half + 4, :],
                            in_=xsrc[
                                i * 1024 + half * 512 :
                                i * 1024 + (half + 1) * 512,
                                tch * TC : (tch + 1) * TC,
                            ].rearrange("(n k) t -> k n t", k=128),
                        )
                    return {"cost": 100, "fn": f, "dma": 1456, "kind": "dma",
                            "grp": "kv", "bar": tch}

                def comp_i(i):
                    def f():
                        if i == 0:
                            st["ps"] = mmps.tile(
                                [128, TC], F32, tag="mm", name="kv_ps"
                            )
                        ps = st["ps"]
                        for j in range(8):
                            nc.tensor.matmul(
                                ps,
                                w_sb[:, i * 8 + j, :],
                                st[i][:, j, :],
                                start=(i == 0 and j == 0),
                                stop=(i == 3 and j == 7),
                            )
                        if i == 3:
                            dst_cb(ps)
                    return {"cost": 1710, "fn": f, "dma": 0, "kind": "comp",
                            "grp": "kv", "bar": tch}

                return [dma_i(0, 0), dma_i(0, 1), dma_i(1, 0), comp_i(0),
                        dma_i(1, 1), dma_i(2, 0), comp_i(1), dma_i(2, 1),
                        dma_i(3, 0), comp_i(2), dma_i(3, 1), comp_i(3)]

            def rope(dst, src, t0, tw):
                """dst[128, tw] = RoPE(src[128, tw]) at positions t0.. (fp32
                math; src/dst bf16)."""
                rp = mmps.tile([128, TC], F32, tag="mm", name="rope_ps")
                nc.tensor.matmul(rp[:, :tw], rot_sb, src, start=True, stop=True)
                t1 = pa.tile([128, TC], F32, tag="rt1", bufs=1, name="rope_t1")
                nc.gpsimd.tensor_mul(t1[:, :tw], src, cos_sb[:, t0 : t0 + tw])
                t2 = pa.tile([128, TC], F32, tag="rt2", bufs=1, name="rope_t2")
                nc.vector.tensor_mul(t2[:, :tw], rp[:, :tw], sin_sb[:, t0 : t0 + tw])
                nc.vector.tensor_add(dst, t1[:, :tw], t2[:, :tw])

            def k_chunk_fillers(tch):
                ktmp = tmpp.tile([128, TC], BF16, tag="ktmp", name="ktmp")

                def evac(ps):
                    nc.vector.tensor_copy(ktmp, ps)

                items = kv_fillers(xk, wk_sb, tch, evac)

                def rope_k():
                    rope(kT_rope[:, tch * TC : (tch + 1) * TC], ktmp,
                         tch * TC, TC)

                return items + [{"cost": 350, "fn": rope_k, "dma": 0,
                                 "kind": "comp", "grp": "kv", "bar": tch}]

            def v_chunk_fillers(tch):
                vtmp = tmpp.tile([128, TC], BF16, tag="vtmp", name="vtmp")

                def evac(ps):
                    nc.vector.tensor_copy(vtmp, ps)

                items = kv_fillers(xv, wv_sb, tch, evac)

                def vtr(half):
                    def f():
                        for b2 in range(2):
                            b = 4 * tch + 2 * half + b2
                            tp = mmps.tile(
                                [128, TC], BF16, tag="mm", name="vtr_ps"
                            )
                            nc.tensor.transpose(
                                tp[:, :128],
                                vtmp[:, (2 * half + b2) * 128 :
                                     (2 * half + b2 + 1) * 128],
                                identb_sb,
                            )
                            nc.vector.tensor_copy(vaug[:, b, 0:128], tp[:, :128])
                    return {"cost": 220, "fn": f, "dma": 0, "kind": "comp",
                            "grp": "kv", "bar": tch}

                return items + [vtr(0), vtr(1)]

            def qslab_dma_fillers(tcx):
                slabs = []

                def dma_h(dh, q):
                    def f():
                        if q == 0:
                            slab = slabp.tile(
                                [128, 16, TC], BF16, tag="qslab", name="qslab"
                            )
                            slabs.append(slab)
                        slab = slabs[dh]
                        nc.sync.dma_start(
                            out=slab[:, 4 * q : 4 * q + 4, :],
                            in_=xq[
                                dh * 2048 + q * 512 : dh * 2048 + (q + 1) * 512,
                                tcx * TC : (tcx + 1) * TC,
                            ].rearrange("(n k) t -> k n t", k=128),
                        )
                    return {"cost": 100, "fn": f, "dma": 1456, "kind": "dma",
                            "grp": "pre"}

                return slabs, [dma_h(0, q) for q in range(4)] + [
                    dma_h(1, q) for q in range(4)
                ]

            def qproj_chain(slabs, qraw, jh):
                ps = mmps.tile([128, TC], F32, tag="mm", name="q_ps")
                for dh in range(2):
                    for i in range(16):
                        nc.tensor.matmul(
                            ps,
                            wq_sb[:, jh, dh * 16 + i, :],
                            slabs[dh][:, i, :],
                            start=(dh == 0 and i == 0),
                            stop=(dh == 1 and i == 15),
                        )
                nc.vector.tensor_copy(qraw[:, jh, :], ps)

            def q_chunk_fillers(tcx, slabs):
                """Q proj + rope for chunk tcx; returns (qrope, items)."""
                qraw = tmpp.tile([128, NH, TC], BF16, tag="qraw", name="qraw")
                qrope = tmpp.tile([128, NH, TC], BF16, tag="qrope", bufs=2, name="qrope")
                items = []
                for jh in range(NH):
                    items.append({
                        "cost": 6830, "dma": 0, "kind": "comp", "grp": "q",
                        "bar": tcx, "qbar": jh,
                        "fn": lambda jh=jh: qproj_chain(slabs, qraw, jh),
                    })
                for jh in range(NH):
                    items.append({
                        "cost": 350, "dma": 0, "kind": "comp", "grp": "q",
                        "bar": tcx, "qbar": jh,
                        "fn": lambda jh=jh: rope(
                            qrope[:, jh, :], qraw[:, jh, :], tcx * TC, TC
                        ),
                    })
                return qrope, items

            def make_oproj_fillers(attnT, t0, jhs=range(NH), dest=None,
                                   dest_t0=None, evac_alt=False):
                dest = outd if dest is None else dest
                dest_t0 = t0 if dest_t0 is None else dest_t0
                jhs = list(jhs)
                fillers = []
                for nch in range(D // TC):
                    for s4 in range(4):
                        def f(s4=s4, nch=nch):
                            with nc.named_scope("oproj"):
                                ps = mmps.tile(
                                    [128, TC], F32, tag="mm", name="o_ps"
                                )
                                for x, jh in enumerate(jhs):
                                    nc.tensor.matmul(
                                        ps,
                                        attnT[:, jh, s4 * 128 : (s4 + 1) * 128],
                                        wo_sb[:, jh, nch * TC : (nch + 1) * TC],
                                        start=(x == 0),
                                        stop=(x == len(jhs) - 1),
                                    )
                                osb = pa.tile(
                                    [128, TC], BF16, tag="osb", bufs=4,
                                    name="osb",
                                )
                                if evac_alt and (s4 + nch) % 2 == 0:
                                    nc.scalar.copy(out=osb, in_=ps)
                                else:
                                    nc.vector.tensor_copy(osb, ps)
                                nc.sync.dma_start(
                                    out=dest[
                                        dest_t0 + s4 * 128 :
                                        dest_t0 + (s4 + 1) * 128,
                                        nch * TC : (nch + 1) * TC,
                                    ],
                                    in_=osb,
                                )
                        fillers.append(
                            {"cost": 218 * len(jhs), "fn": f,
                             "dma": 364, "kind": "oproj", "grp": "o"}
                        )
                return fillers

            def interleave(a, b):
                out = []
                ia = ib = 0
                na, nb = len(a), len(b)
                while ia < na or ib < nb:
                    if ia * max(nb, 1) <= ib * max(na, 1) and ia < na:
                        out.append(a[ia]); ia += 1
                    elif ib < nb:
                        out.append(b[ib]); ib += 1
                    else:
                        out.append(a[ia]); ia += 1
                return out

            def proj_items_for(tcx):
                """All projection work for chunk tcx as a filler list, DMA
                items placed so transfers land just ahead of their use."""
                slabs_n, qdma = qslab_dma_fillers(tcx)
                kn = k_chunk_fillers(tcx)
                vn = v_chunk_fillers(tcx)
                qrope_n, qn = q_chunk_fillers(tcx, slabs_n)
                qpairs = [qn[0], qn[NH], qn[1], qn[NH + 1], qn[2],
                          qn[NH + 2], qn[3], qn[NH + 3]]
                items = (
                    [kn[0], kn[1], qdma[0], qdma[1], kn[2], kn[3], qdma[2],
                     qdma[3], kn[4], kn[5], qdma[4], qdma[5], kn[6], kn[7],
                     qdma[6], qdma[7], kn[8], kn[9], kn[10], kn[11], kn[12]]
                    + vn[:12] + [vn[12], vn[13]]
                    + qpairs
                )
                return qrope_n, items

            # ---------------- chunk 0 prologue (inline, DMA-ordered) ----
            # Critical path to the first q chain: wq head 0 + both qslabs;
            # everything else (k/v slabs, cos/sin) streams behind and PE
            # picks it up between/after the q chains.
            def wq_head_dma(jh):
                nc.sync.dma_start(
                    out=wq_sb[:, jh, :, :],
                    in_=wq[:, jh * NDT * 128 : (jh + 1) * NDT * 128].rearrange(
                        "k (n j) -> k n j", j=128
                    ),
                )

            k0 = k_chunk_fillers(0)
            v0 = v_chunk_fillers(0)
            qslabs0, qdma0 = qslab_dma_fillers(0)
            wq_head_dma(0)
            for it in qdma0:                   # 8 quarter-slab dmas
                it["fn"]()
            wq_head_dma(1)
            nc.sync.dma_start(out=wk_sb, in_=wk.rearrange("k (n j) -> k n j", j=128))
            wq_head_dma(2)
            for it in k0[0:3]:                 # kslab dmas
                it["fn"]()
            wq_head_dma(3)
            qrope0, q0 = q_chunk_fillers(0, qslabs0)
            q0[0]["fn"](); q0[1]["fn"]()       # qproj chains 0,1
            k0[4]["fn"](); k0[5]["fn"]()       # kslab dmas
            q0[2]["fn"]()                      # qproj chain 2
            k0[3]["fn"]()                      # comp k piece 0
            nc.sync.dma_start(out=cos_sb, in_=cosd[:])
            q0[3]["fn"]()                      # qproj chain 3
            nc.sync.dma_start(out=sin_sb, in_=sind[:])
            k0[7]["fn"](); k0[8]["fn"](); k0[10]["fn"]()   # kslab dmas
            k0[6]["fn"](); k0[9]["fn"](); k0[11]["fn"]()   # comp k 1-3 + evac
            nc.sync.dma_start(out=wv_sb, in_=wv.rearrange("k (n j) -> k n j", j=128))
            k0[12]["fn"]()                     # rope-k(0)
            for it in q0[NH:]:                 # 4 rope-q(0)
                it["fn"]()
            v0[0]["fn"](); v0[1]["fn"](); v0[2]["fn"]()    # vslab dmas
            v0[3]["fn"]()                      # comp v piece 0
            v0[4]["fn"](); v0[5]["fn"]()       # vslab dmas
            v0[6]["fn"]()                      # comp v piece 1
            v0[7]["fn"](); v0[8]["fn"]()       # vslab dmas
            v0[9]["fn"]()                      # comp v piece 2
            v0[10]["fn"]()                     # vslab dma
            v0[11]["fn"]()                     # comp v piece 3 + evac
            v0[12]["fn"](); v0[13]["fn"]()     # vtr halves

            def wo_slice_dma(nch):
                def f():
                    nc.sync.dma_start(
                        out=wo_sb[:, :, nch * TC : (nch + 1) * TC],
                        in_=wo[:, nch * TC : (nch + 1) * TC].rearrange(
                            "(n k) d -> k n d", k=128
                        ),
                    )
                return {"cost": 100, "fn": f, "dma": 1456, "kind": "dma",
                        "grp": "pre"}

            # ---------------- main loop over t-chunks ----------------
            qrope_cur = qrope0
            prev_attnT = None
            prev_t0 = 0
            carry = []          # deferred oproj fillers from chunk tcx-1
            kv_carry = []       # K/V-proj fillers spilled into their own
                                # attention window (barrier at group 2*tcx)
            for tcx in range(NTC):
                t0 = tcx * TC

                if tcx + 1 < NTC:
                    qrope_next, proj_items = proj_items_for(tcx + 1)
                else:
                    qrope_next, proj_items = None, []
                # wo: first 2 slices during attn(0) (needed by the first
                # oproj pops early in attn(1)), the rest during attn(1)
                # where the DMA queue has slack.
                if tcx == 0:
                    wos = [wo_slice_dma(n) for n in range(D // TC)]
                    proj_items = interleave(proj_items, wos[:2])
                elif tcx == 1:
                    proj_items = interleave(proj_items, wos[2:])
                oproj_items = carry + (
                    make_oproj_fillers(prev_attnT, prev_t0)
                    if prev_attnT is not None
                    else []
                )
                if DEFER_O:
                    if tcx == 1:
                        deferred_o = oproj_items
                        oproj_items = []
                    elif tcx == 2:
                        oproj_items = deferred_o + oproj_items
                fillers = kv_carry + interleave(proj_items, oproj_items)
                kv_carry = []
                # annotate each compute item with the cumulative input-DMA
                # time that precedes it in this window's queue — popping it
                # earlier than that would head-of-line block the in-order
                # PE stream on an un-arrived transfer.
                cum_dma = 0.0
                for it in fillers:
                    if it["kind"] == "dma":
                        cum_dma += it["dma"]
                    if it["kind"] == "comp":
                        it["ready"] = cum_dma
                    elif it["kind"] == "oproj" and tcx == 1:
                        # wo slices still streaming in this window
                        it["ready"] = cum_dma
                    else:
                        it["ready"] = 0.0

                nt_valid = 4 * (tcx + 1) if causal else NTT
                ngroups = nt_valid // GW
                attnT = pa.tile(
                    [128, NH, TC], BF16, tag="attnT", bufs=3, name="attnT"
                )
                budget = 0.0
                popped = 0.0
                qkpv_clock = 0.0
                act_clock = 0.0
                popped_dma = 0.0
                SLACK = float(os.environ.get("MHA_SLACK", "2000"))
                LOOKAHEAD = float(os.environ.get("MHA_LOOKAHEAD", "9000"))
                BMULT = float(os.environ.get("MHA_BMULT", "1.0"))

                def pop_fillers():
                    nonlocal popped, popped_dma
                    while popped < budget and fillers:
                        elapsed = max(act_clock, qkpv_clock + popped)
                        # pull any leading dma items (keep the queue fed,
                        # but no more than LOOKAHEAD ahead of real time)
                        i = 0
                        progress = False
                        while i < len(fillers):
                            it = fillers[i]
                            if (it["kind"] == "dma"
                                    and popped_dma < elapsed + LOOKAHEAD):
                                fillers.pop(i)
                                it["fn"]()
                                popped_dma += it["dma"]
                                progress = True
                                continue
                            if it["kind"] != "dma":
                                break
                            i += 1
                        if not fillers or popped >= budget:
                            break
                        head = fillers[0]
                        if (head["kind"] != "dma"
                                and head["ready"] <= elapsed + SLACK):
                            fillers.pop(0)
                            head["fn"]()
                            popped += head["cost"]
                            popped_dma += head["dma"]
                            progress = True
                        elif head["kind"] != "dma":
                            # head blocked: pop a later independent item
                            # (oproj / q are reorderable; kv chains are not)
                            for j in range(1, min(len(fillers), 12)):
                                itj = fillers[j]
                                if (itj["kind"] != "dma"
                                        and itj.get("grp") in ("o", "q")
                                        and itj["ready"] <= elapsed + SLACK):
                                    fillers.pop(j)
                                    itj["fn"]()
                                    popped += itj["cost"]
                                    popped_dma += itj["dma"]
                                    progress = True
                                    break
                        if not progress:
                            break
                for h in range(NH):
                    if tcx >= 1:
                        i = 0
                        while i < len(fillers):
                            it = fillers[i]
                            if (it.get("grp") == "q" and it.get("bar") == tcx
                                    and it.get("qbar", 9) <= h):
                                fillers.pop(i)
                                it["fn"]()
                                popped += it["cost"]
                                popped_dma += it["dma"]
                            else:
                                i += 1
                    pend = []
                    with nc.named_scope("attn"):
                        pv = pvps.tile(
                            [128, 4, 256], F32, tag="pv", name="pv_ps"
                        )
                        for gg in range(ngroups):
                            if h == 0 and tcx >= 1 and gg == (
                                2 * tcx if causal else 0
                            ):
                                # force-drain this chunk's spilled K/V work:
                                # the next QK group reads the new tiles
                                i = 0
                                while i < len(fillers):
                                    if fillers[i].get("bar") == tcx:
                                        it = fillers.pop(i)
                                        it["fn"]()
                                        popped += it["cost"]
                                        popped_dma += it["dma"]
                                    else:
                                        i += 1
                            qk = qkps.tile(
                                [128, GW, TC], F32, tag="qk", name="qk_ps"
                            )
                            rels = []
                            for b in range(GW):
                                Tt = GW * gg + b
                                rel = Tt - 4 * tcx if causal else -1
                                rels.append(rel)
                                c0 = 128 * rel if rel > 0 else 0
                                nc.tensor.matmul(
                                    qk[:, b, c0:TC],
                                    kT_rope[:, Tt * 128 : (Tt + 1) * 128],
                                    qrope_cur[:, h, c0:TC],
                                    start=True,
                                    stop=True,
                                )
                            # tanh in place in PSUM, then exp to bf16 SBUF;
                            # soft-capping scales fused into ACT. Columns
                            # below the causal diagonal are skipped.
                            pt = pa.tile(
                                [128, GW, TC], BF16, tag="pt", bufs=int(os.environ.get("MHA_PTBUFS", "4")),
                                name="ptile",
                            )
                            act_cols = 0
                            if max(rels) <= 0:
                                nc.scalar.activation(
                                    out=qk, in_=qk, func=Tanh,
                                    scale=ATTN_MULT / CAP,
                                )
                                nc.scalar.activation(
                                    out=pt, in_=qk, func=Exp, scale=CAP
                                )
                                act_cols = GW * TC
                            else:
                                for b in range(GW):
                                    c0 = 128 * max(rels[b], 0)
                                    nc.scalar.activation(
                                        out=qk[:, b, c0:TC],
                                        in_=qk[:, b, c0:TC],
                                        func=Tanh, scale=ATTN_MULT / CAP,
                                    )
                                    nc.scalar.activation(
                                        out=pt[:, b, c0:TC],
                                        in_=qk[:, b, c0:TC],
                                        func=Exp, scale=CAP,
                                    )
                                    act_cols += TC - c0
                            for b in range(GW):
                                rel = rels[b]
                                if 0 <= rel < 4:
                                    # triangular mask on the diagonal block
                                    nc.gpsimd.tensor_mul(
                                        pt[:, b, rel * 128 : (rel + 1) * 128],
                                        pt[:, b, rel * 128 : (rel + 1) * 128],
                                        tri_sb,
                                    )
                            # software-pipelined PV: emit the PREVIOUS
                            # group's PV now, so it reaches PE well after
                            # its exp() finished on ACT (the current QK +
                            # fillers cover the ACT latency).
                            def emit_pv(p_pt, p_rels, p_gg):
                                n_pv = 0
                                for s4 in range(4):
                                    for b in range(GW):
                                        Tt = GW * p_gg + b
                                        rel = p_rels[b]
                                        if causal and rel > s4:
                                            continue
                                        n_pv += 1
                                        nc.tensor.matmul(
                                            pv[:, s4, 0:129],
                                            p_pt[:, b, s4 * 128 : (s4 + 1) * 128],
                                            vaug[:, Tt, 0:129],
                                            start=(
                                                p_gg == 0 and b == 0
                                                and s4 % 2 == 0
                                            ),
                                            stop=(
                                                (Tt == 4 * tcx + s4)
                                                if causal
                                                else (p_gg == ngroups - 1
                                                      and b == GW - 1)
                                            ),
                                            skip_group_check=True,
                                        )
                                return n_pv

                            n_pv = 0
                            pend.append((pt, rels, gg))
                            if len(pend) > PIPED:
                                n_pv = emit_pv(*pend.pop(0))
                            # weave fillers so PE stays busy under ACT
                            act_ns = act_cols * 2 * 0.833 + (
                                330 if max(rels) <= 0 else 660
                            )
                            qkpv_ns = (act_cols + 129 * n_pv) * 0.4167
                            act_clock += act_ns
                            qkpv_clock += qkpv_ns
                            budget += BMULT * max(act_ns - qkpv_ns, 0.0)
                            pop_fillers()
                        while pend:
                            emit_pv(*pend.pop(0))
                    with nc.named_scope("attn_fin"):
                        ans = []
                        for s4 in range(4):
                            rc = pa.tile(
                                [128, 1], F32, tag="rc", bufs=4, name="rc"
                            )
                            nc.vector.reciprocal(rc, pv[:, s4, 128:129])
                            an = pa.tile(
                                [128, 128], BF16, tag="an", bufs=4, name="an"
                            )
                            nc.vector.tensor_scalar_mul(an, pv[:, s4, 0:128], rc)
                            ans.append(an)
                        # cover the DVE normalize latency with a filler
                        budget += 700
                        act_clock += 700
                        pop_fillers()
                        if FINFILL:
                            # transposes aren't needed until next chunk's
                            # O-proj: queue them as fillers instead of
                            # serializing at the head boundary
                            def fin_tr(ans=ans, h=h):
                                for s4 in range(4):
                                    tp = mmps.tile(
                                        [128, TC], BF16, tag="mm", name="atr"
                                    )
                                    nc.tensor.transpose(
                                        tp[:, :128], ans[s4], identb_sb
                                    )
                                    nc.vector.tensor_copy(
                                        attnT[:, h, s4 * 128 : (s4 + 1) * 128],
                                        tp[:, :128],
                                    )
                            fillers.insert(0, {
                                "cost": 900, "fn": fin_tr, "dma": 0,
                                "kind": "oproj", "grp": "pre", "ready": 0.0,
                            })
                        else:
                            for s4 in range(4):
                                tp = mmps.tile(
                                    [128, TC], BF16, tag="mm", name="atr"
                                )
                                nc.tensor.transpose(
                                    tp[:, :128], ans[s4], identb_sb
                                )
                                nc.vector.tensor_copy(
                                    attnT[:, h, s4 * 128 : (s4 + 1) * 128],
                                    tp[:, :128],
                                )
                    if SPLIT_O and tcx == NTC - 1 and h == 1:
                        for it in make_oproj_fillers(
                            attnT, t0, jhs=[0, 1], dest=outa, dest_t0=0
                        ):
                            it["ready"] = 0.0
                            fillers.append(it)
                # drain: 'pre' items (q proj/rope of tc+1) must finish
                # before attn(tcx+1) emits its first QK; K/V items of tc+1
                # spill into attn(tcx+1) (barrier at group 2*(tcx+1)), and
                # up to MHA_CARRY oproj items carry over (attnT bufs=3).
                carry = []
                rest = fillers
                if tcx + 1 < NTC:
                    cap = int(os.environ.get("MHA_CARRY", "16"))
                    o_total = sum(1 for it in rest if it["grp"] == "o")
                    drain_o = max(0, o_total - cap)
                    drain = []
                    for it in rest:
                        if it["grp"] == "pre":
                            drain.append(it)
                        elif it["grp"] == "q":
                            if it["qbar"] == 0 or not QSPILL:
                                drain.append(it)
                            else:
                                kv_carry.append(it)
                        elif it["grp"] == "o" and drain_o > 0:
                            drain.append(it)
                            drain_o -= 1
                        elif it["grp"] == "kv" and KVSPILL:
                            kv_carry.append(it)
                        elif it["grp"] == "kv":
                            drain.append(it)
                        else:
                            carry.append(it)
                    rest = drain
                # drain with the same dma-forwarding discipline: keep
                # transfers ~LOOKAHEAD ahead of the estimated PE clock so
                # in-order compute items rarely wait on arrival.
                if not DRAIN2:
                    for it in rest:
                        it["fn"]()
                    rest = []
                el = max(act_clock, qkpv_clock + popped)
                dma_el = popped_dma
                while rest:
                    i = 0
                    while i < len(rest):
                        if (rest[i]["kind"] == "dma"
                                and dma_el < el + LOOKAHEAD):
                            it = rest.pop(i)
                            it["fn"]()
                            dma_el += it["dma"]
                            continue
                        if rest[i]["kind"] != "dma":
                            break
                        i += 1
                    if not rest:
                        break
                    it = rest.pop(0)
                    it["fn"]()
                    el = max(el, it.get("ready", 0.0)) + it["cost"]
                    dma_el += it["dma"]
                qrope_cur = qrope_next
                prev_attnT, prev_t0 = attnT, t0

            # tail: O proj pass B of the last chunk (host adds outa+outb)
            tail_items = (
                make_oproj_fillers(prev_attnT, prev_t0, jhs=[2, 3],
                                   dest=outb, dest_t0=0, evac_alt=True)
                if SPLIT_O
                else make_oproj_fillers(prev_attnT, prev_t0, evac_alt=True)
            )
            for it in carry + tail_items:
                it["fn"]()

    nc.compile()
    return nc


def vbgd_dst(vaug):
    return vaug[:, :, 128:132]


def _host_constants(T: int):
    d = KEY_SIZE
    inv_freq = 1.0 / (10000.0 ** (np.arange(0, d, 2, dtype=np.float64) / d))  # [64]
    pos = np.arange(T, dtype=np.float64)
    phase_half = pos[None, :] * inv_freq[:, None]  # [64, T]
    phase = np.concatenate([phase_half, phase_half], axis=0)  # [128, T] (tiled)
    cosT = np.cos(phase).astype(np.float32)
    sinT = np.sin(phase).astype(np.float32)

    R = np.zeros((128, 128), dtype=np.float32)
    R[:64, 64:] = -np.eye(64, dtype=np.float32)
    R[64:, :64] = np.eye(64, dtype=np.float32)
    rot = np.ascontiguousarray(R.T)

    ident = np.eye(128, dtype=np.float32)

    # tri[k, c] = 1 if k <= c (valid: query col >= key row inside the
    # diagonal 128x128 block)
    tri = (np.arange(128)[:, None] <= np.arange(128)[None, :]).astype(
        ml_dtypes.bfloat16
    )

    NTT = T // 128
    vbg = np.zeros((128, NTT, 4), dtype=ml_dtypes.bfloat16)
    vbg[:, :, 0] = 1.0
    return cosT, sinT, rot, ident, tri, vbg


_NC_CACHE: dict = {}
LAST_RESULT = None
_LAST_IN_MAPS = None


def kernel(query, key, value, mask, Wq, Wk, Wv, Wo):
    global LAST_RESULT, _LAST_IN_MAPS
    query = np.asarray(query)
    key = np.asarray(key)
    value = np.asarray(value)
    mask = np.asarray(mask)
    Wq = np.asarray(Wq, dtype=np.float32)
    Wk = np.asarray(Wk, dtype=np.float32)
    Wv = np.asarray(Wv, dtype=np.float32)
    Wo = np.asarray(Wo, dtype=np.float32)

    b, T, D = query.shape
    assert b == 1 and D == D_MODEL, (b, D)

    m2 = np.asarray(mask).reshape(T, T).astype(bool)
    if np.array_equal(m2, np.tril(np.ones((T, T), dtype=bool))):
        causal = True
    elif m2.all():
        causal = False
    else:
        raise ValueError("unsupported mask pattern (expected causal or full)")

    kkey = (T, causal)
    if kkey not in _NC_CACHE:
        _NC_CACHE[kkey] = build_nc(T, causal)
    nc = _NC_CACHE[kkey]

    pnp = ml_dtypes.bfloat16
    xq = np.ascontiguousarray(query[0].T).astype(pnp)  # [D, T]
    xk = np.ascontiguousarray(key[0].T).astype(pnp)
    xv = np.ascontiguousarray(value[0].T).astype(pnp)
    cosT, sinT, rot, ident, tri, vbg = _host_constants(T)

    JW = NH * KEY_SIZE
    NDT = D // 128

    def pack_w(w, nh):
        # [D, nh*128] -> [k, jh, n, j] flattened per-partition-contiguous
        a = np.ascontiguousarray(w).astype(pnp)
        a = a.reshape(NDT, 128, nh, 128).transpose(1, 2, 0, 3)
        return np.ascontiguousarray(a.reshape(128, nh * NDT * 128))

    in_maps = []
    for c in range(N_CORES):
        in_maps.append(
            {
                "xq": xq,
                "xk": xk,
                "xv": xv,
                "wq": pack_w(Wq[:, c * JW : (c + 1) * JW], NH),
                "wk": pack_w(Wk[:, c * KEY_SIZE : (c + 1) * KEY_SIZE], 1),
                "wv": pack_w(Wv[:, c * KEY_SIZE : (c + 1) * KEY_SIZE], 1),
                "wo": np.ascontiguousarray(Wo[c * JW : (c + 1) * JW, :]).astype(pnp),
                "cosT": cosT.astype(pnp),
                "sinT": sinT.astype(pnp),
                "rot": rot.astype(pnp),
                "identb": ident.astype(pnp),
                "tri": tri,
            }
        )

    _LAST_IN_MAPS = in_maps
    trace = os.environ.get("MHA_TRACE") == "1"
    res = run_bass_kernel_spmd(nc, in_maps, list(range(N_CORES)), trace=trace)
    LAST_RESULT = res

    out = np.zeros((T, D), dtype=np.float64)
    for c in range(N_CORES):
        out += res.results[c]["out"].astype(np.float64)
    return out.astype(np.float32).reshape(1, T, D)


# revision 45
# speedup vs baseline: 1.2562x; 1.0168x over previous
"""Trainium2 Bass kernel for nn_MultiHeadAttention_83056077570808.

GQA multi-head attention (32 q heads, 8 kv heads, d_head=128, T=2048,
D=4096) with RoPE, tanh soft-capping at 30, causal mask, fp32 reference.

Sharding: tensor-parallel over heads across 8 cores. Core c owns kv head c
and q heads 4c..4c+3: Wq/Wk/Wv column-sharded, Wo row-sharded; activations
replicated. Each core computes a partial output (its heads' contribution
through its Wo rows); the host sums the 8 partials.

Fully streamed schedule: causality means attention chunk tcx only needs
K/V tiles 0..4*tcx+3, so K/V/Q projections for chunk tcx+1 run *during*
attention of chunk tcx as filler work woven between QK groups (covering
the ACT-engine tanh/exp latency); O-proj of chunk tcx-1 likewise. DMA is
spread across the whole timeline instead of front-loaded. Fillers are
paced by an explicit cost model (popping too fast blocks the in-order PE
stream on un-arrived slab DMAs; too slow starves PE under ACT).

Causal diagonal trim: for key tile Tt in the diagonal block of chunk tcx
(rel = Tt-4*tcx in 0..3), query columns < 128*rel are entirely masked, so
QK / tanh / exp are column-trimmed, only the [128,128] diagonal block is
tri-masked (Pool), and PV skips s4-blocks with s4 < rel.

All matmuls are bf16; PSUM accumulation fp32; rope arithmetic fp32.

PSUM bank rule in the PV accumulation: start=True clears has_written for
the WHOLE bank and two s-chains share each bank, so only the bank's first
chain issues start=True; the sibling chain's first write lands on cleared
bits and overwrites.
"""

import os
import sys

for _p in ("/opt/trn_rl_repo", os.path.expanduser("~/.axon_site/_ro/trn_rl_repo")):
    if os.path.isdir(_p) and _p not in sys.path:
        sys.path.insert(0, _p)

import numpy as np
import ml_dtypes

import concourse.bass as bass
import concourse.tile as tile
from concourse import bacc, mybir
from concourse.bass_utils import run_bass_kernel_spmd

F32 = mybir.dt.float32
BF16 = mybir.dt.bfloat16

D_MODEL = 4096
KEY_SIZE = 128
NUM_Q_HEADS = 32
NUM_KV_HEADS = 8
N_CORES = 8
NH = NUM_Q_HEADS // NUM_KV_HEADS  # q heads per core = 4
ATTN_MULT = 0.08838834764831845
CAP = 30.0

Tanh = mybir.ActivationFunctionType.Tanh
Exp = mybir.ActivationFunctionType.Exp


def build_nc(T: int, causal: bool):
    """Emit the Bass program for one core (SPMD: all cores run this).

    Tile builds a STATIC per-engine schedule in (priority = emission)
    order, so overlap must be engineered in the emission order itself.
    """
    D = D_MODEL
    TC = 512                 # t-chunk width
    NTC = T // TC            # t-chunks
    NTT = T // 128           # 128-tiles along T (key side)
    NDT = D // 128           # contraction tiles over d_model = 32
    JW = NH * KEY_SIZE       # per-core q/o width = 512
    GW = 2                   # key tiles per QK group (1 PSUM bank each)

    SPLIT_O = os.environ.get("MHA_SPLIT_O", "0") == "1"
    EVAC_ALT = os.environ.get("MHA_EVAC_ALT", "0") == "1"
    PROJ_ORDER2 = os.environ.get("MHA_PORDER2", "0") == "1"
    DRAIN2 = os.environ.get("MHA_DRAIN2", "0") == "1"
    DEFER_O = os.environ.get("MHA_DEFER_O", "0") == "1"
    KVSPILL = os.environ.get("MHA_KVSPILL", "1") == "1"
    QSPILL = os.environ.get("MHA_QSPILL", "0") == "1"
    PIPED = int(os.environ.get("MHA_PIPED", "3"))
    FINFILL = os.environ.get("MHA_FINFILL", "0") == "1"

    nc = bacc.Bacc(None, target_bir_lowering=False)

    xq = nc.dram_tensor("xq", [D, T], BF16, kind="ExternalInput")
    xk = nc.dram_tensor("xk", [D, T], BF16, kind="ExternalInput")
    xv = nc.dram_tensor("xv", [D, T], BF16, kind="ExternalInput")
    wq = nc.dram_tensor("wq", [128, NH * NDT * 128], BF16, kind="ExternalInput")
    wk = nc.dram_tensor("wk", [128, NDT * 128], BF16, kind="ExternalInput")
    wv = nc.dram_tensor("wv", [128, NDT * 128], BF16, kind="ExternalInput")
    wo = nc.dram_tensor("wo", [JW, D], BF16, kind="ExternalInput")
    cosd = nc.dram_tensor("cosT", [128, T], BF16, kind="ExternalInput")
    sind = nc.dram_tensor("sinT", [128, T], BF16, kind="ExternalInput")
    rotd = nc.dram_tensor("rot", [128, 128], BF16, kind="ExternalInput")
    identbd = nc.dram_tensor("identb", [128, 128], BF16, kind="ExternalInput")
    trid = nc.dram_tensor("tri", [128, 128], BF16, kind="ExternalInput")
    outd = nc.dram_tensor("out", [T, D], BF16, kind="ExternalOutput")
    outa = nc.dram_tensor("outa", [TC, D], BF16, kind="ExternalOutput")
    outb = nc.dram_tensor("outb", [TC, D], BF16, kind="ExternalOutput")

    with tile.TileContext(nc) as tc:
        with (
            tc.tile_pool(name="const", bufs=1) as constp,
            tc.tile_pool(name="persist", bufs=1) as persist,
            tc.tile_pool(name="slabs", bufs=2) as slabp,
            tc.tile_pool(name="tmps", bufs=2) as tmpp,
            tc.tile_pool(name="pa", bufs=2) as pa,
            tc.tile_pool(name="qkps", bufs=2, space="PSUM") as qkps,
            tc.tile_pool(name="pvps", bufs=1, space="PSUM") as pvps,
            tc.tile_pool(name="mmps", bufs=2, space="PSUM") as mmps,
        ):
            # ---- persistent SBUF ----
            rot_sb = constp.tile([128, 128], BF16)
            identb_sb = constp.tile([128, 128], BF16)
            tri_sb = constp.tile([128, 128], BF16)
            cos_sb = constp.tile([128, T], BF16)
            sin_sb = constp.tile([128, T], BF16)
            kT_rope = persist.tile([128, T], BF16)
            vaug = persist.tile([128, NTT, 132], BF16)
            wq_sb = persist.tile([128, NH, NDT, 128], BF16)
            wk_sb = persist.tile([128, NDT, 128], BF16)
            wv_sb = persist.tile([128, NDT, 128], BF16)
            wo_sb = persist.tile([128, NH, D], BF16)

            # ---- tiny consts first (clears the DMA queue fast) ----
            nc.sync.dma_start(out=rot_sb, in_=rotd[:])
            nc.sync.dma_start(out=identb_sb, in_=identbd[:])
            nc.sync.dma_start(out=tri_sb, in_=trid[:])
            nc.any.memset(vaug[:, :, 128:132], 1.0)

            # ---------------- emit-helper closures ----------------
            # Filler items are (cost_ns, fn) pairs.

            def kv_fillers(xsrc, w_sb, tch, dst_cb):
                """K or V projection of t-columns [tch*512,(tch+1)*512)."""
                st = {}

                def dma_i(i, half):
                    def f():
                        if half == 0:
                            st[i] = slabp.tile(
                                [128, 8, TC], BF16, tag="kvslab", bufs=3,
                                name="kvslab",
                            )
                        nc.sync.dma_start(
                            out=st[i][:, 4 * half : 4 * half + 4, :],
                            in_=xsrc[
                                i * 1024 + half * 512 :
                                i * 1024 + (half + 1) * 512,
                                tch * TC : (tch + 1) * TC,
                            ].rearrange("(n k) t -> k n t", k=128),
                        )
                    return {"cost": 100, "fn": f, "dma": 1456, "kind": "dma",
                            "grp": "kv", "bar": tch}

                def comp_i(i):
                    def f():
                        if i == 0:
                            st["ps"] = mmps.tile(
                                [128, TC], F32, tag="mm", name="kv_ps"
                            )
                        ps = st["ps"]
                        for j in range(8):
                            nc.tensor.matmul(
                                ps,
                                w_sb[:, i * 8 + j, :],
                                st[i][:, j, :],
                                start=(i == 0 and j == 0),
                                stop=(i == 3 and j == 7),
                            )
                        if i == 3:
                            dst_cb(ps)
                    return {"cost": 1710, "fn": f, "dma": 0, "kind": "comp",
                            "grp": "kv", "bar": tch}

                return [dma_i(0, 0), dma_i(0, 1), dma_i(1, 0), comp_i(0),
                        dma_i(1, 1), dma_i(2, 0), comp_i(1), dma_i(2, 1),
                        dma_i(3, 0), comp_i(2), dma_i(3, 1), comp_i(3)]

            def rope(dst, src, t0, tw):
                """dst[128, tw] = RoPE(src[128, tw]) at positions t0.. (fp32
                math; src/dst bf16)."""
                rp = mmps.tile([128, TC], F32, tag="mm", name="rope_ps")
                nc.tensor.matmul(rp[:, :tw], rot_sb, src, start=True, stop=True)
                t1 = pa.tile([128, TC], F32, tag="rt1", bufs=1, name="rope_t1")
                nc.gpsimd.tensor_mul(t1[:, :tw], src, cos_sb[:, t0 : t0 + tw])
                t2 = pa.tile([128, TC], F32, tag="rt2", bufs=1, name="rope_t2")
                nc.vector.tensor_mul(t2[:, :tw], rp[:, :tw], sin_sb[:, t0 : t0 + tw])
                nc.vector.tensor_add(dst, t1[:, :tw], t2[:, :tw])

            def k_chunk_fillers(tch):
                ktmp = tmpp.tile([128, TC], BF16, tag="ktmp", name="ktmp")

                def evac(ps):
                    nc.vector.tensor_copy(ktmp, ps)

                items = kv_fillers(xk, wk_sb, tch, evac)

                def rope_k():
                    rope(kT_rope[:, tch * TC : (tch + 1) * TC], ktmp,
                         tch * TC, TC)

                return items + [{"cost": 350, "fn": rope_k, "dma": 0,
                                 "kind": "comp", "grp": "kv", "bar": tch}]

            def v_chunk_fillers(tch):
                vtmp = tmpp.tile([128, TC], BF16, tag="vtmp", name="vtmp")

                def evac(ps):
                    nc.vector.tensor_copy(vtmp, ps)

                items = kv_fillers(xv, wv_sb, tch, evac)

                def vtr(half):
                    def f():
                        for b2 in range(2):
                            b = 4 * tch + 2 * half + b2
                            tp = mmps.tile(
                                [128, TC], BF16, tag="mm", name="vtr_ps"
                            )
                            nc.tensor.transpose(
                                tp[:, :128],
                                vtmp[:, (2 * half + b2) * 128 :
                                     (2 * half + b2 + 1) * 128],
                                identb_sb,
                            )
                            nc.vector.tensor_copy(vaug[:, b, 0:128], tp[:, :128])
                    return {"cost": 220, "fn": f, "dma": 0, "kind": "comp",
                            "grp": "kv", "bar": tch}

                return items + [vtr(0), vtr(1)]

            def qslab_dma_fillers(tcx):
                slabs = []

                def dma_h(dh, q):
                    def f():
                        if q == 0:
                            slab = slabp.tile(
                                [128, 16, TC], BF16, tag="qslab", name="qslab"
                            )
                            slabs.append(slab)
                        slab = slabs[dh]
                        nc.sync.dma_start(
                            out=slab[:, 4 * q : 4 * q + 4, :],
                            in_=xq[
                                dh * 2048 + q * 512 : dh * 2048 + (q + 1) * 512,
                                tcx * TC : (tcx + 1) * TC,
                            ].rearrange("(n k) t -> k n t", k=128),
                        )
                    return {"cost": 100, "fn": f, "dma": 1456, "kind": "dma",
                            "grp": "pre"}

                return slabs, [dma_h(0, q) for q in range(4)] + [
                    dma_h(1, q) for q in range(4)
                ]

            def qproj_chain(slabs, qraw, jh):
                ps = mmps.tile([128, TC], F32, tag="mm", name="q_ps")
                for dh in range(2):
                    for i in range(16):
                        nc.tensor.matmul(
                            ps,
                            wq_sb[:, jh, dh * 16 + i, :],
                            slabs[dh][:, i, :],
                            start=(dh == 0 and i == 0),
                            stop=(dh == 1 and i == 15),
                        )
                nc.vector.tensor_copy(qraw[:, jh, :], ps)

            def q_chunk_fillers(tcx, slabs):
                """Q proj + rope for chunk tcx; returns (qrope, items)."""
                qraw = tmpp.tile([128, NH, TC], BF16, tag="qraw", name="qraw")
                qrope = tmpp.tile([128, NH, TC], BF16, tag="qrope", bufs=2, name="qrope")
                items = []
                for jh in range(NH):
                    items.append({
                        "cost": 6830, "dma": 0, "kind": "comp", "grp": "q",
                        "bar": tcx, "qbar": jh,
                        "fn": lambda jh=jh: qproj_chain(slabs, qraw, jh),
                    })
                for jh in range(NH):
                    items.append({
                        "cost": 350, "dma": 0, "kind": "comp", "grp": "q",
                        "bar": tcx, "qbar": jh,
                        "fn": lambda jh=jh: rope(
                            qrope[:, jh, :], qraw[:, jh, :], tcx * TC, TC
                        ),
                    })
                return qrope, items

            def make_oproj_fillers(attnT, t0, jhs=range(NH), dest=None,
                                   dest_t0=None, evac_alt=False,
                                   use_qkps=False):
                dest = outd if dest is None else dest
                dest_t0 = t0 if dest_t0 is None else dest_t0
                jhs = list(jhs)
                fillers = []
                for nch in range(D // TC):
                    for s4 in range(4):
                        def f(s4=s4, nch=nch):
                            with nc.named_scope("oproj"):
                                if use_qkps and (s4 + nch) % 2 == 0:
                                    # qk PSUM banks are idle in the tail:
                                    # alternate into them for a deeper
                                    # chain pipeline
                                    ps = qkps.tile(
                                        [128, GW, TC], F32, tag="qk",
                                        name="o_ps2",
                                    )[:, 0, :]
                                else:
                                    ps = mmps.tile(
                                        [128, TC], F32, tag="mm", name="o_ps"
                                    )
                                for x, jh in enumerate(jhs):
                                    nc.tensor.matmul(
                                        ps,
                                        attnT[:, jh, s4 * 128 : (s4 + 1) * 128],
                                        wo_sb[:, jh, nch * TC : (nch + 1) * TC],
                                        start=(x == 0),
                                        stop=(x == len(jhs) - 1),
                                    )
                                osb = pa.tile(
                                    [128, TC], BF16, tag="osb", bufs=4,
                                    name="osb",
                                )
                                if evac_alt and (s4 + nch) % 2 == 0:
                                    nc.scalar.copy(out=osb, in_=ps)
                                else:
                                    nc.vector.tensor_copy(osb, ps)
                                nc.sync.dma_start(
                                    out=dest[
                                        dest_t0 + s4 * 128 :
                                        dest_t0 + (s4 + 1) * 128,
                                        nch * TC : (nch + 1) * TC,
                                    ],
                                    in_=osb,
                                )
                        fillers.append(
                            {"cost": 218 * len(jhs), "fn": f,
                             "dma": 364, "kind": "oproj", "grp": "o"}
                        )
                return fillers

            def interleave(a, b):
                out = []
                ia = ib = 0
                na, nb = len(a), len(b)
                while ia < na or ib < nb:
                    if ia * max(nb, 1) <= ib * max(na, 1) and ia < na:
                        out.append(a[ia]); ia += 1
                    elif ib < nb:
                        out.append(b[ib]); ib += 1
                    else:
                        out.append(a[ia]); ia += 1
                return out

            def proj_items_for(tcx):
                """All projection work for chunk tcx as a filler list, DMA
                items placed so transfers land just ahead of their use."""
                slabs_n, qdma = qslab_dma_fillers(tcx)
                kn = k_chunk_fillers(tcx)
                vn = v_chunk_fillers(tcx)
                qrope_n, qn = q_chunk_fillers(tcx, slabs_n)
                qpairs = [qn[0], qn[NH], qn[1], qn[NH + 1], qn[2],
                          qn[NH + 2], qn[3], qn[NH + 3]]
                items = (
                    [kn[0], kn[1], qdma[0], qdma[1], kn[2], kn[3], qdma[2],
                     qdma[3], kn[4], kn[5], qdma[4], qdma[5], kn[6], kn[7],
                     qdma[6], qdma[7], kn[8], kn[9], kn[10], kn[11], kn[12]]
                    + vn[:12] + [vn[12], vn[13]]
                    + qpairs
                )
                return qrope_n, items

            # ---------------- chunk 0 prologue (inline, DMA-ordered) ----
            # Critical path to the first q chain: wq head 0 + both qslabs;
            # everything else (k/v slabs, cos/sin) streams behind and PE
            # picks it up between/after the q chains.
            def wq_head_dma(jh):
                nc.sync.dma_start(
                    out=wq_sb[:, jh, :, :],
                    in_=wq[:, jh * NDT * 128 : (jh + 1) * NDT * 128].rearrange(
                        "k (n j) -> k n j", j=128
                    ),
                )

            k0 = k_chunk_fillers(0)
            v0 = v_chunk_fillers(0)
            qslabs0, qdma0 = qslab_dma_fillers(0)
            wq_head_dma(0)
            for it in qdma0:                   # 8 quarter-slab dmas
                it["fn"]()
            wq_head_dma(1)
            nc.sync.dma_start(out=wk_sb, in_=wk.rearrange("k (n j) -> k n j", j=128))
            wq_head_dma(2)
            for it in k0[0:3]:                 # kslab dmas
                it["fn"]()
            wq_head_dma(3)
            qrope0, q0 = q_chunk_fillers(0, qslabs0)
            q0[0]["fn"](); q0[1]["fn"]()       # qproj chains 0,1
            k0[4]["fn"](); k0[5]["fn"]()       # kslab dmas
            q0[2]["fn"]()                      # qproj chain 2
            k0[3]["fn"]()                      # comp k piece 0
            nc.sync.dma_start(out=cos_sb, in_=cosd[:])
            q0[3]["fn"]()                      # qproj chain 3
            nc.sync.dma_start(out=sin_sb, in_=sind[:])
            k0[7]["fn"](); k0[8]["fn"](); k0[10]["fn"]()   # kslab dmas
            k0[6]["fn"](); k0[9]["fn"](); k0[11]["fn"]()   # comp k 1-3 + evac
            nc.sync.dma_start(out=wv_sb, in_=wv.rearrange("k (n j) -> k n j", j=128))
            k0[12]["fn"]()                     # rope-k(0)
            for it in q0[NH:]:                 # 4 rope-q(0)
                it["fn"]()
            v0[0]["fn"](); v0[1]["fn"](); v0[2]["fn"]()    # vslab dmas
            v0[3]["fn"]()                      # comp v piece 0
            v0[4]["fn"](); v0[5]["fn"]()       # vslab dmas
            v0[6]["fn"]()                      # comp v piece 1
            v0[7]["fn"](); v0[8]["fn"]()       # vslab dmas
            v0[9]["fn"]()                      # comp v piece 2
            v0[10]["fn"]()                     # vslab dma
            v0[11]["fn"]()                     # comp v piece 3 + evac
            v0[12]["fn"](); v0[13]["fn"]()     # vtr halves

            def wo_slice_dma(nch):
                def f():
                    nc.sync.dma_start(
                        out=wo_sb[:, :, nch * TC : (nch + 1) * TC],
                        in_=wo[:, nch * TC : (nch + 1) * TC].rearrange(
                            "(n k) d -> k n d", k=128
                        ),
                    )
                return {"cost": 100, "fn": f, "dma": 1456, "kind": "dma",
                        "grp": "pre"}

            # ---------------- main loop over t-chunks ----------------
            qrope_cur = qrope0
            prev_attnT = None
            prev_t0 = 0
            carry = []          # deferred oproj fillers from chunk tcx-1
            kv_carry = []       # K/V-proj fillers spilled into their own
                                # attention window (barrier at group 2*tcx)
            for tcx in range(NTC):
                t0 = tcx * TC

                if tcx + 1 < NTC:
                    qrope_next, proj_items = proj_items_for(tcx + 1)
                else:
                    qrope_next, proj_items = None, []
                # wo: first 2 slices during attn(0) (needed by the first
                # oproj pops early in attn(1)), the rest during attn(1)
                # where the DMA queue has slack.
                if tcx == 0:
                    wos = [wo_slice_dma(n) for n in range(D // TC)]
                    proj_items = interleave(proj_items, wos[:2])
                elif tcx == 1:
                    proj_items = interleave(proj_items, wos[2:])
                oproj_items = carry + (
                    make_oproj_fillers(prev_attnT, prev_t0)
                    if prev_attnT is not None
                    else []
                )
                if DEFER_O:
                    if tcx == 1:
                        deferred_o = oproj_items
                        oproj_items = []
                    elif tcx == 2:
                        oproj_items = deferred_o + oproj_items
                fillers = kv_carry + interleave(proj_items, oproj_items)
                kv_carry = []
                # annotate each compute item with the cumulative input-DMA
                # time that precedes it in this window's queue — popping it
                # earlier than that would head-of-line block the in-order
                # PE stream on an un-arrived transfer.
                cum_dma = 0.0
                for it in fillers:
                    if it["kind"] == "dma":
                        cum_dma += it["dma"]
                    if it["kind"] == "comp":
                        it["ready"] = cum_dma
                    elif it["kind"] == "oproj" and tcx == 1:
                        # wo slices still streaming in this window
                        it["ready"] = cum_dma
                    else:
                        it["ready"] = 0.0

                nt_valid = 4 * (tcx + 1) if causal else NTT
                ngroups = nt_valid // GW
                attnT = pa.tile(
                    [128, NH, TC], BF16, tag="attnT", bufs=3, name="attnT"
                )
                budget = 0.0
                popped = 0.0
                qkpv_clock = 0.0
                act_clock = 0.0
                popped_dma = 0.0
                SLACK = float(os.environ.get("MHA_SLACK", "2000"))
                LOOKAHEAD = float(os.environ.get("MHA_LOOKAHEAD", "9000"))
                BMULT = float(os.environ.get("MHA_BMULT", "1.0"))

                def pop_fillers():
                    nonlocal popped, popped_dma
                    while popped < budget and fillers:
                        elapsed = max(act_clock, qkpv_clock + popped)
                        # pull any leading dma items (keep the queue fed,
                        # but no more than LOOKAHEAD ahead of real time)
                        i = 0
                        progress = False
                        while i < len(fillers):
                            it = fillers[i]
                            if (it["kind"] == "dma"
                                    and popped_dma < elapsed + LOOKAHEAD):
                                fillers.pop(i)
                                it["fn"]()
                                popped_dma += it["dma"]
                                progress = True
                                continue
                            if it["kind"] != "dma":
                                break
                            i += 1
                        if not fillers or popped >= budget:
                            break
                        head = fillers[0]
                        if (head["kind"] != "dma"
                                and head["ready"] <= elapsed + SLACK):
                            fillers.pop(0)
                            head["fn"]()
                            popped += head["cost"]
                            popped_dma += head["dma"]
                            progress = True
                        elif head["kind"] != "dma":
                            # head blocked: pop a later independent item
                            # (oproj / q are reorderable; kv chains are not)
                            for j in range(1, min(len(fillers), 12)):
                                itj = fillers[j]
                                if (itj["kind"] != "dma"
                                        and itj.get("grp") in ("o", "q")
                                        and itj["ready"] <= elapsed + SLACK):
                                    fillers.pop(j)
                                    itj["fn"]()
                                    popped += itj["cost"]
                                    popped_dma += itj["dma"]
                                    progress = True
                                    break
                        if not progress:
                            break
                for h in range(NH):
                    if tcx >= 1:
                        i = 0
                        while i < len(fillers):
                            it = fillers[i]
                            if (it.get("grp") == "q" and it.get("bar") == tcx
                                    and it.get("qbar", 9) <= h):
                                fillers.pop(i)
                                it["fn"]()
                                popped += it["cost"]
                                popped_dma += it["dma"]
                            else:
                                i += 1
                    pend = []
                    with nc.named_scope("attn"):
                        pv = pvps.tile(
                            [128, 4, 256], F32, tag="pv", name="pv_ps"
                        )
                        for gg in range(ngroups):
                            if h == 0 and tcx >= 1 and gg == (
                                2 * tcx if causal else 0
                            ):
                                # force-drain this chunk's spilled K/V work:
                                # the next QK group reads the new tiles
                                i = 0
                                while i < len(fillers):
                                    if fillers[i].get("bar") == tcx:
                                        it = fillers.pop(i)
                                        it["fn"]()
                                        popped += it["cost"]
                                        popped_dma += it["dma"]
                                    else:
                                        i += 1
                            qk = qkps.tile(
                                [128, GW, TC], F32, tag="qk", name="qk_ps"
                            )
                            rels = []
                            for b in range(GW):
                                Tt = GW * gg + b
                                rel = Tt - 4 * tcx if causal else -1
                                rels.append(rel)
                                c0 = 128 * rel if rel > 0 else 0
                                nc.tensor.matmul(
                                    qk[:, b, c0:TC],
                                    kT_rope[:, Tt * 128 : (Tt + 1) * 128],
                                    qrope_cur[:, h, c0:TC],
                                    start=True,
                                    stop=True,
                                )
                            # tanh in place in PSUM, then exp to bf16 SBUF;
                            # soft-capping scales fused into ACT. Columns
                            # below the causal diagonal are skipped.
                            pt = pa.tile(
                                [128, GW, TC], BF16, tag="pt", bufs=int(os.environ.get("MHA_PTBUFS", "4")),
                                name="ptile",
                            )
                            act_cols = 0
                            if max(rels) <= 0:
                                nc.scalar.activation(
                                    out=qk, in_=qk, func=Tanh,
                                    scale=ATTN_MULT / CAP,
                                )
                                nc.scalar.activation(
                                    out=pt, in_=qk, func=Exp, scale=CAP
                                )
                                act_cols = GW * TC
                            else:
                                for b in range(GW):
                                    c0 = 128 * max(rels[b], 0)
                                    nc.scalar.activation(
                                        out=qk[:, b, c0:TC],
                                        in_=qk[:, b, c0:TC],
                                        func=Tanh, scale=ATTN_MULT / CAP,
                                    )
                                    nc.scalar.activation(
                                        out=pt[:, b, c0:TC],
                                        in_=qk[:, b, c0:TC],
                                        func=Exp, scale=CAP,
                                    )
                                    act_cols += TC - c0
                            for b in range(GW):
                                rel = rels[b]
                                if 0 <= rel < 4:
                                    # triangular mask on the diagonal block
                                    nc.gpsimd.tensor_mul(
                                        pt[:, b, rel * 128 : (rel + 1) * 128],
                                        pt[:, b, rel * 128 : (rel + 1) * 128],
                                        tri_sb,
                                    )
                            # software-pipelined PV: emit the PREVIOUS
                            # group's PV now, so it reaches PE well after
                            # its exp() finished on ACT (the current QK +
                            # fillers cover the ACT latency).
                            def emit_pv(p_pt, p_rels, p_gg):
                                n_pv = 0
                                for s4 in range(4):
                                    for b in range(GW):
                                        Tt = GW * p_gg + b
                                        rel = p_rels[b]
                                        if causal and rel > s4:
                                            continue
                                        n_pv += 1
                                        nc.tensor.matmul(
                                            pv[:, s4, 0:129],
                                            p_pt[:, b, s4 * 128 : (s4 + 1) * 128],
                                            vaug[:, Tt, 0:129],
                                            start=(
                                                p_gg == 0 and b == 0
                                                and s4 % 2 == 0
                                            ),
                                            stop=(
                                                (Tt == 4 * tcx + s4)
                                                if causal
                                                else (p_gg == ngroups - 1
                                                      and b == GW - 1)
                                            ),
                                            skip_group_check=True,
                                        )
                                return n_pv

                            n_pv = 0
                            pend.append((pt, rels, gg))
                            if len(pend) > PIPED:
                                n_pv = emit_pv(*pend.pop(0))
                            # weave fillers so PE stays busy under ACT
                            act_ns = act_cols * 2 * 0.833 + (
                                330 if max(rels) <= 0 else 660
                            )
                            qkpv_ns = (act_cols + 129 * n_pv) * 0.4167
                            act_clock += act_ns
                            qkpv_clock += qkpv_ns
                            budget += BMULT * max(act_ns - qkpv_ns, 0.0)
                            pop_fillers()
                        while pend:
                            emit_pv(*pend.pop(0))
                    with nc.named_scope("attn_fin"):
                        ans = []
                        for s4 in range(4):
                            rc = pa.tile(
                                [128, 1], F32, tag="rc", bufs=4, name="rc"
                            )
                            nc.vector.reciprocal(rc, pv[:, s4, 128:129])
                            an = pa.tile(
                                [128, 128], BF16, tag="an", bufs=4, name="an"
                            )
                            nc.vector.tensor_scalar_mul(an, pv[:, s4, 0:128], rc)
                            ans.append(an)
                        # cover the DVE normalize latency with a filler
                        budget += 700
                        act_clock += 700
                        pop_fillers()
                        if FINFILL:
                            # transposes aren't needed until next chunk's
                            # O-proj: queue them as fillers instead of
                            # serializing at the head boundary
                            def fin_tr(ans=ans, h=h):
                                for s4 in range(4):
                                    tp = mmps.tile(
                                        [128, TC], BF16, tag="mm", name="atr"
                                    )
                                    nc.tensor.transpose(
                                        tp[:, :128], ans[s4], identb_sb
                                    )
                                    nc.vector.tensor_copy(
                                        attnT[:, h, s4 * 128 : (s4 + 1) * 128],
                                        tp[:, :128],
                                    )
                            fillers.insert(0, {
                                "cost": 900, "fn": fin_tr, "dma": 0,
                                "kind": "oproj", "grp": "pre", "ready": 0.0,
                            })
                        else:
                            for s4 in range(4):
                                tp = mmps.tile(
                                    [128, TC], BF16, tag="mm", name="atr"
                                )
                                nc.tensor.transpose(
                                    tp[:, :128], ans[s4], identb_sb
                                )
                                nc.vector.tensor_copy(
                                    attnT[:, h, s4 * 128 : (s4 + 1) * 128],
                                    tp[:, :128],
                                )
                    if SPLIT_O and tcx == NTC - 1 and h == 1:
                        for it in make_oproj_fillers(
                            attnT, t0, jhs=[0, 1], dest=outa, dest_t0=0
                        ):
                            it["ready"] = 0.0
                            fillers.append(it)
                # drain: 'pre' items (q proj/rope of tc+1) must finish
                # before attn(tcx+1) emits its first QK; K/V items of tc+1
                # spill into attn(tcx+1) (barrier at group 2*(tcx+1)), and
                # up to MHA_CARRY oproj items carry over (attnT bufs=3).
                carry = []
                rest = fillers
                if tcx + 1 < NTC:
                    cap = int(os.environ.get("MHA_CARRY", "16"))
                    o_total = sum(1 for it in rest if it["grp"] == "o")
                    drain_o = max(0, o_total - cap)
                    drain = []
                    for it in rest:
                        if it["grp"] == "pre":
                            drain.append(it)
                        elif it["grp"] == "q":
                            if it["qbar"] == 0 or not QSPILL:
                                drain.append(it)
                            else:
                                kv_carry.append(it)
                        elif it["grp"] == "o" and drain_o > 0:
                            drain.append(it)
                            drain_o -= 1
                        elif it["grp"] == "kv" and KVSPILL:
                            kv_carry.append(it)
                        elif it["grp"] == "kv":
                            drain.append(it)
                        else:
                            carry.append(it)
                    rest = drain
                # drain with the same dma-forwarding discipline: keep
                # transfers ~LOOKAHEAD ahead of the estimated PE clock so
                # in-order compute items rarely wait on arrival.
                if not DRAIN2:
                    for it in rest:
                        it["fn"]()
                    rest = []
                el = max(act_clock, qkpv_clock + popped)
                dma_el = popped_dma
                while rest:
                    i = 0
                    while i < len(rest):
                        if (rest[i]["kind"] == "dma"
                                and dma_el < el + LOOKAHEAD):
                            it = rest.pop(i)
                            it["fn"]()
                            dma_el += it["dma"]
                            continue
                        if rest[i]["kind"] != "dma":
                            break
                        i += 1
                    if not rest:
                        break
                    it = rest.pop(0)
                    it["fn"]()
                    el = max(el, it.get("ready", 0.0)) + it["cost"]
                    dma_el += it["dma"]
                qrope_cur = qrope_next
                prev_attnT, prev_t0 = attnT, t0

            # tail: O proj pass B of the last chunk (host adds outa+outb)
            tail_items = (
                make_oproj_fillers(prev_attnT, prev_t0, jhs=[2, 3],
                                   dest=outb, dest_t0=0, evac_alt=True)
                if SPLIT_O
                else make_oproj_fillers(prev_attnT, prev_t0, evac_alt=True,
                                        use_qkps=True)
            )
            for it in carry + tail_items:
                it["fn"]()

    nc.compile()
    return nc


def vbgd_dst(vaug):
    return vaug[:, :, 128:132]


def _host_constants(T: int):
    d = KEY_SIZE
    inv_freq = 1.0 / (10000.0 ** (np.arange(0, d, 2, dtype=np.float64) / d))  # [64]
    pos = np.arange(T, dtype=np.float64)
    phase_half = pos[None, :] * inv_freq[:, None]  # [64, T]
    phase = np.concatenate([phase_half, phase_half], axis=0)  # [128, T] (tiled)
    cosT = np.cos(phase).astype(np.float32)
    sinT = np.sin(phase).astype(np.float32)

    R = np.zeros((128, 128), dtype=np.float32)
    R[:64, 64:] = -np.eye(64, dtype=np.float32)
    R[64:, :64] = np.eye(64, dtype=np.float32)
    rot = np.ascontiguousarray(R.T)

    ident = np.eye(128, dtype=np.float32)

    # tri[k, c] = 1 if k <= c (valid: query col >= key row inside the
    # diagonal 128x128 block)
    tri = (np.arange(128)[:, None] <= np.arange(128)[None, :]).astype(
        ml_dtypes.bfloat16
    )

    NTT = T // 128
    vbg = np.zeros((128, NTT, 4), dtype=ml_dtypes.bfloat16)
    vbg[:, :, 0] = 1.0
    return cosT, sinT, rot, ident, tri, vbg


_NC_CACHE: dict = {}
LAST_RESULT = None
_LAST_IN_MAPS = None


def kernel(query, key, value, mask, Wq, Wk, Wv, Wo):
    global LAST_RESULT, _LAST_IN_MAPS
    query = np.asarray(query)
    key = np.asarray(key)
    value = np.asarray(value)
    mask = np.asarray(mask)
    Wq = np.asarray(Wq, dtype=np.float32)
    Wk = np.asarray(Wk, dtype=np.float32)
    Wv = np.asarray(Wv, dtype=np.float32)
    Wo = np.asarray(Wo, dtype=np.float32)

    b, T, D = query.shape
    assert b == 1 and D == D_MODEL, (b, D)

    m2 = np.asarray(mask).reshape(T, T).astype(bool)
    if np.array_equal(m2, np.tril(np.ones((T, T), dtype=bool))):
        causal = True
    elif m2.all():
        causal = False
    else:
        raise ValueError("unsupported mask pattern (expected causal or full)")

    kkey = (T, causal)
    if kkey not in _NC_CACHE:
        _NC_CACHE[kkey] = build_nc(T, causal)
    nc = _NC_CACHE[kkey]

    pnp = ml_dtypes.bfloat16
    xq = np.ascontiguousarray(query[0].T).astype(pnp)  # [D, T]
    xk = np.ascontiguousarray(key[0].T).astype(pnp)
    xv = np.ascontiguousarray(value[0].T).astype(pnp)
    cosT, sinT, rot, ident, tri, vbg = _host_constants(T)

    JW = NH * KEY_SIZE
    NDT = D // 128

    def pack_w(w, nh):
        # [D, nh*128] -> [k, jh, n, j] flattened per-partition-contiguous
        a = np.ascontiguousarray(w).astype(pnp)
        a = a.reshape(NDT, 128, nh, 128).transpose(1, 2, 0, 3)
        return np.ascontiguousarray(a.reshape(128, nh * NDT * 128))

    in_maps = []
    for c in range(N_CORES):
        in_maps.append(
            {
                "xq": xq,
                "xk": xk,
                "xv": xv,
                "wq": pack_w(Wq[:, c * JW : (c + 1) * JW], NH),
                "wk": pack_w(Wk[:, c * KEY_SIZE : (c + 1) * KEY_SIZE], 1),
                "wv": pack_w(Wv[:, c * KEY_SIZE : (c + 1) * KEY_SIZE], 1),
                "wo": np.ascontiguousarray(Wo[c * JW : (c + 1) * JW, :]).astype(pnp),
                "cosT": cosT.astype(pnp),
                "sinT": sinT.astype(pnp),
                "rot": rot.astype(pnp),
                "identb": ident.astype(pnp),
                "tri": tri,
            }
        )

    _LAST_IN_MAPS = in_maps
    trace = os.environ.get("MHA_TRACE") == "1"
    res = run_bass_kernel_spmd(nc, in_maps, list(range(N_CORES)), trace=trace)
    LAST_RESULT = res

    out = np.zeros((T, D), dtype=np.float64)
    for c in range(N_CORES):
        out += res.results[c]["out"].astype(np.float64)
    return out.astype(np.float32).reshape(1, T, D)


# revision 46
# speedup vs baseline: 1.2573x; 1.0008x over previous
"""Trainium2 Bass kernel for nn_MultiHeadAttention_83056077570808.

GQA multi-head attention (32 q heads, 8 kv heads, d_head=128, T=2048,
D=4096) with RoPE, tanh soft-capping at 30, causal mask, fp32 reference.

Sharding: tensor-parallel over heads across 8 cores. Core c owns kv head c
and q heads 4c..4c+3: Wq/Wk/Wv column-sharded, Wo row-sharded; activations
replicated. Each core computes a partial output (its heads' contribution
through its Wo rows); the host sums the 8 partials.

Fully streamed schedule: causality means attention chunk tcx only needs
K/V tiles 0..4*tcx+3, so K/V/Q projections for chunk tcx+1 run *during*
attention of chunk tcx as filler work woven between QK groups (covering
the ACT-engine tanh/exp latency); O-proj of chunk tcx-1 likewise. DMA is
spread across the whole timeline instead of front-loaded. Fillers are
paced by an explicit cost model (popping too fast blocks the in-order PE
stream on un-arrived slab DMAs; too slow starves PE under ACT).

Causal diagonal trim: for key tile Tt in the diagonal block of chunk tcx
(rel = Tt-4*tcx in 0..3), query columns < 128*rel are entirely masked, so
QK / tanh / exp are column-trimmed, only the [128,128] diagonal block is
tri-masked (Pool), and PV skips s4-blocks with s4 < rel.

All matmuls are bf16; PSUM accumulation fp32; rope arithmetic fp32.

PSUM bank rule in the PV accumulation: start=True clears has_written for
the WHOLE bank and two s-chains share each bank, so only the bank's first
chain issues start=True; the sibling chain's first write lands on cleared
bits and overwrites.
"""

import os
import sys

for _p in ("/opt/trn_rl_repo", os.path.expanduser("~/.axon_site/_ro/trn_rl_repo")):
    if os.path.isdir(_p) and _p not in sys.path:
        sys.path.insert(0, _p)

import numpy as np
import ml_dtypes

import concourse.bass as bass
import concourse.tile as tile
from concourse import bacc, mybir
from concourse.bass_utils import run_bass_kernel_spmd

F32 = mybir.dt.float32
BF16 = mybir.dt.bfloat16

D_MODEL = 4096
KEY_SIZE = 128
NUM_Q_HEADS = 32
NUM_KV_HEADS = 8
N_CORES = 8
NH = NUM_Q_HEADS // NUM_KV_HEADS  # q heads per core = 4
ATTN_MULT = 0.08838834764831845
CAP = 30.0

Tanh = mybir.ActivationFunctionType.Tanh
Exp = mybir.ActivationFunctionType.Exp


def build_nc(T: int, causal: bool):
    """Emit the Bass program for one core (SPMD: all cores run this).

    Tile builds a STATIC per-engine schedule in (priority = emission)
    order, so overlap must be engineered in the emission order itself.
    """
    D = D_MODEL
    TC = 512                 # t-chunk width
    NTC = T // TC            # t-chunks
    NTT = T // 128           # 128-tiles along T (key side)
    NDT = D // 128           # contraction tiles over d_model = 32
    JW = NH * KEY_SIZE       # per-core q/o width = 512
    GW = 2                   # key tiles per QK group (1 PSUM bank each)

    SPLIT_O = os.environ.get("MHA_SPLIT_O", "0") == "1"
    EVAC_ALT = os.environ.get("MHA_EVAC_ALT", "0") == "1"
    PROJ_ORDER2 = os.environ.get("MHA_PORDER2", "0") == "1"
    DRAIN2 = os.environ.get("MHA_DRAIN2", "0") == "1"
    DEFER_O = os.environ.get("MHA_DEFER_O", "0") == "1"
    KVSPILL = os.environ.get("MHA_KVSPILL", "1") == "1"
    QSPILL = os.environ.get("MHA_QSPILL", "0") == "1"
    PIPED = int(os.environ.get("MHA_PIPED", "3"))
    FINFILL = os.environ.get("MHA_FINFILL", "0") == "1"
    VLATE = os.environ.get("MHA_VLATE", "1") == "1"

    nc = bacc.Bacc(None, target_bir_lowering=False)

    xq = nc.dram_tensor("xq", [D, T], BF16, kind="ExternalInput")
    xk = nc.dram_tensor("xk", [D, T], BF16, kind="ExternalInput")
    xv = nc.dram_tensor("xv", [D, T], BF16, kind="ExternalInput")
    wq = nc.dram_tensor("wq", [128, NH * NDT * 128], BF16, kind="ExternalInput")
    wk = nc.dram_tensor("wk", [128, NDT * 128], BF16, kind="ExternalInput")
    wv = nc.dram_tensor("wv", [128, NDT * 128], BF16, kind="ExternalInput")
    wo = nc.dram_tensor("wo", [JW, D], BF16, kind="ExternalInput")
    cosd = nc.dram_tensor("cosT", [128, T], BF16, kind="ExternalInput")
    sind = nc.dram_tensor("sinT", [128, T], BF16, kind="ExternalInput")
    rotd = nc.dram_tensor("rot", [128, 128], BF16, kind="ExternalInput")
    identbd = nc.dram_tensor("identb", [128, 128], BF16, kind="ExternalInput")
    trid = nc.dram_tensor("tri", [128, 128], BF16, kind="ExternalInput")
    outd = nc.dram_tensor("out", [T, D], BF16, kind="ExternalOutput")
    outa = nc.dram_tensor("outa", [TC, D], BF16, kind="ExternalOutput")
    outb = nc.dram_tensor("outb", [TC, D], BF16, kind="ExternalOutput")

    with tile.TileContext(nc) as tc:
        with (
            tc.tile_pool(name="const", bufs=1) as constp,
            tc.tile_pool(name="persist", bufs=1) as persist,
            tc.tile_pool(name="slabs", bufs=2) as slabp,
            tc.tile_pool(name="tmps", bufs=2) as tmpp,
            tc.tile_pool(name="pa", bufs=2) as pa,
            tc.tile_pool(name="qkps", bufs=2, space="PSUM") as qkps,
            tc.tile_pool(name="pvps", bufs=1, space="PSUM") as pvps,
            tc.tile_pool(name="mmps", bufs=2, space="PSUM") as mmps,
        ):
            # ---- persistent SBUF ----
            rot_sb = constp.tile([128, 128], BF16)
            identb_sb = constp.tile([128, 128], BF16)
            tri_sb = constp.tile([128, 128], BF16)
            cos_sb = constp.tile([128, T], BF16)
            sin_sb = constp.tile([128, T], BF16)
            kT_rope = persist.tile([128, T], BF16)
            vaug = persist.tile([128, NTT, 132], BF16)
            wq_sb = persist.tile([128, NH, NDT, 128], BF16)
            wk_sb = persist.tile([128, NDT, 128], BF16)
            wv_sb = persist.tile([128, NDT, 128], BF16)
            wo_sb = persist.tile([128, NH, D], BF16)

            # ---- tiny consts first (clears the DMA queue fast) ----
            nc.sync.dma_start(out=rot_sb, in_=rotd[:])
            nc.sync.dma_start(out=identb_sb, in_=identbd[:])
            nc.sync.dma_start(out=tri_sb, in_=trid[:])
            nc.any.memset(vaug[:, :, 128:132], 1.0)

            # ---------------- emit-helper closures ----------------
            # Filler items are (cost_ns, fn) pairs.

            def kv_fillers(xsrc, w_sb, tch, dst_cb):
                """K or V projection of t-columns [tch*512,(tch+1)*512)."""
                st = {}

                def dma_i(i, half):
                    def f():
                        if half == 0:
                            st[i] = slabp.tile(
                                [128, 8, TC], BF16, tag="kvslab", bufs=3,
                                name="kvslab",
                            )
                        nc.sync.dma_start(
                            out=st[i][:, 4 * half : 4 * half + 4, :],
                            in_=xsrc[
                                i * 1024 + half * 512 :
                                i * 1024 + (half + 1) * 512,
                                tch * TC : (tch + 1) * TC,
                            ].rearrange("(n k) t -> k n t", k=128),
                        )
                    return {"cost": 100, "fn": f, "dma": 1456, "kind": "dma",
                            "grp": "kv", "bar": tch}

                def comp_i(i):
                    def f():
                        if i == 0:
                            st["ps"] = mmps.tile(
                                [128, TC], F32, tag="mm", name="kv_ps"
                            )
                        ps = st["ps"]
                        for j in range(8):
                            nc.tensor.matmul(
                                ps,
                                w_sb[:, i * 8 + j, :],
                                st[i][:, j, :],
                                start=(i == 0 and j == 0),
                                stop=(i == 3 and j == 7),
                            )
                        if i == 3:
                            dst_cb(ps)
                    return {"cost": 1710, "fn": f, "dma": 0, "kind": "comp",
                            "grp": "kv", "bar": tch}

                return [dma_i(0, 0), dma_i(0, 1), dma_i(1, 0), comp_i(0),
                        dma_i(1, 1), dma_i(2, 0), comp_i(1), dma_i(2, 1),
                        dma_i(3, 0), comp_i(2), dma_i(3, 1), comp_i(3)]

            def rope(dst, src, t0, tw):
                """dst[128, tw] = RoPE(src[128, tw]) at positions t0.. (fp32
                math; src/dst bf16)."""
                rp = mmps.tile([128, TC], F32, tag="mm", name="rope_ps")
                nc.tensor.matmul(rp[:, :tw], rot_sb, src, start=True, stop=True)
                t1 = pa.tile([128, TC], F32, tag="rt1", bufs=1, name="rope_t1")
                nc.gpsimd.tensor_mul(t1[:, :tw], src, cos_sb[:, t0 : t0 + tw])
                t2 = pa.tile([128, TC], F32, tag="rt2", bufs=1, name="rope_t2")
                nc.vector.tensor_mul(t2[:, :tw], rp[:, :tw], sin_sb[:, t0 : t0 + tw])
                nc.vector.tensor_add(dst, t1[:, :tw], t2[:, :tw])

            def k_chunk_fillers(tch):
                ktmp = tmpp.tile([128, TC], BF16, tag="ktmp", name="ktmp")

                def evac(ps):
                    nc.vector.tensor_copy(ktmp, ps)

                items = kv_fillers(xk, wk_sb, tch, evac)

                def rope_k():
                    rope(kT_rope[:, tch * TC : (tch + 1) * TC], ktmp,
                         tch * TC, TC)

                return items + [{"cost": 350, "fn": rope_k, "dma": 0,
                                 "kind": "comp", "grp": "kv", "bar": tch}]

            def v_chunk_fillers(tch):
                vtmp = tmpp.tile([128, TC], BF16, tag="vtmp", name="vtmp")

                def evac(ps):
                    nc.vector.tensor_copy(vtmp, ps)

                items = kv_fillers(xv, wv_sb, tch, evac)

                def vtr(half):
                    def f():
                        for b2 in range(2):
                            b = 4 * tch + 2 * half + b2
                            tp = mmps.tile(
                                [128, TC], BF16, tag="mm", name="vtr_ps"
                            )
                            nc.tensor.transpose(
                                tp[:, :128],
                                vtmp[:, (2 * half + b2) * 128 :
                                     (2 * half + b2 + 1) * 128],
                                identb_sb,
                            )
                            nc.vector.tensor_copy(vaug[:, b, 0:128], tp[:, :128])
                    return {"cost": 220, "fn": f, "dma": 0, "kind": "comp",
                            "grp": "kv", "bar": tch}

                return items + [vtr(0), vtr(1)]

            def qslab_dma_fillers(tcx):
                slabs = []

                def dma_h(dh, q):
                    def f():
                        if q == 0:
                            slab = slabp.tile(
                                [128, 16, TC], BF16, tag="qslab", name="qslab"
                            )
                            slabs.append(slab)
                        slab = slabs[dh]
                        nc.sync.dma_start(
                            out=slab[:, 4 * q : 4 * q + 4, :],
                            in_=xq[
                                dh * 2048 + q * 512 : dh * 2048 + (q + 1) * 512,
                                tcx * TC : (tcx + 1) * TC,
                            ].rearrange("(n k) t -> k n t", k=128),
                        )
                    return {"cost": 100, "fn": f, "dma": 1456, "kind": "dma",
                            "grp": "pre"}

                return slabs, [dma_h(0, q) for q in range(4)] + [
                    dma_h(1, q) for q in range(4)
                ]

            def qproj_chain(slabs, qraw, jh):
                ps = mmps.tile([128, TC], F32, tag="mm", name="q_ps")
                for dh in range(2):
                    for i in range(16):
                        nc.tensor.matmul(
                            ps,
                            wq_sb[:, jh, dh * 16 + i, :],
                            slabs[dh][:, i, :],
                            start=(dh == 0 and i == 0),
                            stop=(dh == 1 and i == 15),
                        )
                nc.vector.tensor_copy(qraw[:, jh, :], ps)

            def q_chunk_fillers(tcx, slabs):
                """Q proj + rope for chunk tcx; returns (qrope, items)."""
                qraw = tmpp.tile([128, NH, TC], BF16, tag="qraw", name="qraw")
                qrope = tmpp.tile([128, NH, TC], BF16, tag="qrope", bufs=2, name="qrope")
                items = []
                for jh in range(NH):
                    items.append({
                        "cost": 6830, "dma": 0, "kind": "comp", "grp": "q",
                        "bar": tcx, "qbar": jh,
                        "fn": lambda jh=jh: qproj_chain(slabs, qraw, jh),
                    })
                for jh in range(NH):
                    items.append({
                        "cost": 350, "dma": 0, "kind": "comp", "grp": "q",
                        "bar": tcx, "qbar": jh,
                        "fn": lambda jh=jh: rope(
                            qrope[:, jh, :], qraw[:, jh, :], tcx * TC, TC
                        ),
                    })
                return qrope, items

            def make_oproj_fillers(attnT, t0, jhs=range(NH), dest=None,
                                   dest_t0=None, evac_alt=False,
                                   use_qkps=False):
                dest = outd if dest is None else dest
                dest_t0 = t0 if dest_t0 is None else dest_t0
                jhs = list(jhs)
                fillers = []
                for nch in range(D // TC):
                    for s4 in range(4):
                        def f(s4=s4, nch=nch):
                            with nc.named_scope("oproj"):
                                if use_qkps and (s4 + nch) % 2 == 0:
                                    # qk PSUM banks are idle in the tail:
                                    # alternate into them for a deeper
                                    # chain pipeline
                                    ps = qkps.tile(
                                        [128, GW, TC], F32, tag="qk",
                                        name="o_ps2",
                                    )[:, 0, :]
                                else:
                                    ps = mmps.tile(
                                        [128, TC], F32, tag="mm", name="o_ps"
                                    )
                                for x, jh in enumerate(jhs):
                                    nc.tensor.matmul(
                                        ps,
                                        attnT[:, jh, s4 * 128 : (s4 + 1) * 128],
                                        wo_sb[:, jh, nch * TC : (nch + 1) * TC],
                                        start=(x == 0),
                                        stop=(x == len(jhs) - 1),
                                    )
                                osb = pa.tile(
                                    [128, TC], BF16, tag="osb", bufs=4,
                                    name="osb",
                                )
                                if evac_alt and (s4 + nch) % 2 == 0:
                                    nc.scalar.copy(out=osb, in_=ps)
                                else:
                                    nc.vector.tensor_copy(osb, ps)
                                nc.sync.dma_start(
                                    out=dest[
                                        dest_t0 + s4 * 128 :
                                        dest_t0 + (s4 + 1) * 128,
                                        nch * TC : (nch + 1) * TC,
                                    ],
                                    in_=osb,
                                )
                        fillers.append(
                            {"cost": 218 * len(jhs), "fn": f,
                             "dma": 364, "kind": "oproj", "grp": "o"}
                        )
                return fillers

            def interleave(a, b):
                out = []
                ia = ib = 0
                na, nb = len(a), len(b)
                while ia < na or ib < nb:
                    if ia * max(nb, 1) <= ib * max(na, 1) and ia < na:
                        out.append(a[ia]); ia += 1
                    elif ib < nb:
                        out.append(b[ib]); ib += 1
                    else:
                        out.append(a[ia]); ia += 1
                return out

            def proj_items_for(tcx):
                """All projection work for chunk tcx as a filler list, DMA
                items placed so transfers land just ahead of their use."""
                slabs_n, qdma = qslab_dma_fillers(tcx)
                kn = k_chunk_fillers(tcx)
                vn = v_chunk_fillers(tcx)
                qrope_n, qn = q_chunk_fillers(tcx, slabs_n)
                qpairs = [qn[0], qn[NH], qn[1], qn[NH + 1], qn[2],
                          qn[NH + 2], qn[3], qn[NH + 3]]
                if VLATE:
                    # v-stream last: its data isn't needed until group
                    # 2*tcx of the NEXT attention window (kv barrier), so
                    # keep the congested window's DMA queue for k/q
                    items = (
                        [kn[0], kn[1], qdma[0], qdma[1], kn[2], kn[3],
                         qdma[2], qdma[3], kn[4], kn[5], qdma[4], qdma[5],
                         kn[6], kn[7], qdma[6], qdma[7], kn[8], kn[9],
                         kn[10], kn[11], kn[12]]
                        + qpairs
                        + vn[:12] + [vn[12], vn[13]]
                    )
                else:
                    items = (
                        [kn[0], kn[1], qdma[0], qdma[1], kn[2], kn[3],
                         qdma[2], qdma[3], kn[4], kn[5], qdma[4], qdma[5],
                         kn[6], kn[7], qdma[6], qdma[7], kn[8], kn[9],
                         kn[10], kn[11], kn[12]]
                        + vn[:12] + [vn[12], vn[13]]
                        + qpairs
                    )
                return qrope_n, items

            # ---------------- chunk 0 prologue (inline, DMA-ordered) ----
            # Critical path to the first q chain: wq head 0 + both qslabs;
            # everything else (k/v slabs, cos/sin) streams behind and PE
            # picks it up between/after the q chains.
            def wq_head_dma(jh):
                nc.sync.dma_start(
                    out=wq_sb[:, jh, :, :],
                    in_=wq[:, jh * NDT * 128 : (jh + 1) * NDT * 128].rearrange(
                        "k (n j) -> k n j", j=128
                    ),
                )

            k0 = k_chunk_fillers(0)
            v0 = v_chunk_fillers(0)
            qslabs0, qdma0 = qslab_dma_fillers(0)
            wq_head_dma(0)
            for it in qdma0:                   # 8 quarter-slab dmas
                it["fn"]()
            wq_head_dma(1)
            nc.sync.dma_start(out=wk_sb, in_=wk.rearrange("k (n j) -> k n j", j=128))
            wq_head_dma(2)
            for it in k0[0:3]:                 # kslab dmas
                it["fn"]()
            wq_head_dma(3)
            qrope0, q0 = q_chunk_fillers(0, qslabs0)
            q0[0]["fn"](); q0[1]["fn"]()       # qproj chains 0,1
            k0[4]["fn"](); k0[5]["fn"]()       # kslab dmas
            q0[2]["fn"]()                      # qproj chain 2
            k0[3]["fn"]()                      # comp k piece 0
            nc.sync.dma_start(out=cos_sb, in_=cosd[:])
            q0[3]["fn"]()                      # qproj chain 3
            nc.sync.dma_start(out=sin_sb, in_=sind[:])
            k0[7]["fn"](); k0[8]["fn"](); k0[10]["fn"]()   # kslab dmas
            k0[6]["fn"](); k0[9]["fn"](); k0[11]["fn"]()   # comp k 1-3 + evac
            nc.sync.dma_start(out=wv_sb, in_=wv.rearrange("k (n j) -> k n j", j=128))
            k0[12]["fn"]()                     # rope-k(0)
            for it in q0[NH:]:                 # 4 rope-q(0)
                it["fn"]()
            v0[0]["fn"](); v0[1]["fn"](); v0[2]["fn"]()    # vslab dmas
            v0[3]["fn"]()                      # comp v piece 0
            v0[4]["fn"](); v0[5]["fn"]()       # vslab dmas
            v0[6]["fn"]()                      # comp v piece 1
            v0[7]["fn"](); v0[8]["fn"]()       # vslab dmas
            v0[9]["fn"]()                      # comp v piece 2
            v0[10]["fn"]()                     # vslab dma
            v0[11]["fn"]()                     # comp v piece 3 + evac
            v0[12]["fn"](); v0[13]["fn"]()     # vtr halves

            def wo_slice_dma(nch):
                def f():
                    nc.sync.dma_start(
                        out=wo_sb[:, :, nch * TC : (nch + 1) * TC],
                        in_=wo[:, nch * TC : (nch + 1) * TC].rearrange(
                            "(n k) d -> k n d", k=128
                        ),
                    )
                return {"cost": 100, "fn": f, "dma": 1456, "kind": "dma",
                        "grp": "pre"}

            # ---------------- main loop over t-chunks ----------------
            qrope_cur = qrope0
            prev_attnT = None
            prev_t0 = 0
            carry = []          # deferred oproj fillers from chunk tcx-1
            kv_carry = []       # K/V-proj fillers spilled into their own
                                # attention window (barrier at group 2*tcx)
            for tcx in range(NTC):
                t0 = tcx * TC

                if tcx + 1 < NTC:
                    qrope_next, proj_items = proj_items_for(tcx + 1)
                else:
                    qrope_next, proj_items = None, []
                # wo: first 2 slices during attn(0) (needed by the first
                # oproj pops early in attn(1)), the rest during attn(1)
                # where the DMA queue has slack.
                if tcx == 0:
                    wos = [wo_slice_dma(n) for n in range(D // TC)]
                    proj_items = interleave(proj_items, wos[:2])
                elif tcx == 1:
                    proj_items = interleave(proj_items, wos[2:])
                oproj_items = carry + (
                    make_oproj_fillers(prev_attnT, prev_t0)
                    if prev_attnT is not None
                    else []
                )
                if DEFER_O:
                    if tcx == 1:
                        deferred_o = oproj_items
                        oproj_items = []
                    elif tcx == 2:
                        oproj_items = deferred_o + oproj_items
                fillers = kv_carry + interleave(proj_items, oproj_items)
                kv_carry = []
                # annotate each compute item with the cumulative input-DMA
                # time that precedes it in this window's queue — popping it
                # earlier than that would head-of-line block the in-order
                # PE stream on an un-arrived transfer.
                cum_dma = 0.0
                for it in fillers:
                    if it["kind"] == "dma":
                        cum_dma += it["dma"]
                    if it["kind"] == "comp":
                        it["ready"] = cum_dma
                    elif it["kind"] == "oproj" and tcx == 1:
                        # wo slices still streaming in this window
                        it["ready"] = cum_dma
                    else:
                        it["ready"] = 0.0

                nt_valid = 4 * (tcx + 1) if causal else NTT
                ngroups = nt_valid // GW
                attnT = pa.tile(
                    [128, NH, TC], BF16, tag="attnT", bufs=3, name="attnT"
                )
                budget = 0.0
                popped = 0.0
                qkpv_clock = 0.0
                act_clock = 0.0
                popped_dma = 0.0
                SLACK = float(os.environ.get("MHA_SLACK", "2000"))
                LOOKAHEAD = float(os.environ.get("MHA_LOOKAHEAD", "9000"))
                BMULT = float(os.environ.get("MHA_BMULT", "1.0"))

                def pop_fillers():
                    nonlocal popped, popped_dma
                    while popped < budget and fillers:
                        elapsed = max(act_clock, qkpv_clock + popped)
                        # pull any leading dma items (keep the queue fed,
                        # but no more than LOOKAHEAD ahead of real time)
                        i = 0
                        progress = False
                        while i < len(fillers):
                            it = fillers[i]
                            if (it["kind"] == "dma"
                                    and popped_dma < elapsed + LOOKAHEAD):
                                fillers.pop(i)
                                it["fn"]()
                                popped_dma += it["dma"]
                                progress = True
                                continue
                            if it["kind"] != "dma":
                                break
                            i += 1
                        if not fillers or popped >= budget:
                            break
                        head = fillers[0]
                        if (head["kind"] != "dma"
                                and head["ready"] <= elapsed + SLACK):
                            fillers.pop(0)
                            head["fn"]()
                            popped += head["cost"]
                            popped_dma += head["dma"]
                            progress = True
                        elif head["kind"] != "dma":
                            # head blocked: pop a later independent item
                            # (oproj / q are reorderable; kv chains are not)
                            for j in range(1, min(len(fillers), 12)):
                                itj = fillers[j]
                                if (itj["kind"] != "dma"
                                        and itj.get("grp") in ("o", "q")
                                        and itj["ready"] <= elapsed + SLACK):
                                    fillers.pop(j)
                                    itj["fn"]()
                                    popped += itj["cost"]
                                    popped_dma += itj["dma"]
                                    progress = True
                                    break
                        if not progress:
                            break
                for h in range(NH):
                    if tcx >= 1:
                        i = 0
                        while i < len(fillers):
                            it = fillers[i]
                            if (it.get("grp") == "q" and it.get("bar") == tcx
                                    and it.get("qbar", 9) <= h):
                                fillers.pop(i)
                                it["fn"]()
                                popped += it["cost"]
                                popped_dma += it["dma"]
                            else:
                                i += 1
                    pend = []
                    with nc.named_scope("attn"):
                        pv = pvps.tile(
                            [128, 4, 256], F32, tag="pv", name="pv_ps"
                        )
                        for gg in range(ngroups):
                            if h == 0 and tcx >= 1 and gg == (
                                2 * tcx if causal else 0
                            ):
                                # force-drain this chunk's spilled K/V work:
                                # the next QK group reads the new tiles
                                i = 0
                                while i < len(fillers):
                                    if fillers[i].get("bar") == tcx:
                                        it = fillers.pop(i)
                                        it["fn"]()
                                        popped += it["cost"]
                                        popped_dma += it["dma"]
                                    else:
                                        i += 1
                            qk = qkps.tile(
                                [128, GW, TC], F32, tag="qk", name="qk_ps"
                            )
                            rels = []
                            for b in range(GW):
                                Tt = GW * gg + b
                                rel = Tt - 4 * tcx if causal else -1
                                rels.append(rel)
                                c0 = 128 * rel if rel > 0 else 0
                                nc.tensor.matmul(
                                    qk[:, b, c0:TC],
                                    kT_rope[:, Tt * 128 : (Tt + 1) * 128],
                                    qrope_cur[:, h, c0:TC],
                                    start=True,
                                    stop=True,
                                )
                            # tanh in place in PSUM, then exp to bf16 SBUF;
                            # soft-capping scales fused into ACT. Columns
                            # below the causal diagonal are skipped.
                            pt = pa.tile(
                                [128, GW, TC], BF16, tag="pt", bufs=int(os.environ.get("MHA_PTBUFS", "4")),
                                name="ptile",
                            )
                            act_cols = 0
                            if max(rels) <= 0:
                                nc.scalar.activation(
                                    out=qk, in_=qk, func=Tanh,
                                    scale=ATTN_MULT / CAP,
                                )
                                nc.scalar.activation(
                                    out=pt, in_=qk, func=Exp, scale=CAP
                                )
                                act_cols = GW * TC
                            else:
                                for b in range(GW):
                                    c0 = 128 * max(rels[b], 0)
                                    nc.scalar.activation(
                                        out=qk[:, b, c0:TC],
                                        in_=qk[:, b, c0:TC],
                                        func=Tanh, scale=ATTN_MULT / CAP,
                                    )
                                    nc.scalar.activation(
                                        out=pt[:, b, c0:TC],
                                        in_=qk[:, b, c0:TC],
                                        func=Exp, scale=CAP,
                                    )
                                    act_cols += TC - c0
                            for b in range(GW):
                                rel = rels[b]
                                if 0 <= rel < 4:
                                    # triangular mask on the diagonal block
                                    nc.gpsimd.tensor_mul(
                                        pt[:, b, rel * 128 : (rel + 1) * 128],
                                        pt[:, b, rel * 128 : (rel + 1) * 128],
                                        tri_sb,
                                    )
                            # software-pipelined PV: emit the PREVIOUS
                            # group's PV now, so it reaches PE well after
                            # its exp() finished on ACT (the current QK +
                            # fillers cover the ACT latency).
                            def emit_pv(p_pt, p_rels, p_gg):
                                n_pv = 0
                                for s4 in range(4):
                                    for b in range(GW):
                                        Tt = GW * p_gg + b
                                        rel = p_rels[b]
                                        if causal and rel > s4:
                                            continue
                                        n_pv += 1
                                        nc.tensor.matmul(
                                            pv[:, s4, 0:129],
                                            p_pt[:, b, s4 * 128 : (s4 + 1) * 128],
                                            vaug[:, Tt, 0:129],
                                            start=(
                                                p_gg == 0 and b == 0
                                                and s4 % 2 == 0
                                            ),
                                            stop=(
                                                (Tt == 4 * tcx + s4)
                                                if causal
                                                else (p_gg == ngroups - 1
                                                      and b == GW - 1)
                                            ),
                                            skip_group_check=True,
                                        )
                                return n_pv

                            n_pv = 0
                            pend.append((pt, rels, gg))
                            if len(pend) > PIPED:
                                n_pv = emit_pv(*pend.pop(0))
                            # weave fillers so PE stays busy under ACT
                            act_ns = act_cols * 2 * 0.833 + (
                                330 if max(rels) <= 0 else 660
                            )
                            qkpv_ns = (act_cols + 129 * n_pv) * 0.4167
                            act_clock += act_ns
                            qkpv_clock += qkpv_ns
                            budget += BMULT * max(act_ns - qkpv_ns, 0.0)
                            pop_fillers()
                        while pend:
                            emit_pv(*pend.pop(0))
                    with nc.named_scope("attn_fin"):
                        ans = []
                        for s4 in range(4):
                            rc = pa.tile(
                                [128, 1], F32, tag="rc", bufs=4, name="rc"
                            )
                            nc.vector.reciprocal(rc, pv[:, s4, 128:129])
                            an = pa.tile(
                                [128, 128], BF16, tag="an", bufs=4, name="an"
                            )
                            nc.vector.tensor_scalar_mul(an, pv[:, s4, 0:128], rc)
                            ans.append(an)
                        # cover the DVE normalize latency with a filler
                        budget += 700
                        act_clock += 700
                        pop_fillers()
                        if FINFILL:
                            # transposes aren't needed until next chunk's
                            # O-proj: queue them as fillers instead of
                            # serializing at the head boundary
                            def fin_tr(ans=ans, h=h):
                                for s4 in range(4):
                                    tp = mmps.tile(
                                        [128, TC], BF16, tag="mm", name="atr"
                                    )
                                    nc.tensor.transpose(
                                        tp[:, :128], ans[s4], identb_sb
                                    )
                                    nc.vector.tensor_copy(
                                        attnT[:, h, s4 * 128 : (s4 + 1) * 128],
                                        tp[:, :128],
                                    )
                            fillers.insert(0, {
                                "cost": 900, "fn": fin_tr, "dma": 0,
                                "kind": "oproj", "grp": "pre", "ready": 0.0,
                            })
                        else:
                            for s4 in range(4):
                                tp = mmps.tile(
                                    [128, TC], BF16, tag="mm", name="atr"
                                )
                                nc.tensor.transpose(
                                    tp[:, :128], ans[s4], identb_sb
                                )
                                nc.vector.tensor_copy(
                                    attnT[:, h, s4 * 128 : (s4 + 1) * 128],
                                    tp[:, :128],
                                )
                    if SPLIT_O and tcx == NTC - 1 and h == 1:
                        for it in make_oproj_fillers(
                            attnT, t0, jhs=[0, 1], dest=outa, dest_t0=0
                        ):
                            it["ready"] = 0.0
                            fillers.append(it)
                # drain: 'pre' items (q proj/rope of tc+1) must finish
                # before attn(tcx+1) emits its first QK; K/V items of tc+1
                # spill into attn(tcx+1) (barrier at group 2*(tcx+1)), and
                # up to MHA_CARRY oproj items carry over (attnT bufs=3).
                carry = []
                rest = fillers
                if tcx + 1 < NTC:
                    cap = int(os.environ.get("MHA_CARRY", "16"))
                    o_total = sum(1 for it in rest if it["grp"] == "o")
                    drain_o = max(0, o_total - cap)
                    drain = []
                    for it in rest:
                        if it["grp"] == "pre":
                            drain.append(it)
                        elif it["grp"] == "q":
                            if it["qbar"] == 0 or not QSPILL:
                                drain.append(it)
                            else:
                                kv_carry.append(it)
                        elif it["grp"] == "o" and drain_o > 0:
                            drain.append(it)
                            drain_o -= 1
                        elif it["grp"] == "kv" and KVSPILL:
                            kv_carry.append(it)
                        elif it["grp"] == "kv":
                            drain.append(it)
                        else:
                            carry.append(it)
                    rest = drain
                # drain with the same dma-forwarding discipline: keep
                # transfers ~LOOKAHEAD ahead of the estimated PE clock so
                # in-order compute items rarely wait on arrival.
                if not DRAIN2:
                    for it in rest:
                        it["fn"]()
                    rest = []
                el = max(act_clock, qkpv_clock + popped)
                dma_el = popped_dma
                while rest:
                    i = 0
                    while i < len(rest):
                        if (rest[i]["kind"] == "dma"
                                and dma_el < el + LOOKAHEAD):
                            it = rest.pop(i)
                            it["fn"]()
                            dma_el += it["dma"]
                            continue
                        if rest[i]["kind"] != "dma":
                            break
                        i += 1
                    if not rest:
                        break
                    it = rest.pop(0)
                    it["fn"]()
                    el = max(el, it.get("ready", 0.0)) + it["cost"]
                    dma_el += it["dma"]
                qrope_cur = qrope_next
                prev_attnT, prev_t0 = attnT, t0

            # tail: O proj pass B of the last chunk (host adds outa+outb)
            tail_items = (
                make_oproj_fillers(prev_attnT, prev_t0, jhs=[2, 3],
                                   dest=outb, dest_t0=0, evac_alt=True)
                if SPLIT_O
                else make_oproj_fillers(prev_attnT, prev_t0, evac_alt=True,
                                        use_qkps=True)
            )
            for it in carry + tail_items:
                it["fn"]()

    nc.compile()
    return nc


def vbgd_dst(vaug):
    return vaug[:, :, 128:132]


def _host_constants(T: int):
    d = KEY_SIZE
    inv_freq = 1.0 / (10000.0 ** (np.arange(0, d, 2, dtype=np.float64) / d))  # [64]
    pos = np.arange(T, dtype=np.float64)
    phase_half = pos[None, :] * inv_freq[:, None]  # [64, T]
    phase = np.concatenate([phase_half, phase_half], axis=0)  # [128, T] (tiled)
    cosT = np.cos(phase).astype(np.float32)
    sinT = np.sin(phase).astype(np.float32)

    R = np.zeros((128, 128), dtype=np.float32)
    R[:64, 64:] = -np.eye(64, dtype=np.float32)
    R[64:, :64] = np.eye(64, dtype=np.float32)
    rot = np.ascontiguousarray(R.T)

    ident = np.eye(128, dtype=np.float32)

    # tri[k, c] = 1 if k <= c (valid: query col >= key row inside the
    # diagonal 128x128 block)
    tri = (np.arange(128)[:, None] <= np.arange(128)[None, :]).astype(
        ml_dtypes.bfloat16
    )

    NTT = T // 128
    vbg = np.zeros((128, NTT, 4), dtype=ml_dtypes.bfloat16)
    vbg[:, :, 0] = 1.0
    return cosT, sinT, rot, ident, tri, vbg


_NC_CACHE: dict = {}
LAST_RESULT = None
_LAST_IN_MAPS = None


def kernel(query, key, value, mask, Wq, Wk, Wv, Wo):
    global LAST_RESULT, _LAST_IN_MAPS
    query = np.asarray(query)
    key = np.asarray(key)
    value = np.asarray(value)
    mask = np.asarray(mask)
    Wq = np.asarray(Wq, dtype=np.float32)
    Wk = np.asarray(Wk, dtype=np.float32)
    Wv = np.asarray(Wv, dtype=np.float32)
    Wo = np.asarray(Wo, dtype=np.float32)

    b, T, D = query.shape
    assert b == 1 and D == D_MODEL, (b, D)

    m2 = np.asarray(mask).reshape(T, T).astype(bool)
    if np.array_equal(m2, np.tril(np.ones((T, T), dtype=bool))):
        causal = True
    elif m2.all():
        causal = False
    else:
        raise ValueError("unsupported mask pattern (expected causal or full)")

    kkey = (T, causal)
    if kkey not in _NC_CACHE:
        _NC_CACHE[kkey] = build_nc(T, causal)
    nc = _NC_CACHE[kkey]

    pnp = ml_dtypes.bfloat16
    xq = np.ascontiguousarray(query[0].T).astype(pnp)  # [D, T]
    xk = np.ascontiguousarray(key[0].T).astype(pnp)
    xv = np.ascontiguousarray(value[0].T).astype(pnp)
    cosT, sinT, rot, ident, tri, vbg = _host_constants(T)

    JW = NH * KEY_SIZE
    NDT = D // 128

    def pack_w(w, nh):
        # [D, nh*128] -> [k, jh, n, j] flattened per-partition-contiguous
        a = np.ascontiguousarray(w).astype(pnp)
        a = a.reshape(NDT, 128, nh, 128).transpose(1, 2, 0, 3)
        return np.ascontiguousarray(a.reshape(128, nh * NDT * 128))

    in_maps = []
    for c in range(N_CORES):
        in_maps.append(
            {
                "xq": xq,
                "xk": xk,
                "xv": xv,
                "wq": pack_w(Wq[:, c * JW : (c + 1) * JW], NH),
                "wk": pack_w(Wk[:, c * KEY_SIZE : (c + 1) * KEY_SIZE], 1),
                "wv": pack_w(Wv[:, c * KEY_SIZE : (c + 1) * KEY_SIZE], 1),
                "wo": np.ascontiguousarray(Wo[c * JW : (c + 1) * JW, :]).astype(pnp),
                "cosT": cosT.astype(pnp),
                "sinT": sinT.astype(pnp),
                "rot": rot.astype(pnp),
                "identb": ident.astype(pnp),
                "tri": tri,
            }
        )

    _LAST_IN_MAPS = in_maps
    trace = os.environ.get("MHA_TRACE") == "1"
    res = run_bass_kernel_spmd(nc, in_maps, list(range(N_CORES)), trace=trace)
    LAST_RESULT = res

    out = np.zeros((T, D), dtype=np.float64)
    for c in range(N_CORES):
        out += res.results[c]["out"].astype(np.float64)
    return out.astype(np.float32).reshape(1, T, D)


# revision 47
# speedup vs baseline: 1.2590x; 1.0013x over previous
"""Trainium2 Bass kernel for nn_MultiHeadAttention_83056077570808.

GQA multi-head attention (32 q heads, 8 kv heads, d_head=128, T=2048,
D=4096) with RoPE, tanh soft-capping at 30, causal mask, fp32 reference.

Sharding: tensor-parallel over heads across 8 cores. Core c owns kv head c
and q heads 4c..4c+3: Wq/Wk/Wv column-sharded, Wo row-sharded; activations
replicated. Each core computes a partial output (its heads' contribution
through its Wo rows); the host sums the 8 partials.

Fully streamed schedule: causality means attention chunk tcx only needs
K/V tiles 0..4*tcx+3, so K/V/Q projections for chunk tcx+1 run *during*
attention of chunk tcx as filler work woven between QK groups (covering
the ACT-engine tanh/exp latency); O-proj of chunk tcx-1 likewise. DMA is
spread across the whole timeline instead of front-loaded. Fillers are
paced by an explicit cost model (popping too fast blocks the in-order PE
stream on un-arrived slab DMAs; too slow starves PE under ACT).

Causal diagonal trim: for key tile Tt in the diagonal block of chunk tcx
(rel = Tt-4*tcx in 0..3), query columns < 128*rel are entirely masked, so
QK / tanh / exp are column-trimmed, only the [128,128] diagonal block is
tri-masked (Pool), and PV skips s4-blocks with s4 < rel.

All matmuls are bf16; PSUM accumulation fp32; rope arithmetic fp32.

PSUM bank rule in the PV accumulation: start=True clears has_written for
the WHOLE bank and two s-chains share each bank, so only the bank's first
chain issues start=True; the sibling chain's first write lands on cleared
bits and overwrites.
"""

import os
import sys

for _p in ("/opt/trn_rl_repo", os.path.expanduser("~/.axon_site/_ro/trn_rl_repo")):
    if os.path.isdir(_p) and _p not in sys.path:
        sys.path.insert(0, _p)

import numpy as np
import ml_dtypes

import concourse.bass as bass
import concourse.tile as tile
from concourse import bacc, mybir
from concourse.bass_utils import run_bass_kernel_spmd

F32 = mybir.dt.float32
BF16 = mybir.dt.bfloat16

D_MODEL = 4096
KEY_SIZE = 128
NUM_Q_HEADS = 32
NUM_KV_HEADS = 8
N_CORES = 8
NH = NUM_Q_HEADS // NUM_KV_HEADS  # q heads per core = 4
ATTN_MULT = 0.08838834764831845
CAP = 30.0

Tanh = mybir.ActivationFunctionType.Tanh
Exp = mybir.ActivationFunctionType.Exp


def build_nc(T: int, causal: bool):
    """Emit the Bass program for one core (SPMD: all cores run this).

    Tile builds a STATIC per-engine schedule in (priority = emission)
    order, so overlap must be engineered in the emission order itself.
    """
    D = D_MODEL
    TC = 512                 # t-chunk width
    NTC = T // TC            # t-chunks
    NTT = T // 128           # 128-tiles along T (key side)
    NDT = D // 128           # contraction tiles over d_model = 32
    JW = NH * KEY_SIZE       # per-core q/o width = 512
    GW = 2                   # key tiles per QK group (1 PSUM bank each)

    SPLIT_O = os.environ.get("MHA_SPLIT_O", "0") == "1"
    EVAC_ALT = os.environ.get("MHA_EVAC_ALT", "0") == "1"
    PROJ_ORDER2 = os.environ.get("MHA_PORDER2", "0") == "1"
    DRAIN2 = os.environ.get("MHA_DRAIN2", "0") == "1"
    DEFER_O = os.environ.get("MHA_DEFER_O", "0") == "1"
    KVSPILL = os.environ.get("MHA_KVSPILL", "1") == "1"
    QSPILL = os.environ.get("MHA_QSPILL", "0") == "1"
    PIPED = int(os.environ.get("MHA_PIPED", "3"))
    FINFILL = os.environ.get("MHA_FINFILL", "0") == "1"
    VLATE = os.environ.get("MHA_VLATE", "1") == "1"

    nc = bacc.Bacc(None, target_bir_lowering=False)

    xq = nc.dram_tensor("xq", [D, T], BF16, kind="ExternalInput")
    xk = nc.dram_tensor("xk", [D, T], BF16, kind="ExternalInput")
    xv = nc.dram_tensor("xv", [D, T], BF16, kind="ExternalInput")
    wq = nc.dram_tensor("wq", [128, NH * NDT * 128], BF16, kind="ExternalInput")
    wk = nc.dram_tensor("wk", [128, NDT * 128], BF16, kind="ExternalInput")
    wv = nc.dram_tensor("wv", [128, NDT * 128], BF16, kind="ExternalInput")
    wo = nc.dram_tensor("wo", [JW, D], BF16, kind="ExternalInput")
    cosd = nc.dram_tensor("cosT", [128, T], BF16, kind="ExternalInput")
    sind = nc.dram_tensor("sinT", [128, T], BF16, kind="ExternalInput")
    rotd = nc.dram_tensor("rot", [128, 128], BF16, kind="ExternalInput")
    identbd = nc.dram_tensor("identb", [128, 128], BF16, kind="ExternalInput")
    trid = nc.dram_tensor("tri", [128, 128], BF16, kind="ExternalInput")
    outd = nc.dram_tensor("out", [T, D], BF16, kind="ExternalOutput")
    outa = nc.dram_tensor("outa", [TC, D], BF16, kind="ExternalOutput")
    outb = nc.dram_tensor("outb", [TC, D], BF16, kind="ExternalOutput")

    with tile.TileContext(nc) as tc:
        with (
            tc.tile_pool(name="const", bufs=1) as constp,
            tc.tile_pool(name="persist", bufs=1) as persist,
            tc.tile_pool(name="slabs", bufs=2) as slabp,
            tc.tile_pool(name="tmps", bufs=2) as tmpp,
            tc.tile_pool(name="pa", bufs=2) as pa,
            tc.tile_pool(name="qkps", bufs=2, space="PSUM") as qkps,
            tc.tile_pool(name="pvps", bufs=1, space="PSUM") as pvps,
            tc.tile_pool(name="mmps", bufs=2, space="PSUM") as mmps,
        ):
            # ---- persistent SBUF ----
            rot_sb = constp.tile([128, 128], BF16)
            identb_sb = constp.tile([128, 128], BF16)
            tri_sb = constp.tile([128, 128], BF16)
            cos_sb = constp.tile([128, T], BF16)
            sin_sb = constp.tile([128, T], BF16)
            kT_rope = persist.tile([128, T], BF16)
            vaug = persist.tile([128, NTT, 132], BF16)
            wq_sb = persist.tile([128, NH, NDT, 128], BF16)
            wk_sb = persist.tile([128, NDT, 128], BF16)
            wv_sb = persist.tile([128, NDT, 128], BF16)
            wo_sb = persist.tile([128, NH, D], BF16)

            # ---- tiny consts first (clears the DMA queue fast) ----
            nc.sync.dma_start(out=rot_sb, in_=rotd[:])
            nc.sync.dma_start(out=identb_sb, in_=identbd[:])
            nc.sync.dma_start(out=tri_sb, in_=trid[:])
            nc.any.memset(vaug[:, :, 128:132], 1.0)

            # ---------------- emit-helper closures ----------------
            # Filler items are (cost_ns, fn) pairs.

            def kv_fillers(xsrc, w_sb, tch, dst_cb):
                """K or V projection of t-columns [tch*512,(tch+1)*512)."""
                st = {}

                def dma_i(i, half):
                    def f():
                        if half == 0:
                            st[i] = slabp.tile(
                                [128, 8, TC], BF16, tag="kvslab", bufs=3,
                                name="kvslab",
                            )
                        nc.sync.dma_start(
                            out=st[i][:, 4 * half : 4 * half + 4, :],
                            in_=xsrc[
                                i * 1024 + half * 512 :
                                i * 1024 + (half + 1) * 512,
                                tch * TC : (tch + 1) * TC,
                            ].rearrange("(n k) t -> k n t", k=128),
                        )
                    return {"cost": 100, "fn": f, "dma": 1456, "kind": "dma",
                            "grp": "kv", "bar": tch}

                def comp_i(i):
                    def f():
                        if i == 0:
                            st["ps"] = mmps.tile(
                                [128, TC], F32, tag="mm", name="kv_ps"
                            )
                        ps = st["ps"]
                        for j in range(8):
                            nc.tensor.matmul(
                                ps,
                                w_sb[:, i * 8 + j, :],
                                st[i][:, j, :],
                                start=(i == 0 and j == 0),
                                stop=(i == 3 and j == 7),
                            )
                        if i == 3:
                            dst_cb(ps)
                    return {"cost": 1710, "fn": f, "dma": 0, "kind": "comp",
                            "grp": "kv", "bar": tch}

                return [dma_i(0, 0), dma_i(0, 1), dma_i(1, 0), comp_i(0),
                        dma_i(1, 1), dma_i(2, 0), comp_i(1), dma_i(2, 1),
                        dma_i(3, 0), comp_i(2), dma_i(3, 1), comp_i(3)]

            def rope(dst, src, t0, tw):
                """dst[128, tw] = RoPE(src[128, tw]) at positions t0.. (fp32
                math; src/dst bf16)."""
                rp = mmps.tile([128, TC], F32, tag="mm", name="rope_ps")
                nc.tensor.matmul(rp[:, :tw], rot_sb, src, start=True, stop=True)
                t1 = pa.tile([128, TC], F32, tag="rt1", bufs=1, name="rope_t1")
                nc.gpsimd.tensor_mul(t1[:, :tw], src, cos_sb[:, t0 : t0 + tw])
                t2 = pa.tile([128, TC], F32, tag="rt2", bufs=1, name="rope_t2")
                nc.vector.tensor_mul(t2[:, :tw], rp[:, :tw], sin_sb[:, t0 : t0 + tw])
                nc.vector.tensor_add(dst, t1[:, :tw], t2[:, :tw])

            def k_chunk_fillers(tch):
                ktmp = tmpp.tile([128, TC], BF16, tag="ktmp", name="ktmp")

                def evac(ps):
                    nc.vector.tensor_copy(ktmp, ps)

                items = kv_fillers(xk, wk_sb, tch, evac)

                def rope_k():
                    rope(kT_rope[:, tch * TC : (tch + 1) * TC], ktmp,
                         tch * TC, TC)

                return items + [{"cost": 350, "fn": rope_k, "dma": 0,
                                 "kind": "comp", "grp": "kv", "bar": tch}]

            def v_chunk_fillers(tch):
                vtmp = tmpp.tile([128, TC], BF16, tag="vtmp", name="vtmp")

                def evac(ps):
                    nc.vector.tensor_copy(vtmp, ps)

                items = kv_fillers(xv, wv_sb, tch, evac)

                def vtr(half):
                    def f():
                        for b2 in range(2):
                            b = 4 * tch + 2 * half + b2
                            tp = mmps.tile(
                                [128, TC], BF16, tag="mm", name="vtr_ps"
                            )
                            nc.tensor.transpose(
                                tp[:, :128],
                                vtmp[:, (2 * half + b2) * 128 :
                                     (2 * half + b2 + 1) * 128],
                                identb_sb,
                            )
                            nc.vector.tensor_copy(vaug[:, b, 0:128], tp[:, :128])
                    return {"cost": 220, "fn": f, "dma": 0, "kind": "comp",
                            "grp": "kv", "bar": tch}

                return items + [vtr(0), vtr(1)]

            def qslab_dma_fillers(tcx):
                slabs = []

                def dma_h(dh, q):
                    def f():
                        if q == 0:
                            slab = slabp.tile(
                                [128, 16, TC], BF16, tag="qslab", name="qslab"
                            )
                            slabs.append(slab)
                        slab = slabs[dh]
                        nc.sync.dma_start(
                            out=slab[:, 4 * q : 4 * q + 4, :],
                            in_=xq[
                                dh * 2048 + q * 512 : dh * 2048 + (q + 1) * 512,
                                tcx * TC : (tcx + 1) * TC,
                            ].rearrange("(n k) t -> k n t", k=128),
                        )
                    return {"cost": 100, "fn": f, "dma": 1456, "kind": "dma",
                            "grp": "pre"}

                return slabs, [dma_h(0, q) for q in range(4)] + [
                    dma_h(1, q) for q in range(4)
                ]

            def qproj_chain(slabs, qraw, jh):
                ps = mmps.tile([128, TC], F32, tag="mm", name="q_ps")
                for dh in range(2):
                    for i in range(16):
                        nc.tensor.matmul(
                            ps,
                            wq_sb[:, jh, dh * 16 + i, :],
                            slabs[dh][:, i, :],
                            start=(dh == 0 and i == 0),
                            stop=(dh == 1 and i == 15),
                        )
                nc.vector.tensor_copy(qraw[:, jh, :], ps)

            def q_chunk_fillers(tcx, slabs):
                """Q proj + rope for chunk tcx; returns (qrope, items)."""
                qraw = tmpp.tile([128, NH, TC], BF16, tag="qraw", name="qraw")
                qrope = tmpp.tile([128, NH, TC], BF16, tag="qrope", bufs=2, name="qrope")
                items = []
                for jh in range(NH):
                    items.append({
                        "cost": 6830, "dma": 0, "kind": "comp", "grp": "q",
                        "bar": tcx, "qbar": jh,
                        "fn": lambda jh=jh: qproj_chain(slabs, qraw, jh),
                    })
                for jh in range(NH):
                    items.append({
                        "cost": 350, "dma": 0, "kind": "comp", "grp": "q",
                        "bar": tcx, "qbar": jh,
                        "fn": lambda jh=jh: rope(
                            qrope[:, jh, :], qraw[:, jh, :], tcx * TC, TC
                        ),
                    })
                return qrope, items

            def make_oproj_fillers(attnT, t0, jhs=range(NH), dest=None,
                                   dest_t0=None, evac_alt=False,
                                   use_qkps=False):
                dest = outd if dest is None else dest
                dest_t0 = t0 if dest_t0 is None else dest_t0
                jhs = list(jhs)
                fillers = []
                for nch in range(D // TC):
                    for s4 in range(4):
                        def f(s4=s4, nch=nch):
                            with nc.named_scope("oproj"):
                                if use_qkps and (s4 + nch) % 2 == 0:
                                    # qk PSUM banks are idle in the tail:
                                    # alternate into them for a deeper
                                    # chain pipeline
                                    ps = qkps.tile(
                                        [128, GW, TC], F32, tag="qk",
                                        name="o_ps2",
                                    )[:, 0, :]
                                else:
                                    ps = mmps.tile(
                                        [128, TC], F32, tag="mm", name="o_ps"
                                    )
                                for x, jh in enumerate(jhs):
                                    nc.tensor.matmul(
                                        ps,
                                        attnT[:, jh, s4 * 128 : (s4 + 1) * 128],
                                        wo_sb[:, jh, nch * TC : (nch + 1) * TC],
                                        start=(x == 0),
                                        stop=(x == len(jhs) - 1),
                                    )
                                osb = pa.tile(
                                    [128, TC], BF16, tag="osb", bufs=4,
                                    name="osb",
                                )
                                if evac_alt and (s4 + nch) % 2 == 0:
                                    nc.scalar.copy(out=osb, in_=ps)
                                else:
                                    nc.vector.tensor_copy(osb, ps)
                                nc.sync.dma_start(
                                    out=dest[
                                        dest_t0 + s4 * 128 :
                                        dest_t0 + (s4 + 1) * 128,
                                        nch * TC : (nch + 1) * TC,
                                    ],
                                    in_=osb,
                                )
                        fillers.append(
                            {"cost": 218 * len(jhs), "fn": f,
                             "dma": 364, "kind": "oproj", "grp": "o"}
                        )
                return fillers

            def interleave(a, b):
                out = []
                ia = ib = 0
                na, nb = len(a), len(b)
                while ia < na or ib < nb:
                    if ia * max(nb, 1) <= ib * max(na, 1) and ia < na:
                        out.append(a[ia]); ia += 1
                    elif ib < nb:
                        out.append(b[ib]); ib += 1
                    else:
                        out.append(a[ia]); ia += 1
                return out

            def proj_items_for(tcx):
                """All projection work for chunk tcx as a filler list, DMA
                items placed so transfers land just ahead of their use."""
                slabs_n, qdma = qslab_dma_fillers(tcx)
                kn = k_chunk_fillers(tcx)
                vn = v_chunk_fillers(tcx)
                qrope_n, qn = q_chunk_fillers(tcx, slabs_n)
                qpairs = [qn[0], qn[NH], qn[1], qn[NH + 1], qn[2],
                          qn[NH + 2], qn[3], qn[NH + 3]]
                if VLATE:
                    # v-stream last: its data isn't needed until group
                    # 2*tcx of the NEXT attention window (kv barrier), so
                    # keep the congested window's DMA queue for k/q
                    items = (
                        [kn[0], kn[1], qdma[0], qdma[1], kn[2], kn[3],
                         qdma[2], qdma[3], kn[4], kn[5], qdma[4], qdma[5],
                         kn[6], kn[7], qdma[6], qdma[7], kn[8], kn[9],
                         kn[10], kn[11], kn[12]]
                        + qpairs
                        + vn[:12] + [vn[12], vn[13]]
                    )
                else:
                    items = (
                        [kn[0], kn[1], qdma[0], qdma[1], kn[2], kn[3],
                         qdma[2], qdma[3], kn[4], kn[5], qdma[4], qdma[5],
                         kn[6], kn[7], qdma[6], qdma[7], kn[8], kn[9],
                         kn[10], kn[11], kn[12]]
                        + vn[:12] + [vn[12], vn[13]]
                        + qpairs
                    )
                return qrope_n, items

            # ---------------- chunk 0 prologue (inline, DMA-ordered) ----
            # Critical path to the first q chain: wq head 0 + both qslabs;
            # everything else (k/v slabs, cos/sin) streams behind and PE
            # picks it up between/after the q chains.
            def wq_head_dma(jh):
                nc.sync.dma_start(
                    out=wq_sb[:, jh, :, :],
                    in_=wq[:, jh * NDT * 128 : (jh + 1) * NDT * 128].rearrange(
                        "k (n j) -> k n j", j=128
                    ),
                )

            k0 = k_chunk_fillers(0)
            v0 = v_chunk_fillers(0)
            qslabs0, qdma0 = qslab_dma_fillers(0)
            wq_head_dma(0)
            for it in qdma0:                   # 8 quarter-slab dmas
                it["fn"]()
            wq_head_dma(1)
            nc.sync.dma_start(out=wk_sb, in_=wk.rearrange("k (n j) -> k n j", j=128))
            wq_head_dma(2)
            for it in k0[0:3]:                 # kslab dmas
                it["fn"]()
            wq_head_dma(3)
            qrope0, q0 = q_chunk_fillers(0, qslabs0)
            q0[0]["fn"](); q0[1]["fn"]()       # qproj chains 0,1
            k0[4]["fn"](); k0[5]["fn"]()       # kslab dmas
            q0[2]["fn"]()                      # qproj chain 2
            k0[3]["fn"]()                      # comp k piece 0
            nc.sync.dma_start(out=cos_sb, in_=cosd[:])
            q0[3]["fn"]()                      # qproj chain 3
            nc.sync.dma_start(out=sin_sb, in_=sind[:])
            k0[7]["fn"](); k0[8]["fn"](); k0[10]["fn"]()   # kslab dmas
            k0[6]["fn"](); k0[9]["fn"](); k0[11]["fn"]()   # comp k 1-3 + evac
            nc.sync.dma_start(out=wv_sb, in_=wv.rearrange("k (n j) -> k n j", j=128))
            k0[12]["fn"]()                     # rope-k(0)
            for it in q0[NH:]:                 # 4 rope-q(0)
                it["fn"]()
            v0[0]["fn"](); v0[1]["fn"](); v0[2]["fn"]()    # vslab dmas
            v0[3]["fn"]()                      # comp v piece 0
            v0[4]["fn"](); v0[5]["fn"]()       # vslab dmas
            v0[6]["fn"]()                      # comp v piece 1
            v0[7]["fn"](); v0[8]["fn"]()       # vslab dmas
            v0[9]["fn"]()                      # comp v piece 2
            v0[10]["fn"]()                     # vslab dma
            v0[11]["fn"]()                     # comp v piece 3 + evac
            v0[12]["fn"](); v0[13]["fn"]()     # vtr halves

            def wo_slice_dma(nch):
                def f():
                    nc.sync.dma_start(
                        out=wo_sb[:, :, nch * TC : (nch + 1) * TC],
                        in_=wo[:, nch * TC : (nch + 1) * TC].rearrange(
                            "(n k) d -> k n d", k=128
                        ),
                    )
                return {"cost": 100, "fn": f, "dma": 1456, "kind": "dma",
                        "grp": "pre"}

            # ---------------- main loop over t-chunks ----------------
            qrope_cur = qrope0
            prev_attnT = None
            prev_t0 = 0
            carry = []          # deferred oproj fillers from chunk tcx-1
            kv_carry = []       # K/V-proj fillers spilled into their own
                                # attention window (barrier at group 2*tcx)
            for tcx in range(NTC):
                t0 = tcx * TC

                if tcx + 1 < NTC:
                    qrope_next, proj_items = proj_items_for(tcx + 1)
                else:
                    qrope_next, proj_items = None, []
                # wo: first 2 slices during attn(0) (needed by the first
                # oproj pops early in attn(1)), the rest during attn(1)
                # where the DMA queue has slack.
                if tcx == 0:
                    wos = [wo_slice_dma(n) for n in range(D // TC)]
                    proj_items = interleave(proj_items, wos[:2])
                elif tcx == 1:
                    proj_items = interleave(proj_items, wos[2:])
                oproj_items = carry + (
                    make_oproj_fillers(prev_attnT, prev_t0)
                    if prev_attnT is not None
                    else []
                )
                if DEFER_O:
                    if tcx == 1:
                        deferred_o = oproj_items
                        oproj_items = []
                    elif tcx == 2:
                        oproj_items = deferred_o + oproj_items
                fillers = kv_carry + interleave(proj_items, oproj_items)
                kv_carry = []
                # annotate each compute item with the cumulative input-DMA
                # time that precedes it in this window's queue — popping it
                # earlier than that would head-of-line block the in-order
                # PE stream on an un-arrived transfer.
                cum_dma = 0.0
                for it in fillers:
                    if it["kind"] == "dma":
                        cum_dma += it["dma"]
                    if it["kind"] == "comp":
                        it["ready"] = cum_dma
                    elif it["kind"] == "oproj" and tcx == 1:
                        # wo slices still streaming in this window
                        it["ready"] = cum_dma
                    else:
                        it["ready"] = 0.0

                nt_valid = 4 * (tcx + 1) if causal else NTT
                ngroups = nt_valid // GW
                attnT = pa.tile(
                    [128, NH, TC], BF16, tag="attnT", bufs=3, name="attnT"
                )
                budget = 0.0
                popped = 0.0
                qkpv_clock = 0.0
                act_clock = 0.0
                popped_dma = 0.0
                SLACK = float(os.environ.get("MHA_SLACK", "3000"))
                LOOKAHEAD = float(os.environ.get("MHA_LOOKAHEAD", "9000"))
                BMULT = float(os.environ.get("MHA_BMULT", "1.0"))

                def pop_fillers():
                    nonlocal popped, popped_dma
                    while popped < budget and fillers:
                        elapsed = max(act_clock, qkpv_clock + popped)
                        # pull any leading dma items (keep the queue fed,
                        # but no more than LOOKAHEAD ahead of real time)
                        i = 0
                        progress = False
                        while i < len(fillers):
                            it = fillers[i]
                            if (it["kind"] == "dma"
                                    and popped_dma < elapsed + LOOKAHEAD):
                                fillers.pop(i)
                                it["fn"]()
                                popped_dma += it["dma"]
                                progress = True
                                continue
                            if it["kind"] != "dma":
                                break
                            i += 1
                        if not fillers or popped >= budget:
                            break
                        head = fillers[0]
                        if (head["kind"] != "dma"
                                and head["ready"] <= elapsed + SLACK):
                            fillers.pop(0)
                            head["fn"]()
                            popped += head["cost"]
                            popped_dma += head["dma"]
                            progress = True
                        elif head["kind"] != "dma":
                            # head blocked: pop a later independent item
                            # (oproj / q are reorderable; kv chains are not)
                            for j in range(1, min(len(fillers), 12)):
                                itj = fillers[j]
                                if (itj["kind"] != "dma"
                                        and itj.get("grp") in ("o", "q")
                                        and itj["ready"] <= elapsed + SLACK):
                                    fillers.pop(j)
                                    itj["fn"]()
                                    popped += itj["cost"]
                                    popped_dma += itj["dma"]
                                    progress = True
                                    break
                        if not progress:
                            break
                for h in range(NH):
                    if tcx >= 1:
                        i = 0
                        while i < len(fillers):
                            it = fillers[i]
                            if (it.get("grp") == "q" and it.get("bar") == tcx
                                    and it.get("qbar", 9) <= h):
                                fillers.pop(i)
                                it["fn"]()
                                popped += it["cost"]
                                popped_dma += it["dma"]
                            else:
                                i += 1
                    pend = []
                    with nc.named_scope("attn"):
                        pv = pvps.tile(
                            [128, 4, 256], F32, tag="pv", name="pv_ps"
                        )
                        for gg in range(ngroups):
                            if h == 0 and tcx >= 1 and gg == (
                                2 * tcx if causal else 0
                            ):
                                # force-drain this chunk's spilled K/V work:
                                # the next QK group reads the new tiles
                                i = 0
                                while i < len(fillers):
                                    if fillers[i].get("bar") == tcx:
                                        it = fillers.pop(i)
                                        it["fn"]()
                                        popped += it["cost"]
                                        popped_dma += it["dma"]
                                    else:
                                        i += 1
                            qk = qkps.tile(
                                [128, GW, TC], F32, tag="qk", name="qk_ps"
                            )
                            rels = []
                            for b in range(GW):
                                Tt = GW * gg + b
                                rel = Tt - 4 * tcx if causal else -1
                                rels.append(rel)
                                c0 = 128 * rel if rel > 0 else 0
                                nc.tensor.matmul(
                                    qk[:, b, c0:TC],
                                    kT_rope[:, Tt * 128 : (Tt + 1) * 128],
                                    qrope_cur[:, h, c0:TC],
                                    start=True,
                                    stop=True,
                                )
                            # tanh in place in PSUM, then exp to bf16 SBUF;
                            # soft-capping scales fused into ACT. Columns
                            # below the causal diagonal are skipped.
                            pt = pa.tile(
                                [128, GW, TC], BF16, tag="pt", bufs=int(os.environ.get("MHA_PTBUFS", "4")),
                                name="ptile",
                            )
                            act_cols = 0
                            if max(rels) <= 0:
                                nc.scalar.activation(
                                    out=qk, in_=qk, func=Tanh,
                                    scale=ATTN_MULT / CAP,
                                )
                                nc.scalar.activation(
                                    out=pt, in_=qk, func=Exp, scale=CAP
                                )
                                act_cols = GW * TC
                            else:
                                for b in range(GW):
                                    c0 = 128 * max(rels[b], 0)
                                    nc.scalar.activation(
                                        out=qk[:, b, c0:TC],
                                        in_=qk[:, b, c0:TC],
                                        func=Tanh, scale=ATTN_MULT / CAP,
                                    )
                                    nc.scalar.activation(
                                        out=pt[:, b, c0:TC],
                                        in_=qk[:, b, c0:TC],
                                        func=Exp, scale=CAP,
                                    )
                                    act_cols += TC - c0
                            for b in range(GW):
                                rel = rels[b]
                                if 0 <= rel < 4:
                                    # triangular mask on the diagonal block
                                    nc.gpsimd.tensor_mul(
                                        pt[:, b, rel * 128 : (rel + 1) * 128],
                                        pt[:, b, rel * 128 : (rel + 1) * 128],
                                        tri_sb,
                                    )
                            # software-pipelined PV: emit the PREVIOUS
                            # group's PV now, so it reaches PE well after
                            # its exp() finished on ACT (the current QK +
                            # fillers cover the ACT latency).
                            def emit_pv(p_pt, p_rels, p_gg):
                                n_pv = 0
                                for s4 in range(4):
                                    for b in range(GW):
                                        Tt = GW * p_gg + b
                                        rel = p_rels[b]
                                        if causal and rel > s4:
                                            continue
                                        n_pv += 1
                                        nc.tensor.matmul(
                                            pv[:, s4, 0:129],
                                            p_pt[:, b, s4 * 128 : (s4 + 1) * 128],
                                            vaug[:, Tt, 0:129],
                                            start=(
                                                p_gg == 0 and b == 0
                                                and s4 % 2 == 0
                                            ),
                                            stop=(
                                                (Tt == 4 * tcx + s4)
                                                if causal
                                                else (p_gg == ngroups - 1
                                                      and b == GW - 1)
                                            ),
                                            skip_group_check=True,
                                        )
                                return n_pv

                            n_pv = 0
                            pend.append((pt, rels, gg))
                            if len(pend) > PIPED:
                                n_pv = emit_pv(*pend.pop(0))
                            # weave fillers so PE stays busy under ACT
                            act_ns = act_cols * 2 * 0.833 + (
                                330 if max(rels) <= 0 else 660
                            )
                            qkpv_ns = (act_cols + 129 * n_pv) * 0.4167
                            act_clock += act_ns
                            qkpv_clock += qkpv_ns
                            budget += BMULT * max(act_ns - qkpv_ns, 0.0)
                            pop_fillers()
                        while pend:
                            emit_pv(*pend.pop(0))
                    with nc.named_scope("attn_fin"):
                        ans = []
                        for s4 in range(4):
                            rc = pa.tile(
                                [128, 1], F32, tag="rc", bufs=4, name="rc"
                            )
                            nc.vector.reciprocal(rc, pv[:, s4, 128:129])
                            an = pa.tile(
                                [128, 128], BF16, tag="an", bufs=4, name="an"
                            )
                            nc.vector.tensor_scalar_mul(an, pv[:, s4, 0:128], rc)
                            ans.append(an)
                        # cover the DVE normalize latency with a filler
                        budget += 700
                        act_clock += 700
                        pop_fillers()
                        if FINFILL:
                            # transposes aren't needed until next chunk's
                            # O-proj: queue them as fillers instead of
                            # serializing at the head boundary
                            def fin_tr(ans=ans, h=h):
                                for s4 in range(4):
                                    tp = mmps.tile(
                                        [128, TC], BF16, tag="mm", name="atr"
                                    )
                                    nc.tensor.transpose(
                                        tp[:, :128], ans[s4], identb_sb
                                    )
                                    nc.vector.tensor_copy(
                                        attnT[:, h, s4 * 128 : (s4 + 1) * 128],
                                        tp[:, :128],
                                    )
                            fillers.insert(0, {
                                "cost": 900, "fn": fin_tr, "dma": 0,
                                "kind": "oproj", "grp": "pre", "ready": 0.0,
                            })
                        else:
                            for s4 in range(4):
                                tp = mmps.tile(
                                    [128, TC], BF16, tag="mm", name="atr"
                                )
                                nc.tensor.transpose(
                                    tp[:, :128], ans[s4], identb_sb
                                )
                                nc.vector.tensor_copy(
                                    attnT[:, h, s4 * 128 : (s4 + 1) * 128],
                                    tp[:, :128],
                                )
                    if SPLIT_O and tcx == NTC - 1 and h == 1:
                        for it in make_oproj_fillers(
                            attnT, t0, jhs=[0, 1], dest=outa, dest_t0=0
                        ):
                            it["ready"] = 0.0
                            fillers.append(it)
                # drain: 'pre' items (q proj/rope of tc+1) must finish
                # before attn(tcx+1) emits its first QK; K/V items of tc+1
                # spill into attn(tcx+1) (barrier at group 2*(tcx+1)), and
                # up to MHA_CARRY oproj items carry over (attnT bufs=3).
                carry = []
                rest = fillers
                if tcx + 1 < NTC:
                    cap = int(os.environ.get("MHA_CARRY", "16"))
                    o_total = sum(1 for it in rest if it["grp"] == "o")
                    drain_o = max(0, o_total - cap)
                    drain = []
                    for it in rest:
                        if it["grp"] == "pre":
                            drain.append(it)
                        elif it["grp"] == "q":
                            if it["qbar"] == 0 or not QSPILL:
                                drain.append(it)
                            else:
                                kv_carry.append(it)
                        elif it["grp"] == "o" and drain_o > 0:
                            drain.append(it)
                            drain_o -= 1
                        elif it["grp"] == "kv" and KVSPILL:
                            kv_carry.append(it)
                        elif it["grp"] == "kv":
                            drain.append(it)
                        else:
                            carry.append(it)
                    rest = drain
                # drain with the same dma-forwarding discipline: keep
                # transfers ~LOOKAHEAD ahead of the estimated PE clock so
                # in-order compute items rarely wait on arrival.
                if not DRAIN2:
                    for it in rest:
                        it["fn"]()
                    rest = []
                el = max(act_clock, qkpv_clock + popped)
                dma_el = popped_dma
                while rest:
                    i = 0
                    while i < len(rest):
                        if (rest[i]["kind"] == "dma"
                                and dma_el < el + LOOKAHEAD):
                            it = rest.pop(i)
                            it["fn"]()
                            dma_el += it["dma"]
                            continue
                        if rest[i]["kind"] != "dma":
                            break
                        i += 1
                    if not rest:
                        break
                    it = rest.pop(0)
                    it["fn"]()
                    el = max(el, it.get("ready", 0.0)) + it["cost"]
                    dma_el += it["dma"]
                qrope_cur = qrope_next
                prev_attnT, prev_t0 = attnT, t0

            # tail: O proj pass B of the last chunk (host adds outa+outb)
            tail_items = (
                make_oproj_fillers(prev_attnT, prev_t0, jhs=[2, 3],
                                   dest=outb, dest_t0=0, evac_alt=True)
                if SPLIT_O
                else make_oproj_fillers(prev_attnT, prev_t0, evac_alt=True,
                                        use_qkps=True)
            )
            for it in carry + tail_items:
                it["fn"]()

    nc.compile()
    return nc


def vbgd_dst(vaug):
    return vaug[:, :, 128:132]


def _host_constants(T: int):
    d = KEY_SIZE
    inv_freq = 1.0 / (10000.0 ** (np.arange(0, d, 2, dtype=np.float64) / d))  # [64]
    pos = np.arange(T, dtype=np.float64)
    phase_half = pos[None, :] * inv_freq[:, None]  # [64, T]
    phase = np.concatenate([phase_half, phase_half], axis=0)  # [128, T] (tiled)
    cosT = np.cos(phase).astype(np.float32)
    sinT = np.sin(phase).astype(np.float32)

    R = np.zeros((128, 128), dtype=np.float32)
    R[:64, 64:] = -np.eye(64, dtype=np.float32)
    R[64:, :64] = np.eye(64, dtype=np.float32)
    rot = np.ascontiguousarray(R.T)

    ident = np.eye(128, dtype=np.float32)

    # tri[k, c] = 1 if k <= c (valid: query col >= key row inside the
    # diagonal 128x128 block)
    tri = (np.arange(128)[:, None] <= np.arange(128)[None, :]).astype(
        ml_dtypes.bfloat16
    )

    NTT = T // 128
    vbg = np.zeros((128, NTT, 4), dtype=ml_dtypes.bfloat16)
    vbg[:, :, 0] = 1.0
    return cosT, sinT, rot, ident, tri, vbg


_NC_CACHE: dict = {}
LAST_RESULT = None
_LAST_IN_MAPS = None


def kernel(query, key, value, mask, Wq, Wk, Wv, Wo):
    global LAST_RESULT, _LAST_IN_MAPS
    query = np.asarray(query)
    key = np.asarray(key)
    value = np.asarray(value)
    mask = np.asarray(mask)
    Wq = np.asarray(Wq, dtype=np.float32)
    Wk = np.asarray(Wk, dtype=np.float32)
    Wv = np.asarray(Wv, dtype=np.float32)
    Wo = np.asarray(Wo, dtype=np.float32)

    b, T, D = query.shape
    assert b == 1 and D == D_MODEL, (b, D)

    m2 = np.asarray(mask).reshape(T, T).astype(bool)
    if np.array_equal(m2, np.tril(np.ones((T, T), dtype=bool))):
        causal = True
    elif m2.all():
        causal = False
    else:
        raise ValueError("unsupported mask pattern (expected causal or full)")

    kkey = (T, causal)
    if kkey not in _NC_CACHE:
        _NC_CACHE[kkey] = build_nc(T, causal)
    nc = _NC_CACHE[kkey]

    pnp = ml_dtypes.bfloat16
    xq = np.ascontiguousarray(query[0].T).astype(pnp)  # [D, T]
    xk = np.ascontiguousarray(key[0].T).astype(pnp)
    xv = np.ascontiguousarray(value[0].T).astype(pnp)
    cosT, sinT, rot, ident, tri, vbg = _host_constants(T)

    JW = NH * KEY_SIZE
    NDT = D // 128

    def pack_w(w, nh):
        # [D, nh*128] -> [k, jh, n, j] flattened per-partition-contiguous
        a = np.ascontiguousarray(w).astype(pnp)
        a = a.reshape(NDT, 128, nh, 128).transpose(1, 2, 0, 3)
        return np.ascontiguousarray(a.reshape(128, nh * NDT * 128))

    in_maps = []
    for c in range(N_CORES):
        in_maps.append(
            {
                "xq": xq,
                "xk": xk,
                "xv": xv,
                "wq": pack_w(Wq[:, c * JW : (c + 1) * JW], NH),
                "wk": pack_w(Wk[:, c * KEY_SIZE : (c + 1) * KEY_SIZE], 1),
                "wv": pack_w(Wv[:, c * KEY_SIZE : (c + 1) * KEY_SIZE], 1),
                "wo": np.ascontiguousarray(Wo[c * JW : (c + 1) * JW, :]).astype(pnp),
                "cosT": cosT.astype(pnp),
                "sinT": sinT.astype(pnp),
                "rot": rot.astype(pnp),
                "identb": ident.astype(pnp),
                "tri": tri,
            }
        )

    _LAST_IN_MAPS = in_maps
    trace = os.environ.get("MHA_TRACE") == "1"
    res = run_bass_kernel_spmd(nc, in_maps, list(range(N_CORES)), trace=trace)
    LAST_RESULT = res

    out = np.zeros((T, D), dtype=np.float64)
    for c in range(N_CORES):
        out += res.results[c]["out"].astype(np.float64)
    return out.astype(np.float32).reshape(1, T, D)


# revision 48
# speedup vs baseline: 1.2660x; 1.0056x over previous
"""Trainium2 Bass kernel for nn_MultiHeadAttention_83056077570808.

GQA multi-head attention (32 q heads, 8 kv heads, d_head=128, T=2048,
D=4096) with RoPE, tanh soft-capping at 30, causal mask, fp32 reference.

Sharding: tensor-parallel over heads across 8 cores. Core c owns kv head c
and q heads 4c..4c+3: Wq/Wk/Wv column-sharded, Wo row-sharded; activations
replicated. Each core computes a partial output (its heads' contribution
through its Wo rows); the host sums the 8 partials.

Fully streamed schedule: causality means attention chunk tcx only needs
K/V tiles 0..4*tcx+3, so K/V/Q projections for chunk tcx+1 run *during*
attention of chunk tcx as filler work woven between QK groups (covering
the ACT-engine tanh/exp latency); O-proj of chunk tcx-1 likewise. DMA is
spread across the whole timeline instead of front-loaded. Fillers are
paced by an explicit cost model (popping too fast blocks the in-order PE
stream on un-arrived slab DMAs; too slow starves PE under ACT).

Causal diagonal trim: for key tile Tt in the diagonal block of chunk tcx
(rel = Tt-4*tcx in 0..3), query columns < 128*rel are entirely masked, so
QK / tanh / exp are column-trimmed, only the [128,128] diagonal block is
tri-masked (Pool), and PV skips s4-blocks with s4 < rel.

All matmuls are bf16; PSUM accumulation fp32; rope arithmetic fp32.

PSUM bank rule in the PV accumulation: start=True clears has_written for
the WHOLE bank and two s-chains share each bank, so only the bank's first
chain issues start=True; the sibling chain's first write lands on cleared
bits and overwrites.
"""

import os
import sys

for _p in ("/opt/trn_rl_repo", os.path.expanduser("~/.axon_site/_ro/trn_rl_repo")):
    if os.path.isdir(_p) and _p not in sys.path:
        sys.path.insert(0, _p)

import numpy as np
import ml_dtypes

import concourse.bass as bass
import concourse.tile as tile
from concourse import bacc, mybir
from concourse.bass_utils import run_bass_kernel_spmd

F32 = mybir.dt.float32
BF16 = mybir.dt.bfloat16

D_MODEL = 4096
KEY_SIZE = 128
NUM_Q_HEADS = 32
NUM_KV_HEADS = 8
N_CORES = 8
NH = NUM_Q_HEADS // NUM_KV_HEADS  # q heads per core = 4
ATTN_MULT = 0.08838834764831845
CAP = 30.0

Tanh = mybir.ActivationFunctionType.Tanh
Exp = mybir.ActivationFunctionType.Exp


def build_nc(T: int, causal: bool):
    """Emit the Bass program for one core (SPMD: all cores run this).

    Tile builds a STATIC per-engine schedule in (priority = emission)
    order, so overlap must be engineered in the emission order itself.
    """
    D = D_MODEL
    TC = 512                 # t-chunk width
    NTC = T // TC            # t-chunks
    NTT = T // 128           # 128-tiles along T (key side)
    NDT = D // 128           # contraction tiles over d_model = 32
    JW = NH * KEY_SIZE       # per-core q/o width = 512
    GW = 2                   # key tiles per QK group (1 PSUM bank each)

    SPLIT_O = os.environ.get("MHA_SPLIT_O", "0") == "1"
    EVAC_ALT = os.environ.get("MHA_EVAC_ALT", "0") == "1"
    PROJ_ORDER2 = os.environ.get("MHA_PORDER2", "0") == "1"
    DRAIN2 = os.environ.get("MHA_DRAIN2", "0") == "1"
    DEFER_O = os.environ.get("MHA_DEFER_O", "0") == "1"
    KVSPILL = os.environ.get("MHA_KVSPILL", "1") == "1"
    QSPILL = os.environ.get("MHA_QSPILL", "0") == "1"
    PIPED = int(os.environ.get("MHA_PIPED", "3"))
    FINFILL = os.environ.get("MHA_FINFILL", "0") == "1"
    VLATE = os.environ.get("MHA_VLATE", "1") == "1"

    nc = bacc.Bacc(None, target_bir_lowering=False)

    xq = nc.dram_tensor("xq", [D, T], BF16, kind="ExternalInput")
    xk = nc.dram_tensor("xk", [D, T], BF16, kind="ExternalInput")
    xv = nc.dram_tensor("xv", [D, T], BF16, kind="ExternalInput")
    wq = nc.dram_tensor("wq", [128, NH * NDT * 128], BF16, kind="ExternalInput")
    wk = nc.dram_tensor("wk", [128, NDT * 128], BF16, kind="ExternalInput")
    wv = nc.dram_tensor("wv", [128, NDT * 128], BF16, kind="ExternalInput")
    wo = nc.dram_tensor("wo", [JW, D], BF16, kind="ExternalInput")
    cosd = nc.dram_tensor("cosT", [128, T], BF16, kind="ExternalInput")
    sind = nc.dram_tensor("sinT", [128, T], BF16, kind="ExternalInput")
    rotd = nc.dram_tensor("rot", [128, 128], BF16, kind="ExternalInput")
    identbd = nc.dram_tensor("identb", [128, 128], BF16, kind="ExternalInput")
    trid = nc.dram_tensor("tri", [128, 128], BF16, kind="ExternalInput")
    outd = nc.dram_tensor("out", [T, D], BF16, kind="ExternalOutput")
    outa = nc.dram_tensor("outa", [TC, D], BF16, kind="ExternalOutput")
    outb = nc.dram_tensor("outb", [TC, D], BF16, kind="ExternalOutput")

    with tile.TileContext(nc) as tc:
        with (
            tc.tile_pool(name="const", bufs=1) as constp,
            tc.tile_pool(name="persist", bufs=1) as persist,
            tc.tile_pool(name="slabs", bufs=2) as slabp,
            tc.tile_pool(name="tmps", bufs=2) as tmpp,
            tc.tile_pool(name="pa", bufs=2) as pa,
            tc.tile_pool(name="qkps", bufs=2, space="PSUM") as qkps,
            tc.tile_pool(name="pvps", bufs=1, space="PSUM") as pvps,
            tc.tile_pool(name="mmps", bufs=2, space="PSUM") as mmps,
        ):
            # ---- persistent SBUF ----
            rot_sb = constp.tile([128, 128], BF16)
            identb_sb = constp.tile([128, 128], BF16)
            tri_sb = constp.tile([128, 128], BF16)
            cos_sb = constp.tile([128, T], BF16)
            sin_sb = constp.tile([128, T], BF16)
            kT_rope = persist.tile([128, T], BF16)
            vaug = persist.tile([128, NTT, 132], BF16)
            wq_sb = persist.tile([128, NH, NDT, 128], BF16)
            wk_sb = persist.tile([128, NDT, 128], BF16)
            wv_sb = persist.tile([128, NDT, 128], BF16)
            wo_sb = persist.tile([128, NH, D], BF16)

            # ---- tiny consts first (clears the DMA queue fast) ----
            nc.sync.dma_start(out=rot_sb, in_=rotd[:])
            nc.sync.dma_start(out=identb_sb, in_=identbd[:])
            nc.sync.dma_start(out=tri_sb, in_=trid[:])
            nc.any.memset(vaug[:, :, 128:132], 1.0)

            # ---------------- emit-helper closures ----------------
            # Filler items are (cost_ns, fn) pairs.

            def kv_fillers(xsrc, w_sb, tch, dst_cb):
                """K or V projection of t-columns [tch*512,(tch+1)*512)."""
                st = {}

                def dma_i(i, half):
                    def f():
                        if half == 0:
                            st[i] = slabp.tile(
                                [128, 8, TC], BF16, tag="kvslab", bufs=3,
                                name="kvslab",
                            )
                        nc.sync.dma_start(
                            out=st[i][:, 4 * half : 4 * half + 4, :],
                            in_=xsrc[
                                i * 1024 + half * 512 :
                                i * 1024 + (half + 1) * 512,
                                tch * TC : (tch + 1) * TC,
                            ].rearrange("(n k) t -> k n t", k=128),
                        )
                    return {"cost": 100, "fn": f, "dma": 1456, "kind": "dma",
                            "grp": "kv", "bar": tch}

                def comp_i(i):
                    def f():
                        if i == 0:
                            st["ps"] = mmps.tile(
                                [128, TC], F32, tag="mm", name="kv_ps"
                            )
                        ps = st["ps"]
                        for j in range(8):
                            nc.tensor.matmul(
                                ps,
                                w_sb[:, i * 8 + j, :],
                                st[i][:, j, :],
                                start=(i == 0 and j == 0),
                                stop=(i == 3 and j == 7),
                            )
                        if i == 3:
                            dst_cb(ps)
                    return {"cost": 1710, "fn": f, "dma": 0, "kind": "comp",
                            "grp": "kv", "bar": tch}

                return [dma_i(0, 0), dma_i(0, 1), dma_i(1, 0), comp_i(0),
                        dma_i(1, 1), dma_i(2, 0), comp_i(1), dma_i(2, 1),
                        dma_i(3, 0), comp_i(2), dma_i(3, 1), comp_i(3)]

            def rope(dst, src, t0, tw):
                """dst[128, tw] = RoPE(src[128, tw]) at positions t0.. (fp32
                math; src/dst bf16)."""
                rp = mmps.tile([128, TC], F32, tag="mm", name="rope_ps")
                nc.tensor.matmul(rp[:, :tw], rot_sb, src, start=True, stop=True)
                t1 = pa.tile([128, TC], F32, tag="rt1", bufs=1, name="rope_t1")
                nc.gpsimd.tensor_mul(t1[:, :tw], src, cos_sb[:, t0 : t0 + tw])
                t2 = pa.tile([128, TC], F32, tag="rt2", bufs=1, name="rope_t2")
                nc.vector.tensor_mul(t2[:, :tw], rp[:, :tw], sin_sb[:, t0 : t0 + tw])
                nc.vector.tensor_add(dst, t1[:, :tw], t2[:, :tw])

            def k_chunk_fillers(tch):
                ktmp = tmpp.tile([128, TC], BF16, tag="ktmp", name="ktmp")

                def evac(ps):
                    nc.vector.tensor_copy(ktmp, ps)

                items = kv_fillers(xk, wk_sb, tch, evac)

                def rope_k():
                    rope(kT_rope[:, tch * TC : (tch + 1) * TC], ktmp,
                         tch * TC, TC)

                return items + [{"cost": 350, "fn": rope_k, "dma": 0,
                                 "kind": "comp", "grp": "kv", "bar": tch}]

            def v_chunk_fillers(tch):
                vtmp = tmpp.tile([128, TC], BF16, tag="vtmp", name="vtmp")

                def evac(ps):
                    nc.vector.tensor_copy(vtmp, ps)

                items = kv_fillers(xv, wv_sb, tch, evac)

                def vtr(half):
                    def f():
                        for b2 in range(2):
                            b = 4 * tch + 2 * half + b2
                            tp = mmps.tile(
                                [128, TC], BF16, tag="mm", name="vtr_ps"
                            )
                            nc.tensor.transpose(
                                tp[:, :128],
                                vtmp[:, (2 * half + b2) * 128 :
                                     (2 * half + b2 + 1) * 128],
                                identb_sb,
                            )
                            nc.vector.tensor_copy(vaug[:, b, 0:128], tp[:, :128])
                    return {"cost": 220, "fn": f, "dma": 0, "kind": "comp",
                            "grp": "kv", "bar": tch}

                return items + [vtr(0), vtr(1)]

            def qslab_dma_fillers(tcx):
                slabs = []

                def dma_h(dh, q):
                    def f():
                        if q == 0:
                            slab = slabp.tile(
                                [128, 16, TC], BF16, tag="qslab", name="qslab"
                            )
                            slabs.append(slab)
                        slab = slabs[dh]
                        nc.sync.dma_start(
                            out=slab[:, 4 * q : 4 * q + 4, :],
                            in_=xq[
                                dh * 2048 + q * 512 : dh * 2048 + (q + 1) * 512,
                                tcx * TC : (tcx + 1) * TC,
                            ].rearrange("(n k) t -> k n t", k=128),
                        )
                    return {"cost": 100, "fn": f, "dma": 1456, "kind": "dma",
                            "grp": "pre"}

                return slabs, [dma_h(0, q) for q in range(4)] + [
                    dma_h(1, q) for q in range(4)
                ]

            def qproj_chain(slabs, qraw, jh):
                ps = mmps.tile([128, TC], F32, tag="mm", name="q_ps")
                for dh in range(2):
                    for i in range(16):
                        nc.tensor.matmul(
                            ps,
                            wq_sb[:, jh, dh * 16 + i, :],
                            slabs[dh][:, i, :],
                            start=(dh == 0 and i == 0),
                            stop=(dh == 1 and i == 15),
                        )
                nc.vector.tensor_copy(qraw[:, jh, :], ps)

            def q_chunk_fillers(tcx, slabs):
                """Q proj + rope for chunk tcx; returns (qrope, items)."""
                qraw = tmpp.tile([128, NH, TC], BF16, tag="qraw", bufs=1, name="qraw")
                qrope = tmpp.tile([128, NH, TC], BF16, tag="qrope", bufs=2, name="qrope")
                items = []
                for jh in range(NH):
                    items.append({
                        "cost": 6830, "dma": 0, "kind": "comp", "grp": "q",
                        "bar": tcx, "qbar": jh,
                        "fn": lambda jh=jh: qproj_chain(slabs, qraw, jh),
                    })
                for jh in range(NH):
                    items.append({
                        "cost": 350, "dma": 0, "kind": "comp", "grp": "q",
                        "bar": tcx, "qbar": jh,
                        "fn": lambda jh=jh: rope(
                            qrope[:, jh, :], qraw[:, jh, :], tcx * TC, TC
                        ),
                    })
                return qrope, items

            def make_oproj_fillers(attnT, t0, jhs=range(NH), dest=None,
                                   dest_t0=None, evac_alt=False,
                                   use_qkps=False):
                dest = outd if dest is None else dest
                dest_t0 = t0 if dest_t0 is None else dest_t0
                jhs = list(jhs)
                fillers = []
                for nch in range(D // TC):
                    for s4 in range(4):
                        def f(s4=s4, nch=nch):
                            with nc.named_scope("oproj"):
                                if use_qkps and (s4 + nch) % 2 == 0:
                                    # qk PSUM banks are idle in the tail:
                                    # alternate into them for a deeper
                                    # chain pipeline
                                    ps = qkps.tile(
                                        [128, GW, TC], F32, tag="qk",
                                        name="o_ps2",
                                    )[:, 0, :]
                                else:
                                    ps = mmps.tile(
                                        [128, TC], F32, tag="mm", name="o_ps"
                                    )
                                for x, jh in enumerate(jhs):
                                    nc.tensor.matmul(
                                        ps,
                                        attnT[:, jh, s4 * 128 : (s4 + 1) * 128],
                                        wo_sb[:, jh, nch * TC : (nch + 1) * TC],
                                        start=(x == 0),
                                        stop=(x == len(jhs) - 1),
                                    )
                                osb = pa.tile(
                                    [128, TC], BF16, tag="osb", bufs=6,
                                    name="osb",
                                )
                                if evac_alt and (s4 + nch) % 2 == 0:
                                    nc.scalar.copy(out=osb, in_=ps)
                                else:
                                    nc.vector.tensor_copy(osb, ps)
                                nc.sync.dma_start(
                                    out=dest[
                                        dest_t0 + s4 * 128 :
                                        dest_t0 + (s4 + 1) * 128,
                                        nch * TC : (nch + 1) * TC,
                                    ],
                                    in_=osb,
                                )
                        fillers.append(
                            {"cost": 218 * len(jhs), "fn": f,
                             "dma": 364, "kind": "oproj", "grp": "o"}
                        )
                return fillers

            def interleave(a, b):
                out = []
                ia = ib = 0
                na, nb = len(a), len(b)
                while ia < na or ib < nb:
                    if ia * max(nb, 1) <= ib * max(na, 1) and ia < na:
                        out.append(a[ia]); ia += 1
                    elif ib < nb:
                        out.append(b[ib]); ib += 1
                    else:
                        out.append(a[ia]); ia += 1
                return out

            def proj_items_for(tcx):
                """All projection work for chunk tcx as a filler list, DMA
                items placed so transfers land just ahead of their use."""
                slabs_n, qdma = qslab_dma_fillers(tcx)
                kn = k_chunk_fillers(tcx)
                vn = v_chunk_fillers(tcx)
                qrope_n, qn = q_chunk_fillers(tcx, slabs_n)
                qpairs = [qn[0], qn[NH], qn[1], qn[NH + 1], qn[2],
                          qn[NH + 2], qn[3], qn[NH + 3]]
                if VLATE:
                    # v-stream last: its data isn't needed until group
                    # 2*tcx of the NEXT attention window (kv barrier), so
                    # keep the congested window's DMA queue for k/q
                    items = (
                        [kn[0], kn[1], qdma[0], qdma[1], kn[2], kn[3],
                         qdma[2], qdma[3], kn[4], kn[5], qdma[4], qdma[5],
                         kn[6], kn[7], qdma[6], qdma[7], kn[8], kn[9],
                         kn[10], kn[11], kn[12]]
                        + qpairs
                        + vn[:12] + [vn[12], vn[13]]
                    )
                else:
                    items = (
                        [kn[0], kn[1], qdma[0], qdma[1], kn[2], kn[3],
                         qdma[2], qdma[3], kn[4], kn[5], qdma[4], qdma[5],
                         kn[6], kn[7], qdma[6], qdma[7], kn[8], kn[9],
                         kn[10], kn[11], kn[12]]
                        + vn[:12] + [vn[12], vn[13]]
                        + qpairs
                    )
                return qrope_n, items

            # ---------------- chunk 0 prologue (inline, DMA-ordered) ----
            # Critical path to the first q chain: wq head 0 + both qslabs;
            # everything else (k/v slabs, cos/sin) streams behind and PE
            # picks it up between/after the q chains.
            def wq_head_dma(jh):
                nc.sync.dma_start(
                    out=wq_sb[:, jh, :, :],
                    in_=wq[:, jh * NDT * 128 : (jh + 1) * NDT * 128].rearrange(
                        "k (n j) -> k n j", j=128
                    ),
                )

            k0 = k_chunk_fillers(0)
            v0 = v_chunk_fillers(0)
            qslabs0, qdma0 = qslab_dma_fillers(0)
            wq_head_dma(0)
            for it in qdma0:                   # 8 quarter-slab dmas
                it["fn"]()
            wq_head_dma(1)
            nc.sync.dma_start(out=wk_sb, in_=wk.rearrange("k (n j) -> k n j", j=128))
            wq_head_dma(2)
            for it in k0[0:3]:                 # kslab dmas
                it["fn"]()
            wq_head_dma(3)
            qrope0, q0 = q_chunk_fillers(0, qslabs0)
            q0[0]["fn"](); q0[1]["fn"]()       # qproj chains 0,1
            k0[4]["fn"](); k0[5]["fn"]()       # kslab dmas
            q0[2]["fn"]()                      # qproj chain 2
            k0[3]["fn"]()                      # comp k piece 0
            nc.sync.dma_start(out=cos_sb, in_=cosd[:])
            q0[3]["fn"]()                      # qproj chain 3
            nc.sync.dma_start(out=sin_sb, in_=sind[:])
            k0[7]["fn"](); k0[8]["fn"](); k0[10]["fn"]()   # kslab dmas
            k0[6]["fn"](); k0[9]["fn"](); k0[11]["fn"]()   # comp k 1-3 + evac
            nc.sync.dma_start(out=wv_sb, in_=wv.rearrange("k (n j) -> k n j", j=128))
            k0[12]["fn"]()                     # rope-k(0)
            for it in q0[NH:]:                 # 4 rope-q(0)
                it["fn"]()
            v0[0]["fn"](); v0[1]["fn"](); v0[2]["fn"]()    # vslab dmas
            v0[3]["fn"]()                      # comp v piece 0
            v0[4]["fn"](); v0[5]["fn"]()       # vslab dmas
            v0[6]["fn"]()                      # comp v piece 1
            v0[7]["fn"](); v0[8]["fn"]()       # vslab dmas
            v0[9]["fn"]()                      # comp v piece 2
            v0[10]["fn"]()                     # vslab dma
            v0[11]["fn"]()                     # comp v piece 3 + evac
            v0[12]["fn"](); v0[13]["fn"]()     # vtr halves

            def wo_slice_dma(nch):
                def f():
                    nc.sync.dma_start(
                        out=wo_sb[:, :, nch * TC : (nch + 1) * TC],
                        in_=wo[:, nch * TC : (nch + 1) * TC].rearrange(
                            "(n k) d -> k n d", k=128
                        ),
                    )
                return {"cost": 100, "fn": f, "dma": 1456, "kind": "dma",
                        "grp": "pre"}

            # ---------------- main loop over t-chunks ----------------
            qrope_cur = qrope0
            prev_attnT = None
            prev_t0 = 0
            carry = []          # deferred oproj fillers from chunk tcx-1
            kv_carry = []       # K/V-proj fillers spilled into their own
                                # attention window (barrier at group 2*tcx)
            for tcx in range(NTC):
                t0 = tcx * TC

                if tcx + 1 < NTC:
                    qrope_next, proj_items = proj_items_for(tcx + 1)
                else:
                    qrope_next, proj_items = None, []
                # wo: first 2 slices during attn(0) (needed by the first
                # oproj pops early in attn(1)), the rest during attn(1)
                # where the DMA queue has slack.
                if tcx == 0:
                    wos = [wo_slice_dma(n) for n in range(D // TC)]
                    proj_items = interleave(proj_items, wos[:2])
                elif tcx == 1:
                    proj_items = interleave(proj_items, wos[2:])
                oproj_items = carry + (
                    make_oproj_fillers(prev_attnT, prev_t0)
                    if prev_attnT is not None
                    else []
                )
                if DEFER_O:
                    if tcx == 1:
                        deferred_o = oproj_items
                        oproj_items = []
                    elif tcx == 2:
                        oproj_items = deferred_o + oproj_items
                fillers = kv_carry + interleave(proj_items, oproj_items)
                kv_carry = []
                # annotate each compute item with the cumulative input-DMA
                # time that precedes it in this window's queue — popping it
                # earlier than that would head-of-line block the in-order
                # PE stream on an un-arrived transfer.
                cum_dma = 0.0
                for it in fillers:
                    if it["kind"] == "dma":
                        cum_dma += it["dma"]
                    if it["kind"] == "comp":
                        it["ready"] = cum_dma
                    elif it["kind"] == "oproj" and tcx == 1:
                        # wo slices still streaming in this window
                        it["ready"] = cum_dma
                    else:
                        it["ready"] = 0.0

                nt_valid = 4 * (tcx + 1) if causal else NTT
                ngroups = nt_valid // GW
                attnT = pa.tile(
                    [128, NH, TC], BF16, tag="attnT", bufs=3, name="attnT"
                )
                budget = 0.0
                popped = 0.0
                qkpv_clock = 0.0
                act_clock = 0.0
                popped_dma = 0.0
                SLACK = float(os.environ.get("MHA_SLACK", "3000"))
                LOOKAHEAD = float(os.environ.get("MHA_LOOKAHEAD", "9000"))
                BMULT = float(os.environ.get("MHA_BMULT", "1.0"))

                def pop_fillers():
                    nonlocal popped, popped_dma
                    while popped < budget and fillers:
                        elapsed = max(act_clock, qkpv_clock + popped)
                        # pull any leading dma items (keep the queue fed,
                        # but no more than LOOKAHEAD ahead of real time)
                        i = 0
                        progress = False
                        while i < len(fillers):
                            it = fillers[i]
                            if (it["kind"] == "dma"
                                    and popped_dma < elapsed + LOOKAHEAD):
                                fillers.pop(i)
                                it["fn"]()
                                popped_dma += it["dma"]
                                progress = True
                                continue
                            if it["kind"] != "dma":
                                break
                            i += 1
                        if not fillers or popped >= budget:
                            break
                        head = fillers[0]
                        if (head["kind"] != "dma"
                                and head["ready"] <= elapsed + SLACK):
                            fillers.pop(0)
                            head["fn"]()
                            popped += head["cost"]
                            popped_dma += head["dma"]
                            progress = True
                        elif head["kind"] != "dma":
                            # head blocked: pop a later independent item
                            # (oproj / q are reorderable; kv chains are not)
                            for j in range(1, min(len(fillers), 12)):
                                itj = fillers[j]
                                if (itj["kind"] != "dma"
                                        and itj.get("grp") in ("o", "q")
                                        and itj["ready"] <= elapsed + SLACK):
                                    fillers.pop(j)
                                    itj["fn"]()
                                    popped += itj["cost"]
                                    popped_dma += itj["dma"]
                                    progress = True
                                    break
                        if not progress:
                            break
                for h in range(NH):
                    if tcx >= 1:
                        i = 0
                        while i < len(fillers):
                            it = fillers[i]
                            if (it.get("grp") == "q" and it.get("bar") == tcx
                                    and it.get("qbar", 9) <= h):
                                fillers.pop(i)
                                it["fn"]()
                                popped += it["cost"]
                                popped_dma += it["dma"]
                            else:
                                i += 1
                    pend = []
                    with nc.named_scope("attn"):
                        pv = pvps.tile(
                            [128, 4, 256], F32, tag="pv", name="pv_ps"
                        )
                        for gg in range(ngroups):
                            if h == 0 and tcx >= 1 and gg == (
                                2 * tcx if causal else 0
                            ):
                                # force-drain this chunk's spilled K/V work:
                                # the next QK group reads the new tiles
                                i = 0
                                while i < len(fillers):
                                    if fillers[i].get("bar") == tcx:
                                        it = fillers.pop(i)
                                        it["fn"]()
                                        popped += it["cost"]
                                        popped_dma += it["dma"]
                                    else:
                                        i += 1
                            qk = qkps.tile(
                                [128, GW, TC], F32, tag="qk", name="qk_ps"
                            )
                            rels = []
                            for b in range(GW):
                                Tt = GW * gg + b
                                rel = Tt - 4 * tcx if causal else -1
                                rels.append(rel)
                                c0 = 128 * rel if rel > 0 else 0
                                nc.tensor.matmul(
                                    qk[:, b, c0:TC],
                                    kT_rope[:, Tt * 128 : (Tt + 1) * 128],
                                    qrope_cur[:, h, c0:TC],
                                    start=True,
                                    stop=True,
                                )
                            # tanh in place in PSUM, then exp to bf16 SBUF;
                            # soft-capping scales fused into ACT. Columns
                            # below the causal diagonal are skipped.
                            pt = pa.tile(
                                [128, GW, TC], BF16, tag="pt", bufs=int(os.environ.get("MHA_PTBUFS", "4")),
                                name="ptile",
                            )
                            act_cols = 0
                            if max(rels) <= 0:
                                nc.scalar.activation(
                                    out=qk, in_=qk, func=Tanh,
                                    scale=ATTN_MULT / CAP,
                                )
                                nc.scalar.activation(
                                    out=pt, in_=qk, func=Exp, scale=CAP
                                )
                                act_cols = GW * TC
                            else:
                                for b in range(GW):
                                    c0 = 128 * max(rels[b], 0)
                                    nc.scalar.activation(
                                        out=qk[:, b, c0:TC],
                                        in_=qk[:, b, c0:TC],
                                        func=Tanh, scale=ATTN_MULT / CAP,
                                    )
                                    nc.scalar.activation(
                                        out=pt[:, b, c0:TC],
                                        in_=qk[:, b, c0:TC],
                                        func=Exp, scale=CAP,
                                    )
                                    act_cols += TC - c0
                            for b in range(GW):
                                rel = rels[b]
                                if 0 <= rel < 4:
                                    # triangular mask on the diagonal block
                                    nc.gpsimd.tensor_mul(
                                        pt[:, b, rel * 128 : (rel + 1) * 128],
                                        pt[:, b, rel * 128 : (rel + 1) * 128],
                                        tri_sb,
                                    )
                            # software-pipelined PV: emit the PREVIOUS
                            # group's PV now, so it reaches PE well after
                            # its exp() finished on ACT (the current QK +
                            # fillers cover the ACT latency).
                            def emit_pv(p_pt, p_rels, p_gg):
                                n_pv = 0
                                for s4 in range(4):
                                    for b in range(GW):
                                        Tt = GW * p_gg + b
                                        rel = p_rels[b]
                                        if causal and rel > s4:
                                            continue
                                        n_pv += 1
                                        nc.tensor.matmul(
                                            pv[:, s4, 0:129],
                                            p_pt[:, b, s4 * 128 : (s4 + 1) * 128],
                                            vaug[:, Tt, 0:129],
                                            start=(
                                                p_gg == 0 and b == 0
                                                and s4 % 2 == 0
                                            ),
                                            stop=(
                                                (Tt == 4 * tcx + s4)
                                                if causal
                                                else (p_gg == ngroups - 1
                                                      and b == GW - 1)
                                            ),
                                            skip_group_check=True,
                                        )
                                return n_pv

                            n_pv = 0
                            pend.append((pt, rels, gg))
                            if len(pend) > PIPED:
                                n_pv = emit_pv(*pend.pop(0))
                            # weave fillers so PE stays busy under ACT
                            act_ns = act_cols * 2 * 0.833 + (
                                330 if max(rels) <= 0 else 660
                            )
                            qkpv_ns = (act_cols + 129 * n_pv) * 0.4167
                            act_clock += act_ns
                            qkpv_clock += qkpv_ns
                            budget += BMULT * max(act_ns - qkpv_ns, 0.0)
                            pop_fillers()
                        while pend:
                            emit_pv(*pend.pop(0))
                    with nc.named_scope("attn_fin"):
                        ans = []
                        for s4 in range(4):
                            rc = pa.tile(
                                [128, 1], F32, tag="rc", bufs=4, name="rc"
                            )
                            nc.vector.reciprocal(rc, pv[:, s4, 128:129])
                            an = pa.tile(
                                [128, 128], BF16, tag="an", bufs=4, name="an"
                            )
                            nc.vector.tensor_scalar_mul(an, pv[:, s4, 0:128], rc)
                            ans.append(an)
                        # cover the DVE normalize latency with a filler
                        budget += 700
                        act_clock += 700
                        pop_fillers()
                        if FINFILL:
                            # transposes aren't needed until next chunk's
                            # O-proj: queue them as fillers instead of
                            # serializing at the head boundary
                            def fin_tr(ans=ans, h=h):
                                for s4 in range(4):
                                    tp = mmps.tile(
                                        [128, TC], BF16, tag="mm", name="atr"
                                    )
                                    nc.tensor.transpose(
                                        tp[:, :128], ans[s4], identb_sb
                                    )
                                    nc.vector.tensor_copy(
                                        attnT[:, h, s4 * 128 : (s4 + 1) * 128],
                                        tp[:, :128],
                                    )
                            fillers.insert(0, {
                                "cost": 900, "fn": fin_tr, "dma": 0,
                                "kind": "oproj", "grp": "pre", "ready": 0.0,
                            })
                        else:
                            for s4 in range(4):
                                tp = mmps.tile(
                                    [128, TC], BF16, tag="mm", name="atr"
                                )
                                nc.tensor.transpose(
                                    tp[:, :128], ans[s4], identb_sb
                                )
                                nc.vector.tensor_copy(
                                    attnT[:, h, s4 * 128 : (s4 + 1) * 128],
                                    tp[:, :128],
                                )
                    if SPLIT_O and tcx == NTC - 1 and h == 1:
                        for it in make_oproj_fillers(
                            attnT, t0, jhs=[0, 1], dest=outa, dest_t0=0
                        ):
                            it["ready"] = 0.0
                            fillers.append(it)
                # drain: 'pre' items (q proj/rope of tc+1) must finish
                # before attn(tcx+1) emits its first QK; K/V items of tc+1
                # spill into attn(tcx+1) (barrier at group 2*(tcx+1)), and
                # up to MHA_CARRY oproj items carry over (attnT bufs=3).
                carry = []
                rest = fillers
                if tcx + 1 < NTC:
                    cap = int(os.environ.get("MHA_CARRY", "16"))
                    o_total = sum(1 for it in rest if it["grp"] == "o")
                    drain_o = max(0, o_total - cap)
                    drain = []
                    for it in rest:
                        if it["grp"] == "pre":
                            drain.append(it)
                        elif it["grp"] == "q":
                            if it["qbar"] == 0 or not QSPILL:
                                drain.append(it)
                            else:
                                kv_carry.append(it)
                        elif it["grp"] == "o" and drain_o > 0:
                            drain.append(it)
                            drain_o -= 1
                        elif it["grp"] == "kv" and KVSPILL:
                            kv_carry.append(it)
                        elif it["grp"] == "kv":
                            drain.append(it)
                        else:
                            carry.append(it)
                    rest = drain
                # drain with the same dma-forwarding discipline: keep
                # transfers ~LOOKAHEAD ahead of the estimated PE clock so
                # in-order compute items rarely wait on arrival.
                if not DRAIN2:
                    for it in rest:
                        it["fn"]()
                    rest = []
                el = max(act_clock, qkpv_clock + popped)
                dma_el = popped_dma
                while rest:
                    i = 0
                    while i < len(rest):
                        if (rest[i]["kind"] == "dma"
                                and dma_el < el + LOOKAHEAD):
                            it = rest.pop(i)
                            it["fn"]()
                            dma_el += it["dma"]
                            continue
                        if rest[i]["kind"] != "dma":
                            break
                        i += 1
                    if not rest:
                        break
                    it = rest.pop(0)
                    it["fn"]()
                    el = max(el, it.get("ready", 0.0)) + it["cost"]
                    dma_el += it["dma"]
                qrope_cur = qrope_next
                prev_attnT, prev_t0 = attnT, t0

            # tail: O proj pass B of the last chunk (host adds outa+outb)
            tail_items = (
                make_oproj_fillers(prev_attnT, prev_t0, jhs=[2, 3],
                                   dest=outb, dest_t0=0, evac_alt=True)
                if SPLIT_O
                else make_oproj_fillers(prev_attnT, prev_t0, evac_alt=True,
                                        use_qkps=True)
            )
            for it in carry + tail_items:
                it["fn"]()

    nc.compile()
    return nc


def vbgd_dst(vaug):
    return vaug[:, :, 128:132]


def _host_constants(T: int):
    d = KEY_SIZE
    inv_freq = 1.0 / (10000.0 ** (np.arange(0, d, 2, dtype=np.float64) / d))  # [64]
    pos = np.arange(T, dtype=np.float64)
    phase_half = pos[None, :] * inv_freq[:, None]  # [64, T]
    phase = np.concatenate([phase_half, phase_half], axis=0)  # [128, T] (tiled)
    cosT = np.cos(phase).astype(np.float32)
    sinT = np.sin(phase).astype(np.float32)

    R = np.zeros((128, 128), dtype=np.float32)
    R[:64, 64:] = -np.eye(64, dtype=np.float32)
    R[64:, :64] = np.eye(64, dtype=np.float32)
    rot = np.ascontiguousarray(R.T)

    ident = np.eye(128, dtype=np.float32)

    # tri[k, c] = 1 if k <= c (valid: query col >= key row inside the
    # diagonal 128x128 block)
    tri = (np.arange(128)[:, None] <= np.arange(128)[None, :]).astype(
        ml_dtypes.bfloat16
    )

    NTT = T // 128
    vbg = np.zeros((128, NTT, 4), dtype=ml_dtypes.bfloat16)
    vbg[:, :, 0] = 1.0
    return cosT, sinT, rot, ident, tri, vbg


_NC_CACHE: dict = {}
LAST_RESULT = None
_LAST_IN_MAPS = None


def kernel(query, key, value, mask, Wq, Wk, Wv, Wo):
    global LAST_RESULT, _LAST_IN_MAPS
    query = np.asarray(query)
    key = np.asarray(key)
    value = np.asarray(value)
    mask = np.asarray(mask)
    Wq = np.asarray(Wq, dtype=np.float32)
    Wk = np.asarray(Wk, dtype=np.float32)
    Wv = np.asarray(Wv, dtype=np.float32)
    Wo = np.asarray(Wo, dtype=np.float32)

    b, T, D = query.shape
    assert b == 1 and D == D_MODEL, (b, D)

    m2 = np.asarray(mask).reshape(T, T).astype(bool)
    if np.array_equal(m2, np.tril(np.ones((T, T), dtype=bool))):
        causal = True
    elif m2.all():
        causal = False
    else:
        raise ValueError("unsupported mask pattern (expected causal or full)")

    kkey = (T, causal)
    if kkey not in _NC_CACHE:
        _NC_CACHE[kkey] = build_nc(T, causal)
    nc = _NC_CACHE[kkey]

    pnp = ml_dtypes.bfloat16
    xq = np.ascontiguousarray(query[0].T).astype(pnp)  # [D, T]
    xk = np.ascontiguousarray(key[0].T).astype(pnp)
    xv = np.ascontiguousarray(value[0].T).astype(pnp)
    cosT, sinT, rot, ident, tri, vbg = _host_constants(T)

    JW = NH * KEY_SIZE
    NDT = D // 128

    def pack_w(w, nh):
        # [D, nh*128] -> [k, jh, n, j] flattened per-partition-contiguous
        a = np.ascontiguousarray(w).astype(pnp)
        a = a.reshape(NDT, 128, nh, 128).transpose(1, 2, 0, 3)
        return np.ascontiguousarray(a.reshape(128, nh * NDT * 128))

    in_maps = []
    for c in range(N_CORES):
        in_maps.append(
            {
                "xq": xq,
                "xk": xk,
                "xv": xv,
                "wq": pack_w(Wq[:, c * JW : (c + 1) * JW], NH),
                "wk": pack_w(Wk[:, c * KEY_SIZE : (c + 1) * KEY_SIZE], 1),
                "wv": pack_w(Wv[:, c * KEY_SIZE : (c + 1) * KEY_SIZE], 1),
                "wo": np.ascontiguousarray(Wo[c * JW : (c + 1) * JW, :]).astype(pnp),
                "cosT": cosT.astype(pnp),
                "sinT": sinT.astype(pnp),
                "rot": rot.astype(pnp),
                "identb": ident.astype(pnp),
                "tri": tri,
            }
        )

    _LAST_IN_MAPS = in_maps
    trace = os.environ.get("MHA_TRACE") == "1"
    res = run_bass_kernel_spmd(nc, in_maps, list(range(N_CORES)), trace=trace)
    LAST_RESULT = res

    out = np.zeros((T, D), dtype=np.float64)
    for c in range(N_CORES):
        out += res.results[c]["out"].astype(np.float64)
    return out.astype(np.float32).reshape(1, T, D)
